# revision 20
# baseline (speedup 1.0000x reference)
"""AudioFrontend Trainium2 kernel: PDM -> CIC(f32 blk16-exact) -> FIR(int64) -> logmel.

Bit-exact replication of jax-CPU float32 cumsum (XLA ReduceWindowRewriter
base-16 blocked scans) through the chaotic CIC stages, exact int64 FIR via
12-bit limbs on gpsimd int32, then matmul STFT/mel/log.
Self-contained: hardcodes all shapes; host code only shards/gathers.
"""
import numpy as np

NCORE = 8
N_PDM = 60_480_000
PERCORE = N_PDM // NCORE          # 7,560,000
P = 125
FREE = PERCORE // P               # 60480
TILE_F = 4032                     # 63*64 = 16*252
NT = FREE // TILE_F               # 15
ROWS_T = TILE_F // 16             # 252
ROWS_P = FREE // 16               # 3780
T0_LOC = P * ROWS_P               # 472500
T0_GLOB = NCORE * T0_LOC          # 3780000
GF = T0_GLOB // P                 # 30240
GCH = 5040
NGC = GF // GCH                   # 6
T1R = GCH // 16                   # 315
T1N = T0_GLOB // 16               # 236250
T1PAD = 236256                    # 123*1920 + 96
T2N = T1PAD // 16                 # 14766
T2PAD = 14768
T3N = T2PAD // 16                 # 923
T3PAD = 928
T4N = T3PAD // 16                 # 58
T4PAD = 64
T5N = T4PAD // 16                 # 4
DECIM = 63
DEC_PC = PERCORE // DECIM         # 120000
DEC_PP = FREE // DECIM            # 960
DEC_T = TILE_F // DECIM           # 64
L = 15
CHALO = 19
NBH = 384
N_FFT = 512
HOP = 160
WIN_LEN = 400
NMEL = 40
FR_PC = 750
T_FRAMES = 1 + (N_PDM // DECIM - N_FFT) // HOP  # 5997
FH = 375
SAT = 9.223372036854775808e18


def _mel_fbanks_np():
    n_freqs = N_FFT // 2 + 1
    all_freqs = np.linspace(0.0, 16000 / 2, n_freqs)
    h2m = lambda f: 2595.0 * np.log10(1.0 + f / 700.0)
    m_pts = np.linspace(h2m(0.0), h2m(8000.0), NMEL + 2)
    f_pts = 700.0 * (10.0 ** (m_pts / 2595.0) - 1.0)
    f_diff = f_pts[1:] - f_pts[:-1]
    slopes = f_pts[None, :] - all_freqs[:, None]
    down = -slopes[:, :-2] / f_diff[:-1]
    up = slopes[:, 2:] / f_diff[1:]
    return np.maximum(0.0, np.minimum(down, up)).astype(np.float32)


_COMPILED = {}
_LAST_RES = None


def _build(taps_list, scale_int):
    import concourse.bass as bass
    import concourse.bacc as bacc
    import concourse.mybir as mybir
    import concourse.tile as tile

    dt = mybir.dt
    A = mybir.AluOpType
    ACTF = mybir.ActivationFunctionType

    nc = bacc.Bacc()
    pdm_in = nc.declare_dram_parameter("pdm", [P, FREE], dt.int32, isOutput=False)
    mask_in = nc.declare_dram_parameter("mask", [128, 5136], dt.float32, isOutput=False)
    cos_in = nc.declare_dram_parameter("cosm", [N_FFT, 257], dt.float32, isOutput=False)
    sin_in = nc.declare_dram_parameter("sinm", [N_FFT, 257], dt.float32, isOutput=False)
    fb_in = nc.declare_dram_parameter("fbm", [257, NMEL], dt.float32, isOutput=False)
    win_in = nc.declare_dram_parameter("winm", [128, 4], dt.float32, isOutput=False)
    f15_in = nc.declare_dram_parameter("f15", [1, L], dt.float32, isOutput=False)
    phi_in = nc.declare_dram_parameter("phiv", [1, 1], dt.int32, isOutput=False)
    r1_in = nc.declare_dram_parameter("r1v", [1, 1], dt.int32, isOutput=False)
    p16_in = nc.declare_dram_parameter("p16v", [1, 1], dt.int32, isOutput=False)
    r1m_in = nc.declare_dram_parameter("r1m1", [1, 1], dt.int32, isOutput=False)
    v0m_in = nc.declare_dram_parameter("v0m", [1, 1], dt.float32, isOutput=False)
    out_p = nc.declare_dram_parameter("out", [NMEL, FR_PC], dt.float32, isOutput=True)

    pA = nc.dram_tensor("pA", [P, FREE], dt.float32)
    pB = nc.dram_tensor("pB", [P, FREE], dt.float32)
    t0loc = nc.dram_tensor("t0loc", [1, T0_LOC], dt.float32)
    e0buf = nc.dram_tensor("e0buf", [1, 16 + T0_LOC + 16], dt.float32)
    h16i = nc.dram_tensor("h16i", [1, 32], dt.float32)
    h16o = nc.dram_tensor("h16o", [NCORE, 32], dt.float32)
    h16p = nc.dram_tensor("h16p", [NCORE + 2, 32], dt.float32)
    t1agi = nc.dram_tensor("t1agi", [1, 29532], dt.float32)
    t1ago = nc.dram_tensor("t1ago", [NCORE, 29532], dt.float32)
    vloc = nc.dram_tensor("vloc", [1, 1 + 472512], dt.float32)
    t1buf = nc.dram_tensor("t1buf", [1, T1PAD], dt.float32)
    t2buf = nc.dram_tensor("t2buf", [1, T2PAD], dt.float32)
    zbuf = nc.dram_tensor("zbuf", [1, 1 + T2PAD], dt.float32)
    zsbuf = nc.dram_tensor("zsbuf", [1, 1 + T1PAD], dt.float32)
    vbuf = nc.dram_tensor("vbuf", [1, 1 + T0_GLOB], dt.float32)
    decb = nc.dram_tensor("decb", [1, CHALO + DEC_PC], dt.float32)
    ybuf = nc.dram_tensor("ybuf", [1, DEC_PC + NBH], dt.float32)
    h19i = nc.dram_tensor("h19i", [1, CHALO], dt.float32)
    h19o = nc.dram_tensor("h19o", [NCORE, CHALO], dt.float32)
    h19p = nc.dram_tensor("h19p", [NCORE + 1, CHALO], dt.float32)
    hnbi = nc.dram_tensor("hnbi", [1, NBH], dt.float32)
    hnbo = nc.dram_tensor("hnbo", [NCORE, NBH], dt.float32)
    hnbp = nc.dram_tensor("hnbp", [NCORE + 1, NBH], dt.float32)
    taild = nc.dram_tensor("taild", [1, CHALO + NBH], dt.float32)
    t3d = nc.dram_tensor("t3d", [1, T3N], dt.float32)
    z3buf = nc.dram_tensor("z3buf", [1, 1 + T3PAD], dt.float32)

    RG = [list(range(NCORE))]
    PHIS = [(4 * c) % 16 for c in range(NCORE)]
    R1S = [(T0_LOC * c - PHIS[c]) // 16 for c in range(NCORE)]
    MCS = [(R1S[c + 1] - R1S[c]) if c + 1 < NCORE else (T1N - R1S[c])
           for c in range(NCORE)]
    taps = [int(t) for t in taps_list]
    assert (1 << 15) == int(scale_int)

    with tile.TileContext(nc) as tc:
        pid = nc.gpsimd.partition_id()

        # ============ scan stages ============
        with tc.tile_pool(name="persist", bufs=1) as pp:
            mask = pp.tile([128, 5136], dt.float32)
            nc.sync.dma_start(mask[:], mask_in[:])
            nc.vector.tensor_copy(mask[:1, :1], mask[:1, :1])
            t0sb = pp.tile([P, ROWS_P], dt.float32)
            carry0 = pp.tile([P, ROWS_P], dt.float32)
            decsb = pp.tile([P, DEC_PP], dt.float32)

            for st in range(5):
                src = [None, pA, pB, pA, pB][st]
                dst = [pA, pB, pA, pB, pA][st]
                with tc.tile_pool(name=f"s{st}", bufs=3) as sp:
                    for t in range(NT):
                        fs = slice(t * TILE_F, (t + 1) * TILE_F)
                        if st == 0:
                            raw = sp.tile([P, TILE_F], dt.int32, tag="raw")
                            nc.sync.dma_start(raw[:], pdm_in[:, fs])
                            xt = sp.tile([P, TILE_F], dt.float32, tag="xt")
                            nc.scalar.activation(xt[:], raw[:], ACTF.Copy,
                                                 bias=-1.0, scale=2.0)
                        else:
                            pin = sp.tile([P, TILE_F], dt.float32, tag="pin")
                            nc.sync.dma_start(pin[:], src[:, fs])
                            nc.vector.tensor_copy(pin[:1, :1], pin[:1, :1])
                            xt = sp.tile([P, TILE_F], dt.float32, tag="xt")
                            rs = slice(t * ROWS_T, (t + 1) * ROWS_T)
                            nc.vector.tensor_tensor(
                                xt[:].rearrange("p (r s) -> p r s", s=16),
                                pin[:].rearrange("p (r s) -> p r s", s=16),
                                carry0[:, rs].broadcast_to([P, ROWS_T, 16]),
                                A.add)
                        po = sp.tile([P, TILE_F], dt.float32, tag="po")
                        nc.vector.tensor_tensor_scan(
                            po[:], mask[:P, :TILE_F], xt[:], 0.0, A.mult, A.add)
                        nc.vector.tensor_copy(
                            t0sb[:, t * ROWS_T:(t + 1) * ROWS_T], po[:, 15::16])
                        nc.sync.dma_start(dst[:, fs], po[:])

                nc.sync.dma_start(
                    bass.AP(t0loc, 0, [[ROWS_P, P], [1, ROWS_P]]), t0sb[:])
                nc.sync.dma_start(h16i[0, :16], t0sb[:1, :16])
                nc.sync.dma_start(h16i[0, 16:], t0sb[P - 1:P, ROWS_P - 16:])
                nc.gpsimd.collective_compute(
                    "AllGather", A.bypass, replica_groups=RG,
                    ins=[h16i[:]], outs=[h16o[:]])
                # padded halo rows: row0=AG7, rows1..8=AG0..7, row9=AG0
                nc.sync.dma_start(h16p[0, :], h16o[NCORE - 1, :])
                nc.sync.dma_start(h16p[1:NCORE + 1, :], h16o[:])
                nc.sync.dma_start(h16p[NCORE + 1, :], h16o[0, :])
                # e0: [left16 halo][own t0][right16 halo]
                nc.gpsimd.dma_start(e0buf[0, :16], h16p[:][pid, 16:])
                nc.sync.dma_start(e0buf[0, 16:16 + T0_LOC], t0loc[0, :])
                nc.gpsimd.dma_start(
                    e0buf[0, 16 + T0_LOC:], h16p[:][pid + 2, :16])

                with tc.tile_pool(name=f"g{st}", bufs=1) as gp:
                    phir = nc.gpsimd.alloc_register(f"phir{st}")
                    nc.gpsimd.reg_load(phir, phi_in[:1, :1])
                    r1r = nc.gpsimd.alloc_register(f"r1r{st}")
                    nc.gpsimd.reg_load(r1r, r1_in[:1, :1])
                    p16r = nc.gpsimd.alloc_register(f"p16r{st}")
                    nc.gpsimd.reg_load(p16r, p16_in[:1, :1])
                    r1mr = nc.gpsimd.alloc_register(f"r1mr{st}")
                    nc.gpsimd.reg_load(r1mr, r1m_in[:1, :1])
                    # local extended-t0 scan: [92, 5136] from e0buf
                    ge = gp.tile([92, 5136], dt.float32, tag="ge")
                    nc.gpsimd.dma_start(
                        ge[:],
                        bass.AP(e0buf, bass.make_scalar_value(p16r),
                                [[5136, 92], [1, 5136]]))
                    nc.vector.tensor_copy(ge[:1, :1], ge[:1, :1])
                    gs = gp.tile([92, 5136], dt.float32, tag="gs")
                    nc.vector.tensor_tensor_scan(
                        gs[:], mask[:92, :5136], ge[:], 0.0, A.mult, A.add)
                    tx = gp.tile([92, 321], dt.float32, tag="tx")
                    nc.vector.tensor_copy(tx[:], gs[:, 15::16])
                    nc.sync.dma_start(
                        bass.AP(t1agi, 0, [[321, 92], [1, 321]]), tx[:])
                    nc.gpsimd.collective_compute(
                        "AllGather", A.bypass, replica_groups=RG,
                        ins=[t1agi[:]], outs=[t1ago[:]])
                    # compact ragged t1 contributions into t1buf
                    for c in range(NCORE):
                        nc.sync.dma_start(
                            t1buf[0, R1S[c]:R1S[c] + MCS[c]],
                            t1ago[c, :MCS[c]])
                    zt = gp.tile([1, 16], dt.float32, tag="zt")
                    nc.vector.memset(zt[:], 0.0)
                    nc.sync.dma_start(t1buf[0, T1N:T1PAD], zt[:1, :T1PAD - T1N])
                    nc.sync.dma_start(zbuf[0, :1], zt[:1, :1])
                    nc.sync.dma_start(zsbuf[0, :1], zt[:1, :1])

                    u1a = gp.tile([123, 1920], dt.float32, tag="u1a")
                    nc.sync.dma_start(
                        u1a[:], bass.AP(t1buf, 0, [[1920, 123], [1, 1920]]))
                    nc.vector.tensor_copy(u1a[:1, :1], u1a[:1, :1])
                    p2a = gp.tile([123, 1920], dt.float32, tag="p2a")
                    nc.vector.tensor_tensor_scan(
                        p2a[:], mask[:123, :1920], u1a[:], 0.0, A.mult, A.add)
                    u1b = gp.tile([1, 96], dt.float32, tag="u1b")
                    nc.sync.dma_start(u1b[:], t1buf[0, 123 * 1920:T1PAD])
                    nc.vector.tensor_copy(u1b[:1, :1], u1b[:1, :1])
                    p2b = gp.tile([1, 96], dt.float32, tag="p2b")
                    nc.vector.tensor_tensor_scan(
                        p2b[:], mask[:1, :96], u1b[:], 0.0, A.mult, A.add)
                    t2a = gp.tile([123, 120], dt.float32, tag="t2a")
                    nc.vector.tensor_copy(t2a[:], p2a[:, 15::16])
                    nc.sync.dma_start(
                        bass.AP(t2buf, 0, [[120, 123], [1, 120]]), t2a[:])
                    t2b = gp.tile([1, 6], dt.float32, tag="t2b")
                    nc.vector.tensor_copy(t2b[:], p2b[:, 15::16])
                    nc.sync.dma_start(t2buf[0, 123 * 120:T2N], t2b[:1, :])
                    nc.sync.dma_start(t2buf[0, T2N:T2PAD], zt[:1, :T2PAD - T2N])

                    u2 = gp.tile([13, 1136], dt.float32, tag="u2")
                    nc.sync.dma_start(
                        u2[:], bass.AP(t2buf, 0, [[1136, 13], [1, 1136]]))
                    nc.vector.tensor_copy(u2[:1, :1], u2[:1, :1])
                    p3 = gp.tile([13, 1136], dt.float32, tag="p3")
                    nc.vector.tensor_tensor_scan(
                        p3[:], mask[:13, :1136], u2[:], 0.0, A.mult, A.add)
                    t3x = gp.tile([13, 71], dt.float32, tag="t3x")
                    nc.vector.tensor_copy(t3x[:], p3[:, 15::16])
                    nc.sync.dma_start(
                        bass.AP(t3d, 0, [[71, 13], [1, 71]]), t3x[:])
                    u3 = gp.tile([1, T3PAD], dt.float32, tag="u3")
                    nc.vector.memset(u3[:], 0.0)
                    nc.sync.dma_start(u3[:1, :T3N], t3d[0, :])
                    nc.vector.tensor_copy(u3[:1, :1], u3[:1, :1])
                    p4 = gp.tile([1, T3PAD], dt.float32, tag="p4")
                    nc.vector.tensor_tensor_scan(
                        p4[:], mask[:1, :T3PAD], u3[:], 0.0, A.mult, A.add)
                    u4 = gp.tile([1, T4PAD], dt.float32, tag="u4")
                    nc.vector.memset(u4[:], 0.0)
                    nc.vector.tensor_copy(u4[:, :T4N], p4[:, 15::16])
                    p5 = gp.tile([1, T4PAD], dt.float32, tag="p5")
                    nc.vector.tensor_tensor_scan(
                        p5[:], mask[:1, :T4PAD], u4[:], 0.0, A.mult, A.add)
                    u5 = gp.tile([1, T5N], dt.float32, tag="u5")
                    nc.vector.tensor_copy(u5[:], p5[:, 15::16])
                    s5 = gp.tile([1, T5N], dt.float32, tag="s5")
                    nc.vector.tensor_tensor_scan(
                        s5[:], mask[:1, :T5N], u5[:], 0.0, A.mult, A.add)
                    nc.vector.tensor_tensor(
                        p5[:, 16:].rearrange("p (r s) -> p r s", s=16),
                        p5[:, 16:].rearrange("p (r s) -> p r s", s=16),
                        s5[:, :3].broadcast_to([1, 3, 16]), A.add)
                    nc.vector.tensor_tensor(
                        p4[:, 16:].rearrange("p (r s) -> p r s", s=16),
                        p4[:, 16:].rearrange("p (r s) -> p r s", s=16),
                        p5[:, :T4N - 1].broadcast_to([1, T4N - 1, 16]), A.add)
                    nc.sync.dma_start(z3buf[0, :1], zt[:1, :1])
                    nc.sync.dma_start(z3buf[0, 1:1 + T3PAD], p4[:1, :])
                    cz3 = gp.tile([13, 71], dt.float32, tag="t3x")
                    nc.sync.dma_start(
                        cz3[:], bass.AP(z3buf, 0, [[71, 13], [1, 71]]))
                    nc.vector.tensor_copy(cz3[:1, :1], cz3[:1, :1])
                    nc.vector.tensor_tensor(
                        p3[:].rearrange("p (r s) -> p r s", s=16),
                        p3[:].rearrange("p (r s) -> p r s", s=16),
                        cz3[:].broadcast_to([13, 71, 16]), A.add)
                    nc.sync.dma_start(
                        bass.AP(zbuf, 1, [[1136, 13], [1, 1136]]), p3[:])
                    cza = gp.tile([123, 120], dt.float32, tag="cza")
                    nc.sync.dma_start(
                        cza[:], bass.AP(zbuf, 0, [[120, 123], [1, 120]]))
                    nc.vector.tensor_copy(cza[:1, :1], cza[:1, :1])
                    nc.vector.tensor_tensor(
                        p2a[:].rearrange("p (r s) -> p r s", s=16),
                        p2a[:].rearrange("p (r s) -> p r s", s=16),
                        cza[:].broadcast_to([123, 120, 16]), A.add)
                    czb = gp.tile([1, 6], dt.float32, tag="czb")
                    nc.sync.dma_start(czb[:], zbuf[0, 123 * 120:123 * 120 + 6])
                    nc.vector.tensor_copy(czb[:1, :1], czb[:1, :1])
                    nc.vector.tensor_tensor(
                        p2b[:].rearrange("p (r s) -> p r s", s=16),
                        p2b[:].rearrange("p (r s) -> p r s", s=16),
                        czb[:].broadcast_to([1, 6, 16]), A.add)
                    nc.sync.dma_start(
                        bass.AP(zsbuf, 1, [[1920, 123], [1, 1920]]), p2a[:])
                    nc.sync.dma_start(
                        zsbuf[0, 1 + 123 * 1920:1 + T1PAD], p2b[:1, :])

                    # own scan_t0: p1_local + bcast(Zs at own rows)
                    ctb = gp.tile([92, 321], dt.float32, tag="tx")
                    nc.gpsimd.dma_start(
                        ctb[:],
                        bass.AP(zsbuf, bass.make_scalar_value(r1r),
                                [[321, 92], [1, 321]]))
                    nc.vector.tensor_copy(ctb[:1, :1], ctb[:1, :1])
                    nc.vector.tensor_tensor(
                        gs[:].rearrange("p (r s) -> p r s", s=16),
                        gs[:].rearrange("p (r s) -> p r s", s=16),
                        ctb[:].broadcast_to([92, 321, 16]), A.add)
                    nc.sync.dma_start(
                        bass.AP(vloc, 1, [[5136, 92], [1, 5136]]), gs[:])
                    # vloc[0]: 0 normally; for the phi==0 mid core (c=4):
                    # scan_t0[A_c - 1] = t1[A_c/16 - 1] + scan_t1[A_c/16 - 2]
                    sv = gp.tile([1, 2], dt.float32, tag="sv")
                    nc.gpsimd.dma_start(
                        sv[:1, :1],
                        bass.AP(t1buf, bass.make_scalar_value(r1mr),
                                [[1, 1], [1, 1]]))
                    nc.gpsimd.dma_start(
                        sv[:1, 1:],
                        bass.AP(zsbuf, bass.make_scalar_value(r1mr),
                                [[1, 1], [1, 1]]))
                    v0t = gp.tile([1, 1], dt.float32, tag="v0t")
                    nc.sync.dma_start(v0t[:], v0m_in[:])
                    nc.vector.tensor_copy(v0t[:1, :1], v0t[:1, :1])
                    sv2 = gp.tile([1, 1], dt.float32, tag="sv2")
                    nc.vector.tensor_tensor(sv2[:], sv[:1, :1], sv[:1, 1:],
                                            A.add)
                    nc.vector.tensor_tensor(sv2[:], sv2[:], v0t[:], A.mult)
                    nc.sync.dma_start(vloc[0, :1], sv2[:1, :])
                    nc.gpsimd.dma_start(
                        carry0[:],
                        bass.AP(vloc, bass.make_scalar_value(phir),
                                [[ROWS_P, P], [1, ROWS_P]]))
                    nc.vector.tensor_copy(carry0[:1, :1], carry0[:1, :1])

            # ============ phase B of stage 5: decimate ============
            with tc.tile_pool(name="pb5", bufs=3) as sp:
                for t in range(NT):
                    fs = slice(t * TILE_F, (t + 1) * TILE_F)
                    pin = sp.tile([P, TILE_F], dt.float32, tag="pin")
                    nc.sync.dma_start(pin[:], pA[:, fs])
                    nc.vector.tensor_copy(pin[:1, :1], pin[:1, :1])
                    xt = sp.tile([P, TILE_F], dt.float32, tag="xt")
                    rs = slice(t * ROWS_T, (t + 1) * ROWS_T)
                    nc.vector.tensor_tensor(
                        xt[:].rearrange("p (r s) -> p r s", s=16),
                        pin[:].rearrange("p (r s) -> p r s", s=16),
                        carry0[:, rs].broadcast_to([P, ROWS_T, 16]), A.add)
                    nc.vector.tensor_copy(
                        decsb[:, t * DEC_T:(t + 1) * DEC_T], xt[:, 0::DECIM])

            nc.sync.dma_start(
                bass.AP(decb, CHALO, [[DEC_PP, P], [1, DEC_PP]]), decsb[:])
            nc.sync.dma_start(h19i[:1, :], decsb[P - 1:P, DEC_PP - CHALO:])
            nc.gpsimd.collective_compute(
                "AllGather", A.bypass, replica_groups=RG,
                ins=[h19i[:]], outs=[h19o[:]])
            nc.sync.dma_start(hnbi[:1, :], decsb[:1, :NBH])
            nc.gpsimd.collective_compute(
                "AllGather", A.bypass, replica_groups=RG,
                ins=[hnbi[:]], outs=[hnbo[:]])
            with tc.tile_pool(name="hx", bufs=1) as hp:
                zt2 = hp.tile([1, NBH], dt.float32)
                nc.vector.memset(zt2[:], 0.0)
                nc.sync.dma_start(h19p[0, :], zt2[:1, :CHALO])
                nc.sync.dma_start(h19p[1:, :], h19o[:])
                nc.sync.dma_start(hnbp[:NCORE, :], hnbo[:])
                nc.sync.dma_start(hnbp[NCORE, :], hnbo[0, :])
                nc.gpsimd.dma_start(decb[0, :CHALO], h19p[:][pid, :])

        # ============ comb + FIR + y ============
        def emit_comb_fir(dp, src_ap, np_, width, ydst, ybase, yrstride):
            A_ = A
            NL = 16
            cmb = dp.tile([np_, width], dt.float32, tag="cmb0")
            nc.sync.dma_start(cmb[:], src_ap)
            nc.vector.tensor_copy(cmb[:1, :1], cmb[:1, :1])
            cur = cmb
            w = width
            for it in range(5):
                nxt = dp.tile([np_, w - 1], dt.float32, tag=f"cmb{1 + it % 2}")
                nc.vector.tensor_tensor(
                    nxt[:], cur[:, 1:w], cur[:, :w - 1], A_.subtract)
                cur = nxt
                w -= 1
            nw_full = w
            ny_full = nw_full - (L - 1)
            CW = 320 if np_ > 1 else ny_full
            nch = ny_full // CW
            assert nch * CW == ny_full
            for ch in range(nch):
                cw = cur[:, ch * CW:ch * CW + CW + (L - 1)]
                nw = CW + (L - 1)
                ny = CW
                satp = dp.tile([np_, nw], dt.float32, tag="satp")
                nc.vector.tensor_scalar(satp[:], cw, SAT, None, A_.is_ge)
                satn = dp.tile([np_, nw], dt.float32, tag="satn")
                nc.vector.tensor_scalar(satn[:], cw, -SAT, None, A_.is_le)
                sgn = dp.tile([np_, nw], dt.float32, tag="sgn")
                nc.vector.tensor_scalar(sgn[:], cw, 0.0, None, A_.is_ge)
                nc.vector.tensor_scalar(sgn[:], sgn[:], 2.0, -1.0, A_.mult,
                                        A_.add)
                mag = dp.tile([np_, nw], dt.float32, tag="mag")
                nc.vector.tensor_tensor(mag[:], cw, sgn[:], A_.mult)
                rs_ = dp.tile([np_, nw], dt.float32, tag="rs")
                nc.vector.tensor_tensor(rs_[:], satp[:], satn[:], A_.add)
                nc.vector.tensor_scalar(rs_[:], rs_[:], -1.0, 1.0, A_.mult,
                                        A_.add)
                nc.vector.tensor_tensor(rs_[:], rs_[:], sgn[:], A_.mult)
                bits = dp.tile([np_, nw], dt.int32, tag="bits")
                nc.vector.tensor_copy(bits[:], mag[:].bitcast(dt.int32))
                ex = dp.tile([np_, nw], dt.int32, tag="ex")
                nc.vector.tensor_scalar(ex[:], bits[:], 23, None,
                                        A_.logical_shift_right)
                nc.vector.tensor_scalar(ex[:], ex[:], 255, None, A_.bitwise_and)
                nc.vector.tensor_scalar(ex[:], ex[:], -150, None, A_.add)
                mant = dp.tile([np_, nw], dt.int32, tag="mant")
                nc.vector.tensor_scalar(mant[:], bits[:], 0x7FFFFF, 0x800000,
                                        A_.bitwise_and, A_.bitwise_or)
                nzm = dp.tile([np_, nw], dt.int32, tag="nzm")
                nc.vector.tensor_scalar(nzm[:], ex[:], -23, None, A_.is_ge)
                nc.vector.tensor_tensor(mant[:], mant[:], nzm[:], A_.mult)
                tmpa = dp.tile([np_, nw], dt.int32, tag="tmpa")
                tmpb = dp.tile([np_, nw], dt.int32, tag="tmpb")
                tmpr = dp.tile([np_, nw], dt.int32, tag="tmpr")
                tmps = dp.tile([np_, nw], dt.int32, tag="tmps")
                sel = dp.tile([np_, nw], dt.int32, tag="sel")
                limbs = []
                for j in range(NL):
                    nc.vector.tensor_scalar(tmpr[:], ex[:], -1, 4 * j, A_.mult,
                                            A_.add)
                    nc.vector.tensor_scalar(tmps[:], tmpr[:], 31, None, A_.min)
                    nc.vector.tensor_scalar(tmps[:], tmps[:], 0, None, A_.max)
                    nc.vector.tensor_tensor(tmpa[:], mant[:], tmps[:],
                                            A_.logical_shift_right)
                    nc.vector.tensor_scalar(tmpa[:], tmpa[:], 15, None,
                                            A_.bitwise_and)
                    nc.vector.tensor_scalar(tmps[:], tmpr[:], -1, 0, A_.mult,
                                            A_.max)
                    nc.vector.tensor_scalar(tmps[:], tmps[:], 3, None, A_.min)
                    nc.vector.tensor_tensor(tmpb[:], mant[:], tmps[:],
                                            A_.logical_shift_left)
                    nc.vector.tensor_scalar(tmpb[:], tmpb[:], 15, None,
                                            A_.bitwise_and)
                    nc.vector.tensor_scalar(sel[:], tmpr[:], 0, None, A_.is_ge)
                    li = dp.tile([np_, nw], dt.int32, tag="li")
                    nc.vector.select(li[:], sel[:], tmpa[:], tmpb[:])
                    lf = dp.tile([np_, nw], dt.float32, tag=f"lf{j}")
                    nc.vector.tensor_copy(lf[:], li[:])
                    nc.vector.tensor_tensor(lf[:], lf[:], rs_[:], A_.mult)
                    limbs.append(lf)
                accA = dp.tile([np_, ny], dt.float32, tag="accA")
                accB = dp.tile([np_, ny], dt.float32, tag="accB")
                for k in range(L):
                    o = L - 1 - k
                    if k == 0:
                        nc.vector.tensor_scalar(accA[:], satp[:, o:o + ny],
                                                float(taps[k]), None, A_.mult)
                        nc.vector.tensor_scalar(accB[:], satn[:, o:o + ny],
                                                float(taps[k]), None, A_.mult)
                    else:
                        nc.vector.scalar_tensor_tensor(
                            accA[:], satp[:, o:o + ny], float(taps[k]),
                            accA[:], A_.mult, A_.add)
                        nc.vector.scalar_tensor_tensor(
                            accB[:], satn[:, o:o + ny], float(taps[k]),
                            accB[:], A_.mult, A_.add)
                cols = []
                for j in range(NL):
                    cj = dp.tile([np_, ny], dt.float32, tag=f"c{j}")
                    nc.vector.tensor_scalar(cj[:],
                                            limbs[j][:, L - 1:L - 1 + ny],
                                            float(taps[0]), None, A_.mult)
                    for k in range(1, L):
                        o = L - 1 - k
                        nc.vector.scalar_tensor_tensor(
                            cj[:], limbs[j][:, o:o + ny], float(taps[k]),
                            cj[:], A_.mult, A_.add)
                    cols.append(cj)
                ai = dp.tile([np_, ny], dt.int32, tag="ai")
                nc.vector.tensor_copy(ai[:], accA[:])
                bi = dp.tile([np_, ny], dt.int32, tag="bi")
                nc.vector.tensor_copy(bi[:], accB[:])
                nc.vector.tensor_tensor(ai[:], ai[:], bi[:], A_.subtract)
                nc.vector.tensor_scalar(ai[:], ai[:], 1, 3, A_.bitwise_and,
                                        A_.logical_shift_left)
                parf = dp.tile([np_, ny], dt.float32, tag="parf")
                nc.vector.tensor_copy(parf[:], ai[:])
                nc.vector.tensor_tensor(cols[0][:], cols[0][:], accA[:],
                                        A_.subtract)
                nc.vector.tensor_tensor(cols[15][:], cols[15][:], parf[:],
                                        A_.add)
                carry = dp.tile([np_, ny], dt.int32, tag="cy")
                vj = dp.tile([np_, ny], dt.int32, tag="vj")
                lmb = [None] * NL
                for j in range(NL):
                    nc.vector.tensor_copy(vj[:], cols[j][:])
                    if j > 0:
                        nc.vector.tensor_tensor(vj[:], vj[:], carry[:], A_.add)
                    if j < NL - 1:
                        nc.vector.tensor_scalar(carry[:], vj[:], 4, None,
                                                A_.arith_shift_right)
                    if j >= 3:
                        mj = dp.tile([np_, ny], dt.int32, tag=f"m{j}")
                        nc.vector.tensor_scalar(mj[:], vj[:], 15, None,
                                                A_.bitwise_and)
                        lmb[j] = mj
                l24 = dp.tile([np_, ny], dt.int32, tag="l24")
                hh = dp.tile([np_, ny], dt.int32, tag="hh")
                def gather_bits(dst, pieces):
                    first = True
                    for (src, shr, andm, shl) in pieces:
                        if andm is not None:
                            nc.vector.tensor_scalar(tmpa[:, :ny], src[:], andm,
                                                    shl, A_.bitwise_and,
                                                    A_.logical_shift_left)
                        elif shr > 0:
                            nc.vector.tensor_scalar(tmpa[:, :ny], src[:], shr,
                                                    None,
                                                    A_.logical_shift_right)
                        else:
                            nc.vector.tensor_scalar(tmpa[:, :ny], src[:], shl,
                                                    None,
                                                    A_.logical_shift_left)
                        if first:
                            nc.vector.tensor_copy(dst[:], tmpa[:, :ny])
                            first = False
                        else:
                            nc.vector.tensor_tensor(dst[:], dst[:],
                                                    tmpa[:, :ny], A_.bitwise_or)
                gather_bits(l24, [(lmb[3], 3, None, 0), (lmb[4], 0, None, 1),
                                  (lmb[5], 0, None, 5), (lmb[6], 0, None, 9),
                                  (lmb[7], 0, None, 13), (lmb[8], 0, None, 17),
                                  (lmb[9], 0, 7, 21)])
                gather_bits(hh, [(lmb[9], 3, None, 0), (lmb[10], 0, None, 1),
                                 (lmb[11], 0, None, 5), (lmb[12], 0, None, 9),
                                 (lmb[13], 0, None, 13),
                                 (lmb[14], 0, None, 17),
                                 (lmb[15], 0, 7, 21)])
                s63 = dp.tile([np_, ny], dt.int32, tag="s63")
                nc.vector.tensor_scalar(s63[:], lmb[15][:], 3, 1,
                                        A_.logical_shift_right, A_.bitwise_and)
                s63f = dp.tile([np_, ny], dt.float32, tag="s63f")
                nc.vector.tensor_copy(s63f[:], s63[:])
                hf = dp.tile([np_, ny], dt.float32, tag="hf")
                nc.vector.tensor_copy(hf[:], hh[:])
                nc.vector.scalar_tensor_tensor(
                    hf[:], s63f[:], -16777216.0, hf[:], A_.mult, A_.add)
                lf24 = dp.tile([np_, ny], dt.float32, tag="lf24")
                nc.vector.tensor_copy(lf24[:], l24[:])
                yv = dp.tile([np_, ny], dt.float32, tag="yv")
                nc.vector.scalar_tensor_tensor(
                    yv[:], hf[:], 16777216.0, lf24[:], A_.mult, A_.add)
                nc.sync.dma_start(
                    bass.AP(ydst, ybase + ch * CW, [[yrstride, np_], [1, ny]]),
                    yv[:])

        with tc.tile_pool(name="fir", bufs=1) as dp:
            emit_comb_fir(
                dp, bass.AP(decb, 0, [[DEC_PP, P], [1, DEC_PP + CHALO]]),
                P, DEC_PP + CHALO, ybuf, 0, DEC_PP)
            f15 = dp.tile([1, L], dt.float32, tag="f15t")
            nc.sync.dma_start(f15[:], f15_in[:])
            y15 = dp.tile([1, L], dt.float32, tag="y15")
            nc.sync.dma_start(y15[:], ybuf[0, :L])
            nc.vector.tensor_copy(y15[:1, :1], y15[:1, :1])
            nc.vector.tensor_tensor(y15[:], y15[:], f15[:], A.mult)
            nc.sync.dma_start(ybuf[0, :L], y15[:1, :])
            tl = dp.tile([1, CHALO + NBH], dt.float32, tag="tl")
            nc.sync.dma_start(tl[:, :CHALO], decb[0, DEC_PC:DEC_PC + CHALO])
            nc.gpsimd.dma_start(tl[:, CHALO:], hnbp[:][pid + 1, :])
            nc.sync.dma_start(taild[:], tl[:])
            emit_comb_fir(dp, taild[:], 1, CHALO + NBH, ybuf, DEC_PC, 1)

        # ============ STFT + mel + log ============
        with (tc.tile_pool(name="stft", bufs=1) as fp,
              tc.tile_pool(name="psum", bufs=1, space="PSUM") as psp):
            wint = fp.tile([128, 4], dt.float32)
            nc.sync.dma_start(wint[:], win_in[:])
            nc.vector.tensor_copy(wint[:1, :1], wint[:1, :1])
            cosm = fp.tile([128, 4 * 257], dt.float32)
            nc.sync.dma_start(
                cosm[:].rearrange("p (k f) -> p k f", f=257),
                bass.AP(cos_in, 0, [[257, 128], [128 * 257, 4], [1, 257]]))
            nc.vector.tensor_copy(cosm[:1, :1], cosm[:1, :1])
            sinm = fp.tile([128, 4 * 257], dt.float32)
            nc.sync.dma_start(
                sinm[:].rearrange("p (k f) -> p k f", f=257),
                bass.AP(sin_in, 0, [[257, 128], [128 * 257, 4], [1, 257]]))
            nc.vector.tensor_copy(sinm[:1, :1], sinm[:1, :1])
            fbm = fp.tile([128, 2 * NMEL], dt.float32)
            nc.sync.dma_start(
                fbm[:].rearrange("p (k f) -> p k f", f=NMEL),
                bass.AP(fb_in, 0, [[NMEL, 128], [128 * NMEL, 2], [1, NMEL]]))
            nc.vector.tensor_copy(fbm[:1, :1], fbm[:1, :1])
            fbm2 = fp.tile([1, NMEL], dt.float32)
            nc.sync.dma_start(fbm2[:], bass.AP(fb_in, 256 * NMEL, [[NMEL, 1], [1, NMEL]]))
            nc.vector.tensor_copy(fbm2[:1, :1], fbm2[:1, :1])
            xts = []
            for k in range(4):
                xk = fp.tile([128, FR_PC], dt.float32, tag=f"xk{k}")
                nc.sync.dma_start(
                    xk[:], bass.AP(ybuf, 128 * k, [[1, 128], [HOP, FR_PC]]))
                nc.vector.tensor_copy(xk[:1, :1], xk[:1, :1])
                nc.vector.tensor_scalar(xk[:], xk[:], wint[:, k:k + 1], None,
                                        A.mult)
                xts.append(xk)
            pw0 = fp.tile([128, FR_PC], dt.float32, tag="pw0")
            pw1 = fp.tile([128, FR_PC], dt.float32, tag="pw1")
            pw2 = fp.tile([1, FR_PC], dt.float32, tag="pw2")
            pwr = [pw0, pw1, pw2]
            fcs = [(0, 128), (128, 256), (256, 257)]
            for fi, (f0, f1) in enumerate(fcs):
                for h in range(2):
                    hs = slice(h * FH, (h + 1) * FH)
                    pc = psp.tile([f1 - f0, FH], dt.float32, tag="pc")
                    ps = psp.tile([f1 - f0, FH], dt.float32, tag="ps")
                    for k in range(4):
                        nc.tensor.matmul(
                            pc[:], cosm[:, 257 * k + f0:257 * k + f1],
                            xts[k][:, hs], start=(k == 0), stop=(k == 3))
                    for k in range(4):
                        nc.tensor.matmul(
                            ps[:], sinm[:, 257 * k + f0:257 * k + f1],
                            xts[k][:, hs], start=(k == 0), stop=(k == 3))
                    t1_ = fp.tile([128, FH], dt.float32, tag="sq1")
                    nc.scalar.activation(t1_[:f1 - f0], pc[:], ACTF.Square)
                    t2_ = fp.tile([128, FH], dt.float32, tag="sq2")
                    nc.scalar.activation(t2_[:f1 - f0], ps[:], ACTF.Square)
                    nc.vector.tensor_tensor(pwr[fi][:, hs], t1_[:f1 - f0],
                                            t2_[:f1 - f0], A.add)
            lm = fp.tile([NMEL, FR_PC], dt.float32, tag="lm")
            for h in range(2):
                hs = slice(h * FH, (h + 1) * FH)
                mm = psp.tile([NMEL, FH], dt.float32, tag="mm")
                nc.tensor.matmul(mm[:], fbm[:, :NMEL], pwr[0][:, hs],
                                 start=True, stop=False)
                nc.tensor.matmul(mm[:], fbm[:, NMEL:], pwr[1][:, hs],
                                 start=False, stop=False)
                nc.tensor.matmul(mm[:], fbm2[:, :], pwr[2][:, hs],
                                 start=False, stop=True)
                xs = fp.tile([NMEL, FH], dt.float32, tag="xs")
                nc.vector.tensor_scalar(xs[:], mm[:], 1e-6, None, A.add)
                bx = fp.tile([NMEL, FH], dt.int32, tag="bx")
                nc.vector.tensor_copy(bx[:], xs[:].bitcast(dt.int32))
                ev = fp.tile([NMEL, FH], dt.int32, tag="ev")
                nc.vector.tensor_scalar(ev[:], bx[:], 23, None,
                                        A.logical_shift_right)
                nc.vector.tensor_scalar(ev[:], ev[:], -127, None, A.add)
                evf = fp.tile([NMEL, FH], dt.float32, tag="evf")
                nc.vector.tensor_copy(evf[:], ev[:])
                nc.vector.tensor_scalar(bx[:], bx[:], 0x7FFFFF, 127 << 23,
                                        A.bitwise_and, A.bitwise_or)
                lnm = fp.tile([NMEL, FH], dt.float32, tag="lnm")
                nc.scalar.activation(lnm[:], bx[:].bitcast(dt.float32), ACTF.Ln)
                nc.vector.scalar_tensor_tensor(
                    lm[:, hs], evf[:], 0.6931471805599453, lnm[:],
                    A.mult, A.add)
            nc.sync.dma_start(out_p[:], lm[:])

    nc.compile()
    return nc


def _constants():
    mask = np.ones((128, 5136), np.float32)
    mask[:, 0::16] = 0.0
    n = np.arange(N_FFT, dtype=np.float64)
    f = np.arange(257, dtype=np.float64)
    ang = 2.0 * np.pi * n[:, None] * f[None, :] / N_FFT
    cosm = np.cos(ang).astype(np.float32)
    sinm = (-np.sin(ang)).astype(np.float32)
    fbm = _mel_fbanks_np()
    t = np.arange(WIN_LEN, dtype=np.float32)
    win = (0.5 * (1.0 - np.cos(2.0 * np.pi * t / WIN_LEN))).astype(np.float32)
    pad_l = (N_FFT - WIN_LEN) // 2
    win_p = np.zeros(N_FFT, np.float32)
    win_p[pad_l:pad_l + WIN_LEN] = win
    winm = win_p.reshape(4, 128).T.copy()
    return mask, cosm, sinm, fbm, winm


def kernel(pdm_bits, taps, scale):
    from concourse.bass_utils import run_bass_kernel_spmd

    pdm = np.asarray(pdm_bits, dtype=np.int32)
    taps_l = [int(x) for x in np.asarray(taps).tolist()]
    key = (tuple(taps_l), int(scale))
    if key not in _COMPILED:
        _COMPILED[key] = _build(taps_l, int(scale))
    nc = _COMPILED[key]

    mask, cosm, sinm, fbm, winm = _constants()
    shards = pdm.reshape(NCORE, P, FREE)
    in_maps = []
    for c in range(NCORE):
        f15 = np.ones((1, L), np.float32)
        if c == 0:
            f15[:] = 0.0
        phi = (4 * c) % 16
        r1 = (472500 * c - phi) // 16
        in_maps.append({
            "pdm": shards[c],
            "mask": mask, "cosm": cosm, "sinm": sinm, "fbm": fbm,
            "winm": winm, "f15": f15,
            "phiv": np.array([[phi]], np.int32),
            "r1v": np.array([[r1]], np.int32),
            "p16v": np.array([[16 - phi]], np.int32),
            "r1m1": np.array([[max(r1 - 1, 0)]], np.int32),
            "v0m": np.array([[1.0 if (phi == 0 and c != 0) else 0.0]],
                            np.float32),
        })
    res = run_bass_kernel_spmd(nc, in_maps, list(range(NCORE)))
    global _LAST_RES
    _LAST_RES = res
    outs = [res.results[c]["out"] for c in range(NCORE)]
    full = np.concatenate(outs, axis=1)[:, :T_FRAMES]
    return full[None, None].astype(np.float32)


# revision 21
# speedup vs baseline: 82.9145x; 82.9145x over previous
"""AudioFrontend Trainium2 kernel: PDM -> CIC(f32 blk16-exact) -> FIR(int64) -> logmel.

Bit-exact replication of jax-CPU float32 cumsum (XLA ReduceWindowRewriter
base-16 blocked scans) through the chaotic CIC stages, exact int64 FIR via
12-bit limbs on gpsimd int32, then matmul STFT/mel/log.
Self-contained: hardcodes all shapes; host code only shards/gathers.
"""
import numpy as np

NCORE = 8
N_PDM = 60_480_000
PERCORE = N_PDM // NCORE          # 7,560,000
P = 125
FREE = PERCORE // P               # 60480
TILE_F = 4032                     # 63*64 = 16*252
NT = FREE // TILE_F               # 15
ROWS_T = TILE_F // 16             # 252
ROWS_P = FREE // 16               # 3780
T0_LOC = P * ROWS_P               # 472500
T0_GLOB = NCORE * T0_LOC          # 3780000
GF = T0_GLOB // P                 # 30240
GCH = 5040
NGC = GF // GCH                   # 6
T1R = GCH // 16                   # 315
T1N = T0_GLOB // 16               # 236250
T1PAD = 236256                    # 123*1920 + 96
T2N = T1PAD // 16                 # 14766
T2PAD = 14768
T3N = T2PAD // 16                 # 923
T3PAD = 928
T4N = T3PAD // 16                 # 58
T4PAD = 64
T5N = T4PAD // 16                 # 4
DECIM = 63
DEC_PC = PERCORE // DECIM         # 120000
DEC_PP = FREE // DECIM            # 960
DEC_T = TILE_F // DECIM           # 64
L = 15
CHALO = 19
NBH = 384
N_FFT = 512
HOP = 160
WIN_LEN = 400
NMEL = 40
FR_PC = 750
T_FRAMES = 1 + (N_PDM // DECIM - N_FFT) // HOP  # 5997
FH = 375
SAT = 9.223372036854775808e18


def _mel_fbanks_np():
    n_freqs = N_FFT // 2 + 1
    all_freqs = np.linspace(0.0, 16000 / 2, n_freqs)
    h2m = lambda f: 2595.0 * np.log10(1.0 + f / 700.0)
    m_pts = np.linspace(h2m(0.0), h2m(8000.0), NMEL + 2)
    f_pts = 700.0 * (10.0 ** (m_pts / 2595.0) - 1.0)
    f_diff = f_pts[1:] - f_pts[:-1]
    slopes = f_pts[None, :] - all_freqs[:, None]
    down = -slopes[:, :-2] / f_diff[:-1]
    up = slopes[:, 2:] / f_diff[1:]
    return np.maximum(0.0, np.minimum(down, up)).astype(np.float32)


_COMPILED = {}
_LAST_RES = None


def _build(taps_list, scale_int):
    import concourse.bass as bass
    import concourse.bacc as bacc
    import concourse.mybir as mybir
    import concourse.tile as tile

    dt = mybir.dt
    A = mybir.AluOpType
    ACTF = mybir.ActivationFunctionType

    nc = bacc.Bacc()
    pdm_in = nc.declare_dram_parameter("pdm", [P, FREE], dt.int32, isOutput=False)
    mask_in = nc.declare_dram_parameter("mask", [128, 5136], dt.float32, isOutput=False)
    cos_in = nc.declare_dram_parameter("cosm", [N_FFT, 257], dt.float32, isOutput=False)
    sin_in = nc.declare_dram_parameter("sinm", [N_FFT, 257], dt.float32, isOutput=False)
    fb_in = nc.declare_dram_parameter("fbm", [257, NMEL], dt.float32, isOutput=False)
    win_in = nc.declare_dram_parameter("winm", [128, 4], dt.float32, isOutput=False)
    f15_in = nc.declare_dram_parameter("f15", [1, L], dt.float32, isOutput=False)
    phi_in = nc.declare_dram_parameter("phiv", [1, 1], dt.int32, isOutput=False)
    r1_in = nc.declare_dram_parameter("r1v", [1, 1], dt.int32, isOutput=False)
    p16_in = nc.declare_dram_parameter("p16v", [1, 1], dt.int32, isOutput=False)
    r1m_in = nc.declare_dram_parameter("r1m1", [1, 1], dt.int32, isOutput=False)
    v0m_in = nc.declare_dram_parameter("v0m", [1, 1], dt.float32, isOutput=False)
    out_p = nc.declare_dram_parameter("out", [NMEL, FR_PC], dt.float32, isOutput=True)

    pA = nc.dram_tensor("pA", [P, FREE], dt.float32)
    pB = nc.dram_tensor("pB", [P, FREE], dt.float32)
    t0loc = nc.dram_tensor("t0loc", [1, T0_LOC], dt.float32)
    e0buf = nc.dram_tensor("e0buf", [1, 16 + T0_LOC + 16], dt.float32)
    h16i = nc.dram_tensor("h16i", [1, 32], dt.float32)
    h16o = nc.dram_tensor("h16o", [NCORE, 32], dt.float32)
    h16p = nc.dram_tensor("h16p", [NCORE + 2, 32], dt.float32)
    t1agi = nc.dram_tensor("t1agi", [1, 29532], dt.float32)
    t1ago = nc.dram_tensor("t1ago", [NCORE, 29532], dt.float32)
    vloc = nc.dram_tensor("vloc", [1, 1 + 472512], dt.float32)
    t1buf = nc.dram_tensor("t1buf", [1, T1PAD], dt.float32)
    t2buf = nc.dram_tensor("t2buf", [1, T2PAD], dt.float32)
    zbuf = nc.dram_tensor("zbuf", [1, 1 + T2PAD], dt.float32)
    zsbuf = nc.dram_tensor("zsbuf", [1, 1 + T1PAD], dt.float32)
    decb = nc.dram_tensor("decb", [1, CHALO + DEC_PC], dt.float32)
    ybuf = nc.dram_tensor("ybuf", [1, DEC_PC + NBH], dt.float32)
    h19i = nc.dram_tensor("h19i", [1, CHALO], dt.float32)
    h19o = nc.dram_tensor("h19o", [NCORE, CHALO], dt.float32)
    h19p = nc.dram_tensor("h19p", [NCORE + 1, CHALO], dt.float32)
    hnbi = nc.dram_tensor("hnbi", [1, NBH], dt.float32)
    hnbo = nc.dram_tensor("hnbo", [NCORE, NBH], dt.float32)
    hnbp = nc.dram_tensor("hnbp", [NCORE + 1, NBH], dt.float32)
    taild = nc.dram_tensor("taild", [1, CHALO + NBH], dt.float32)
    t3d = nc.dram_tensor("t3d", [1, T3N], dt.float32)
    z3buf = nc.dram_tensor("z3buf", [1, 1 + T3PAD], dt.float32)

    RG = [list(range(NCORE))]
    PHIS = [(4 * c) % 16 for c in range(NCORE)]
    R1S = [(T0_LOC * c - PHIS[c]) // 16 for c in range(NCORE)]
    MCS = [(R1S[c + 1] - R1S[c]) if c + 1 < NCORE else (T1N - R1S[c])
           for c in range(NCORE)]
    taps = [int(t) for t in taps_list]
    assert (1 << 15) == int(scale_int)

    with tile.TileContext(nc) as tc:
        pid = nc.gpsimd.partition_id()

        # ============ scan stages ============
        with tc.tile_pool(name="persist", bufs=1) as pp:
            mask = pp.tile([128, 5136], dt.float32)
            nc.sync.dma_start(mask[:], mask_in[:])
            nc.vector.tensor_copy(mask[:1, :1], mask[:1, :1])
            t0sb = pp.tile([P, ROWS_P], dt.float32)
            carry0 = pp.tile([P, ROWS_P], dt.float32)
            decsb = pp.tile([P, DEC_PP], dt.float32)

            for st in range(5):
                src = [None, pA, pB, pA, pB][st]
                dst = [pA, pB, pA, pB, pA][st]
                with tc.tile_pool(name=f"s{st}", bufs=3) as sp:
                    for t in range(NT):
                        fs = slice(t * TILE_F, (t + 1) * TILE_F)
                        if st == 0:
                            raw = sp.tile([P, TILE_F], dt.int32, tag="raw")
                            nc.sync.dma_start(raw[:], pdm_in[:, fs])
                            xt = sp.tile([P, TILE_F], dt.float32, tag="xt")
                            nc.scalar.activation(xt[:], raw[:], ACTF.Copy,
                                                 bias=-1.0, scale=2.0)
                        else:
                            pin = sp.tile([P, TILE_F], dt.float32, tag="pin")
                            nc.sync.dma_start(pin[:], src[:, fs])
                            nc.vector.tensor_copy(pin[:1, :1], pin[:1, :1])
                            xt = sp.tile([P, TILE_F], dt.float32, tag="xt")
                            rs = slice(t * ROWS_T, (t + 1) * ROWS_T)
                            nc.vector.tensor_tensor(
                                xt[:].rearrange("p (r s) -> p r s", s=16),
                                pin[:].rearrange("p (r s) -> p r s", s=16),
                                carry0[:, rs].broadcast_to([P, ROWS_T, 16]),
                                A.add)
                        po = sp.tile([P, TILE_F], dt.float32, tag="po")
                        nc.vector.tensor_tensor_scan(
                            po[:], mask[:P, :TILE_F], xt[:], 0.0, A.mult, A.add)
                        nc.vector.tensor_copy(
                            t0sb[:, t * ROWS_T:(t + 1) * ROWS_T], po[:, 15::16])
                        nc.sync.dma_start(dst[:, fs], po[:])

                nc.sync.dma_start(
                    bass.AP(t0loc, 0, [[ROWS_P, P], [1, ROWS_P]]), t0sb[:])
                nc.sync.dma_start(h16i[0, :16], t0sb[:1, :16])
                nc.sync.dma_start(h16i[0, 16:], t0sb[P - 1:P, ROWS_P - 16:])
                nc.gpsimd.collective_compute(
                    "AllGather", A.bypass, replica_groups=RG,
                    ins=[h16i[:]], outs=[h16o[:]])
                # padded halo rows: row0=AG7, rows1..8=AG0..7, row9=AG0
                nc.sync.dma_start(h16p[0, :], h16o[NCORE - 1, :])
                nc.sync.dma_start(h16p[1:NCORE + 1, :], h16o[:])
                nc.sync.dma_start(h16p[NCORE + 1, :], h16o[0, :])
                # e0: [left16 halo][own t0][right16 halo]
                nc.gpsimd.dma_start(e0buf[0, :16], h16p[:][pid, 16:])
                nc.sync.dma_start(e0buf[0, 16:16 + T0_LOC], t0loc[0, :])
                nc.gpsimd.dma_start(
                    e0buf[0, 16 + T0_LOC:], h16p[:][pid + 2, :16])

                with tc.tile_pool(name=f"g{st}", bufs=1) as gp:
                    phir = nc.gpsimd.alloc_register(f"phir{st}")
                    nc.gpsimd.reg_load(phir, phi_in[:1, :1])
                    r1r = nc.gpsimd.alloc_register(f"r1r{st}")
                    nc.gpsimd.reg_load(r1r, r1_in[:1, :1])
                    p16r = nc.gpsimd.alloc_register(f"p16r{st}")
                    nc.gpsimd.reg_load(p16r, p16_in[:1, :1])
                    r1mr = nc.gpsimd.alloc_register(f"r1mr{st}")
                    nc.gpsimd.reg_load(r1mr, r1m_in[:1, :1])
                    # local extended-t0 scan: [92, 5136] from e0buf
                    ge = gp.tile([92, 5136], dt.float32, tag="ge")
                    nc.gpsimd.dma_start(
                        ge[:],
                        bass.AP(e0buf, bass.make_scalar_value(p16r),
                                [[5136, 92], [1, 5136]]))
                    nc.vector.tensor_copy(ge[:1, :1], ge[:1, :1])
                    gs = gp.tile([92, 5136], dt.float32, tag="gs")
                    nc.vector.tensor_tensor_scan(
                        gs[:], mask[:92, :5136], ge[:], 0.0, A.mult, A.add)
                    tx = gp.tile([92, 321], dt.float32, tag="tx")
                    nc.vector.tensor_copy(tx[:], gs[:, 15::16])
                    nc.sync.dma_start(
                        bass.AP(t1agi, 0, [[321, 92], [1, 321]]), tx[:])
                    nc.gpsimd.collective_compute(
                        "AllGather", A.bypass, replica_groups=RG,
                        ins=[t1agi[:]], outs=[t1ago[:]])
                    # compact ragged t1 contributions into t1buf
                    for c in range(NCORE):
                        nc.sync.dma_start(
                            t1buf[0, R1S[c]:R1S[c] + MCS[c]],
                            t1ago[c, :MCS[c]])
                    zt = gp.tile([1, 16], dt.float32, tag="zt")
                    nc.vector.memset(zt[:], 0.0)
                    nc.sync.dma_start(t1buf[0, T1N:T1PAD], zt[:1, :T1PAD - T1N])
                    nc.sync.dma_start(zbuf[0, :1], zt[:1, :1])
                    nc.sync.dma_start(zsbuf[0, :1], zt[:1, :1])

                    u1a = gp.tile([123, 1920], dt.float32, tag="u1a")
                    nc.sync.dma_start(
                        u1a[:], bass.AP(t1buf, 0, [[1920, 123], [1, 1920]]))
                    nc.vector.tensor_copy(u1a[:1, :1], u1a[:1, :1])
                    p2a = gp.tile([123, 1920], dt.float32, tag="p2a")
                    nc.vector.tensor_tensor_scan(
                        p2a[:], mask[:123, :1920], u1a[:], 0.0, A.mult, A.add)
                    u1b = gp.tile([1, 96], dt.float32, tag="u1b")
                    nc.sync.dma_start(u1b[:], t1buf[0, 123 * 1920:T1PAD])
                    nc.vector.tensor_copy(u1b[:1, :1], u1b[:1, :1])
                    p2b = gp.tile([1, 96], dt.float32, tag="p2b")
                    nc.vector.tensor_tensor_scan(
                        p2b[:], mask[:1, :96], u1b[:], 0.0, A.mult, A.add)
                    t2a = gp.tile([123, 120], dt.float32, tag="t2a")
                    nc.vector.tensor_copy(t2a[:], p2a[:, 15::16])
                    nc.sync.dma_start(
                        bass.AP(t2buf, 0, [[120, 123], [1, 120]]), t2a[:])
                    t2b = gp.tile([1, 6], dt.float32, tag="t2b")
                    nc.vector.tensor_copy(t2b[:], p2b[:, 15::16])
                    nc.sync.dma_start(t2buf[0, 123 * 120:T2N], t2b[:1, :])
                    nc.sync.dma_start(t2buf[0, T2N:T2PAD], zt[:1, :T2PAD - T2N])

                    u2 = gp.tile([13, 1136], dt.float32, tag="u2")
                    nc.sync.dma_start(
                        u2[:], bass.AP(t2buf, 0, [[1136, 13], [1, 1136]]))
                    nc.vector.tensor_copy(u2[:1, :1], u2[:1, :1])
                    p3 = gp.tile([13, 1136], dt.float32, tag="p3")
                    nc.vector.tensor_tensor_scan(
                        p3[:], mask[:13, :1136], u2[:], 0.0, A.mult, A.add)
                    t3x = gp.tile([13, 71], dt.float32, tag="t3x")
                    nc.vector.tensor_copy(t3x[:], p3[:, 15::16])
                    nc.sync.dma_start(
                        bass.AP(t3d, 0, [[71, 13], [1, 71]]), t3x[:])
                    u3 = gp.tile([1, T3PAD], dt.float32, tag="u3")
                    nc.vector.memset(u3[:], 0.0)
                    nc.sync.dma_start(u3[:1, :T3N], t3d[0, :])
                    nc.vector.tensor_copy(u3[:1, :1], u3[:1, :1])
                    p4 = gp.tile([1, T3PAD], dt.float32, tag="p4")
                    nc.vector.tensor_tensor_scan(
                        p4[:], mask[:1, :T3PAD], u3[:], 0.0, A.mult, A.add)
                    u4 = gp.tile([1, T4PAD], dt.float32, tag="u4")
                    nc.vector.memset(u4[:], 0.0)
                    nc.vector.tensor_copy(u4[:, :T4N], p4[:, 15::16])
                    p5 = gp.tile([1, T4PAD], dt.float32, tag="p5")
                    nc.vector.tensor_tensor_scan(
                        p5[:], mask[:1, :T4PAD], u4[:], 0.0, A.mult, A.add)
                    u5 = gp.tile([1, T5N], dt.float32, tag="u5")
                    nc.vector.tensor_copy(u5[:], p5[:, 15::16])
                    s5 = gp.tile([1, T5N], dt.float32, tag="s5")
                    nc.vector.tensor_tensor_scan(
                        s5[:], mask[:1, :T5N], u5[:], 0.0, A.mult, A.add)
                    nc.vector.tensor_tensor(
                        p5[:, 16:].rearrange("p (r s) -> p r s", s=16),
                        p5[:, 16:].rearrange("p (r s) -> p r s", s=16),
                        s5[:, :3].broadcast_to([1, 3, 16]), A.add)
                    nc.vector.tensor_tensor(
                        p4[:, 16:].rearrange("p (r s) -> p r s", s=16),
                        p4[:, 16:].rearrange("p (r s) -> p r s", s=16),
                        p5[:, :T4N - 1].broadcast_to([1, T4N - 1, 16]), A.add)
                    nc.sync.dma_start(z3buf[0, :1], zt[:1, :1])
                    nc.sync.dma_start(z3buf[0, 1:1 + T3PAD], p4[:1, :])
                    cz3 = gp.tile([13, 71], dt.float32, tag="t3x")
                    nc.sync.dma_start(
                        cz3[:], bass.AP(z3buf, 0, [[71, 13], [1, 71]]))
                    nc.vector.tensor_copy(cz3[:1, :1], cz3[:1, :1])
                    nc.vector.tensor_tensor(
                        p3[:].rearrange("p (r s) -> p r s", s=16),
                        p3[:].rearrange("p (r s) -> p r s", s=16),
                        cz3[:].broadcast_to([13, 71, 16]), A.add)
                    nc.sync.dma_start(
                        bass.AP(zbuf, 1, [[1136, 13], [1, 1136]]), p3[:])
                    cza = gp.tile([123, 120], dt.float32, tag="cza")
                    nc.sync.dma_start(
                        cza[:], bass.AP(zbuf, 0, [[120, 123], [1, 120]]))
                    nc.vector.tensor_copy(cza[:1, :1], cza[:1, :1])
                    nc.vector.tensor_tensor(
                        p2a[:].rearrange("p (r s) -> p r s", s=16),
                        p2a[:].rearrange("p (r s) -> p r s", s=16),
                        cza[:].broadcast_to([123, 120, 16]), A.add)
                    czb = gp.tile([1, 6], dt.float32, tag="czb")
                    nc.sync.dma_start(czb[:], zbuf[0, 123 * 120:123 * 120 + 6])
                    nc.vector.tensor_copy(czb[:1, :1], czb[:1, :1])
                    nc.vector.tensor_tensor(
                        p2b[:].rearrange("p (r s) -> p r s", s=16),
                        p2b[:].rearrange("p (r s) -> p r s", s=16),
                        czb[:].broadcast_to([1, 6, 16]), A.add)
                    nc.sync.dma_start(
                        bass.AP(zsbuf, 1, [[1920, 123], [1, 1920]]), p2a[:])
                    nc.sync.dma_start(
                        zsbuf[0, 1 + 123 * 1920:1 + T1PAD], p2b[:1, :])

                    # own scan_t0: p1_local + bcast(Zs at own rows)
                    ctb = gp.tile([92, 321], dt.float32, tag="tx")
                    nc.gpsimd.dma_start(
                        ctb[:],
                        bass.AP(zsbuf, bass.make_scalar_value(r1r),
                                [[321, 92], [1, 321]]))
                    nc.vector.tensor_copy(ctb[:1, :1], ctb[:1, :1])
                    nc.vector.tensor_tensor(
                        gs[:].rearrange("p (r s) -> p r s", s=16),
                        gs[:].rearrange("p (r s) -> p r s", s=16),
                        ctb[:].broadcast_to([92, 321, 16]), A.add)
                    nc.sync.dma_start(
                        bass.AP(vloc, 1, [[5136, 92], [1, 5136]]), gs[:])
                    # vloc[0]: 0 normally; for the phi==0 mid core (c=4):
                    # scan_t0[A_c - 1] = t1[A_c/16 - 1] + scan_t1[A_c/16 - 2]
                    sv = gp.tile([1, 2], dt.float32, tag="sv")
                    nc.gpsimd.dma_start(
                        sv[:1, :1],
                        bass.AP(t1buf, bass.make_scalar_value(r1mr),
                                [[1, 1], [1, 1]]))
                    nc.gpsimd.dma_start(
                        sv[:1, 1:],
                        bass.AP(zsbuf, bass.make_scalar_value(r1mr),
                                [[1, 1], [1, 1]]))
                    v0t = gp.tile([1, 1], dt.float32, tag="v0t")
                    nc.sync.dma_start(v0t[:], v0m_in[:])
                    nc.vector.tensor_copy(v0t[:1, :1], v0t[:1, :1])
                    sv2 = gp.tile([1, 1], dt.float32, tag="sv2")
                    nc.vector.tensor_tensor(sv2[:], sv[:1, :1], sv[:1, 1:],
                                            A.add)
                    nc.vector.tensor_tensor(sv2[:], sv2[:], v0t[:], A.mult)
                    nc.sync.dma_start(vloc[0, :1], sv2[:1, :])
                    nc.gpsimd.dma_start(
                        carry0[:],
                        bass.AP(vloc, bass.make_scalar_value(phir),
                                [[ROWS_P, P], [1, ROWS_P]]))
                    nc.vector.tensor_copy(carry0[:1, :1], carry0[:1, :1])

            # ============ phase B of stage 5: decimate ============
            with tc.tile_pool(name="pb5", bufs=3) as sp:
                for t in range(NT):
                    fs = slice(t * TILE_F, (t + 1) * TILE_F)
                    pin = sp.tile([P, TILE_F], dt.float32, tag="pin")
                    nc.sync.dma_start(pin[:], pA[:, fs])
                    nc.vector.tensor_copy(pin[:1, :1], pin[:1, :1])
                    xt = sp.tile([P, TILE_F], dt.float32, tag="xt")
                    rs = slice(t * ROWS_T, (t + 1) * ROWS_T)
                    nc.vector.tensor_tensor(
                        xt[:].rearrange("p (r s) -> p r s", s=16),
                        pin[:].rearrange("p (r s) -> p r s", s=16),
                        carry0[:, rs].broadcast_to([P, ROWS_T, 16]), A.add)
                    nc.vector.tensor_copy(
                        decsb[:, t * DEC_T:(t + 1) * DEC_T], xt[:, 0::DECIM])

            nc.sync.dma_start(
                bass.AP(decb, CHALO, [[DEC_PP, P], [1, DEC_PP]]), decsb[:])
            nc.sync.dma_start(h19i[:1, :], decsb[P - 1:P, DEC_PP - CHALO:])
            nc.gpsimd.collective_compute(
                "AllGather", A.bypass, replica_groups=RG,
                ins=[h19i[:]], outs=[h19o[:]])
            nc.sync.dma_start(hnbi[:1, :], decsb[:1, :NBH])
            nc.gpsimd.collective_compute(
                "AllGather", A.bypass, replica_groups=RG,
                ins=[hnbi[:]], outs=[hnbo[:]])
            with tc.tile_pool(name="hx", bufs=1) as hp:
                zt2 = hp.tile([1, NBH], dt.float32)
                nc.vector.memset(zt2[:], 0.0)
                nc.sync.dma_start(h19p[0, :], zt2[:1, :CHALO])
                nc.sync.dma_start(h19p[1:, :], h19o[:])
                nc.sync.dma_start(hnbp[:NCORE, :], hnbo[:])
                nc.sync.dma_start(hnbp[NCORE, :], hnbo[0, :])
                nc.gpsimd.dma_start(decb[0, :CHALO], h19p[:][pid, :])

        # ============ comb + FIR + y ============
        def emit_comb_fir(dp, src_ap, np_, width, ydst, ybase, yrstride):
            A_ = A
            NL = 16
            cmb = dp.tile([np_, width], dt.float32, tag="cmb0")
            nc.sync.dma_start(cmb[:], src_ap)
            nc.vector.tensor_copy(cmb[:1, :1], cmb[:1, :1])
            cur = cmb
            w = width
            for it in range(5):
                nxt = dp.tile([np_, w - 1], dt.float32, tag=f"cmb{1 + it % 2}")
                nc.vector.tensor_tensor(
                    nxt[:], cur[:, 1:w], cur[:, :w - 1], A_.subtract)
                cur = nxt
                w -= 1
            nw_full = w
            ny_full = nw_full - (L - 1)
            CW = 320 if np_ > 1 else ny_full
            nch = ny_full // CW
            assert nch * CW == ny_full
            for ch in range(nch):
                cw = cur[:, ch * CW:ch * CW + CW + (L - 1)]
                nw = CW + (L - 1)
                ny = CW
                satp = dp.tile([np_, nw], dt.float32, tag="satp")
                nc.vector.tensor_scalar(satp[:], cw, SAT, None, A_.is_ge)
                satn = dp.tile([np_, nw], dt.float32, tag="satn")
                nc.vector.tensor_scalar(satn[:], cw, -SAT, None, A_.is_le)
                sgn = dp.tile([np_, nw], dt.float32, tag="sgn")
                nc.vector.tensor_scalar(sgn[:], cw, 0.0, None, A_.is_ge)
                nc.vector.tensor_scalar(sgn[:], sgn[:], 2.0, -1.0, A_.mult,
                                        A_.add)
                mag = dp.tile([np_, nw], dt.float32, tag="mag")
                nc.vector.tensor_tensor(mag[:], cw, sgn[:], A_.mult)
                rs_ = dp.tile([np_, nw], dt.float32, tag="rs")
                nc.vector.tensor_tensor(rs_[:], satp[:], satn[:], A_.add)
                nc.vector.tensor_scalar(rs_[:], rs_[:], -1.0, 1.0, A_.mult,
                                        A_.add)
                nc.vector.tensor_tensor(rs_[:], rs_[:], sgn[:], A_.mult)
                bits = dp.tile([np_, nw], dt.int32, tag="bits")
                nc.vector.tensor_copy(bits[:], mag[:].bitcast(dt.int32))
                ex = dp.tile([np_, nw], dt.int32, tag="ex")
                nc.vector.tensor_scalar(ex[:], bits[:], 23, None,
                                        A_.logical_shift_right)
                nc.vector.tensor_scalar(ex[:], ex[:], 255, None, A_.bitwise_and)
                nc.vector.tensor_scalar(ex[:], ex[:], -150, None, A_.add)
                mant = dp.tile([np_, nw], dt.int32, tag="mant")
                nc.vector.tensor_scalar(mant[:], bits[:], 0x7FFFFF, 0x800000,
                                        A_.bitwise_and, A_.bitwise_or)
                nzm = dp.tile([np_, nw], dt.int32, tag="nzm")
                nc.vector.tensor_scalar(nzm[:], ex[:], -23, None, A_.is_ge)
                nc.vector.tensor_tensor(mant[:], mant[:], nzm[:], A_.mult)
                tmpa = dp.tile([np_, nw], dt.int32, tag="tmpa")
                tmpb = dp.tile([np_, nw], dt.int32, tag="tmpb")
                tmpr = dp.tile([np_, nw], dt.int32, tag="tmpr")
                tmps = dp.tile([np_, nw], dt.int32, tag="tmps")
                sel = dp.tile([np_, nw], dt.int32, tag="sel")
                limbs = []
                for j in range(NL):
                    nc.vector.tensor_scalar(tmpr[:], ex[:], -1, 4 * j, A_.mult,
                                            A_.add)
                    nc.vector.tensor_scalar(tmps[:], tmpr[:], 31, None, A_.min)
                    nc.vector.tensor_scalar(tmps[:], tmps[:], 0, None, A_.max)
                    nc.vector.tensor_tensor(tmpa[:], mant[:], tmps[:],
                                            A_.logical_shift_right)
                    nc.vector.tensor_scalar(tmpa[:], tmpa[:], 15, None,
                                            A_.bitwise_and)
                    nc.vector.tensor_scalar(tmps[:], tmpr[:], -1, 0, A_.mult,
                                            A_.max)
                    nc.vector.tensor_scalar(tmps[:], tmps[:], 3, None, A_.min)
                    nc.vector.tensor_tensor(tmpb[:], mant[:], tmps[:],
                                            A_.logical_shift_left)
                    nc.vector.tensor_scalar(tmpb[:], tmpb[:], 15, None,
                                            A_.bitwise_and)
                    nc.vector.tensor_scalar(sel[:], tmpr[:], 0, None, A_.is_ge)
                    li = dp.tile([np_, nw], dt.int32, tag="li")
                    nc.vector.select(li[:], sel[:], tmpa[:], tmpb[:])
                    lf = dp.tile([np_, nw], dt.float32, tag=f"lf{j}")
                    nc.vector.tensor_copy(lf[:], li[:])
                    nc.vector.tensor_tensor(lf[:], lf[:], rs_[:], A_.mult)
                    limbs.append(lf)
                accA = dp.tile([np_, ny], dt.float32, tag="accA")
                accB = dp.tile([np_, ny], dt.float32, tag="accB")
                for k in range(L):
                    o = L - 1 - k
                    if k == 0:
                        nc.vector.tensor_scalar(accA[:], satp[:, o:o + ny],
                                                float(taps[k]), None, A_.mult)
                        nc.vector.tensor_scalar(accB[:], satn[:, o:o + ny],
                                                float(taps[k]), None, A_.mult)
                    else:
                        nc.vector.scalar_tensor_tensor(
                            accA[:], satp[:, o:o + ny], float(taps[k]),
                            accA[:], A_.mult, A_.add)
                        nc.vector.scalar_tensor_tensor(
                            accB[:], satn[:, o:o + ny], float(taps[k]),
                            accB[:], A_.mult, A_.add)
                cols = []
                for j in range(NL):
                    cj = dp.tile([np_, ny], dt.float32, tag=f"c{j}")
                    nc.vector.tensor_scalar(cj[:],
                                            limbs[j][:, L - 1:L - 1 + ny],
                                            float(taps[0]), None, A_.mult)
                    for k in range(1, L):
                        o = L - 1 - k
                        nc.vector.scalar_tensor_tensor(
                            cj[:], limbs[j][:, o:o + ny], float(taps[k]),
                            cj[:], A_.mult, A_.add)
                    cols.append(cj)
                ai = dp.tile([np_, ny], dt.int32, tag="ai")
                nc.vector.tensor_copy(ai[:], accA[:])
                bi = dp.tile([np_, ny], dt.int32, tag="bi")
                nc.vector.tensor_copy(bi[:], accB[:])
                nc.vector.tensor_tensor(ai[:], ai[:], bi[:], A_.subtract)
                nc.vector.tensor_scalar(ai[:], ai[:], 1, 3, A_.bitwise_and,
                                        A_.logical_shift_left)
                parf = dp.tile([np_, ny], dt.float32, tag="parf")
                nc.vector.tensor_copy(parf[:], ai[:])
                nc.vector.tensor_tensor(cols[0][:], cols[0][:], accA[:],
                                        A_.subtract)
                nc.vector.tensor_tensor(cols[15][:], cols[15][:], parf[:],
                                        A_.add)
                carry = dp.tile([np_, ny], dt.int32, tag="cy")
                vj = dp.tile([np_, ny], dt.int32, tag="vj")
                lmb = [None] * NL
                for j in range(NL):
                    nc.vector.tensor_copy(vj[:], cols[j][:])
                    if j > 0:
                        nc.vector.tensor_tensor(vj[:], vj[:], carry[:], A_.add)
                    if j < NL - 1:
                        nc.vector.tensor_scalar(carry[:], vj[:], 4, None,
                                                A_.arith_shift_right)
                    if j >= 3:
                        mj = dp.tile([np_, ny], dt.int32, tag=f"m{j}")
                        nc.vector.tensor_scalar(mj[:], vj[:], 15, None,
                                                A_.bitwise_and)
                        lmb[j] = mj
                l24 = dp.tile([np_, ny], dt.int32, tag="l24")
                hh = dp.tile([np_, ny], dt.int32, tag="hh")
                def gather_bits(dst, pieces):
                    first = True
                    for (src, shr, andm, shl) in pieces:
                        if andm is not None:
                            nc.vector.tensor_scalar(tmpa[:, :ny], src[:], andm,
                                                    shl, A_.bitwise_and,
                                                    A_.logical_shift_left)
                        elif shr > 0:
                            nc.vector.tensor_scalar(tmpa[:, :ny], src[:], shr,
                                                    None,
                                                    A_.logical_shift_right)
                        else:
                            nc.vector.tensor_scalar(tmpa[:, :ny], src[:], shl,
                                                    None,
                                                    A_.logical_shift_left)
                        if first:
                            nc.vector.tensor_copy(dst[:], tmpa[:, :ny])
                            first = False
                        else:
                            nc.vector.tensor_tensor(dst[:], dst[:],
                                                    tmpa[:, :ny], A_.bitwise_or)
                gather_bits(l24, [(lmb[3], 3, None, 0), (lmb[4], 0, None, 1),
                                  (lmb[5], 0, None, 5), (lmb[6], 0, None, 9),
                                  (lmb[7], 0, None, 13), (lmb[8], 0, None, 17),
                                  (lmb[9], 0, 7, 21)])
                gather_bits(hh, [(lmb[9], 3, None, 0), (lmb[10], 0, None, 1),
                                 (lmb[11], 0, None, 5), (lmb[12], 0, None, 9),
                                 (lmb[13], 0, None, 13),
                                 (lmb[14], 0, None, 17),
                                 (lmb[15], 0, 7, 21)])
                s63 = dp.tile([np_, ny], dt.int32, tag="s63")
                nc.vector.tensor_scalar(s63[:], lmb[15][:], 3, 1,
                                        A_.logical_shift_right, A_.bitwise_and)
                s63f = dp.tile([np_, ny], dt.float32, tag="s63f")
                nc.vector.tensor_copy(s63f[:], s63[:])
                hf = dp.tile([np_, ny], dt.float32, tag="hf")
                nc.vector.tensor_copy(hf[:], hh[:])
                nc.vector.scalar_tensor_tensor(
                    hf[:], s63f[:], -16777216.0, hf[:], A_.mult, A_.add)
                lf24 = dp.tile([np_, ny], dt.float32, tag="lf24")
                nc.vector.tensor_copy(lf24[:], l24[:])
                yv = dp.tile([np_, ny], dt.float32, tag="yv")
                nc.vector.scalar_tensor_tensor(
                    yv[:], hf[:], 16777216.0, lf24[:], A_.mult, A_.add)
                nc.sync.dma_start(
                    bass.AP(ydst, ybase + ch * CW, [[yrstride, np_], [1, ny]]),
                    yv[:])

        with tc.tile_pool(name="fir", bufs=1) as dp:
            emit_comb_fir(
                dp, bass.AP(decb, 0, [[DEC_PP, P], [1, DEC_PP + CHALO]]),
                P, DEC_PP + CHALO, ybuf, 0, DEC_PP)
            f15 = dp.tile([1, L], dt.float32, tag="f15t")
            nc.sync.dma_start(f15[:], f15_in[:])
            y15 = dp.tile([1, L], dt.float32, tag="y15")
            nc.sync.dma_start(y15[:], ybuf[0, :L])
            nc.vector.tensor_copy(y15[:1, :1], y15[:1, :1])
            nc.vector.tensor_tensor(y15[:], y15[:], f15[:], A.mult)
            nc.sync.dma_start(ybuf[0, :L], y15[:1, :])
            tl = dp.tile([1, CHALO + NBH], dt.float32, tag="tl")
            nc.sync.dma_start(tl[:, :CHALO], decb[0, DEC_PC:DEC_PC + CHALO])
            nc.gpsimd.dma_start(tl[:, CHALO:], hnbp[:][pid + 1, :])
            nc.sync.dma_start(taild[:], tl[:])
            emit_comb_fir(dp, taild[:], 1, CHALO + NBH, ybuf, DEC_PC, 1)

        # ============ STFT + mel + log ============
        with (tc.tile_pool(name="stft", bufs=1) as fp,
              tc.tile_pool(name="psum", bufs=1, space="PSUM") as psp):
            wint = fp.tile([128, 4], dt.float32)
            nc.sync.dma_start(wint[:], win_in[:])
            nc.vector.tensor_copy(wint[:1, :1], wint[:1, :1])
            cosm = fp.tile([128, 4 * 257], dt.float32)
            nc.sync.dma_start(
                cosm[:].rearrange("p (k f) -> p k f", f=257),
                bass.AP(cos_in, 0, [[257, 128], [128 * 257, 4], [1, 257]]))
            nc.vector.tensor_copy(cosm[:1, :1], cosm[:1, :1])
            sinm = fp.tile([128, 4 * 257], dt.float32)
            nc.sync.dma_start(
                sinm[:].rearrange("p (k f) -> p k f", f=257),
                bass.AP(sin_in, 0, [[257, 128], [128 * 257, 4], [1, 257]]))
            nc.vector.tensor_copy(sinm[:1, :1], sinm[:1, :1])
            fbm = fp.tile([128, 2 * NMEL], dt.float32)
            nc.sync.dma_start(
                fbm[:].rearrange("p (k f) -> p k f", f=NMEL),
                bass.AP(fb_in, 0, [[NMEL, 128], [128 * NMEL, 2], [1, NMEL]]))
            nc.vector.tensor_copy(fbm[:1, :1], fbm[:1, :1])
            fbm2 = fp.tile([1, NMEL], dt.float32)
            nc.sync.dma_start(fbm2[:], bass.AP(fb_in, 256 * NMEL, [[NMEL, 1], [1, NMEL]]))
            nc.vector.tensor_copy(fbm2[:1, :1], fbm2[:1, :1])
            xts = []
            for k in range(4):
                xk = fp.tile([128, FR_PC], dt.float32, tag=f"xk{k}")
                nc.sync.dma_start(
                    xk[:], bass.AP(ybuf, 128 * k, [[1, 128], [HOP, FR_PC]]))
                nc.vector.tensor_copy(xk[:1, :1], xk[:1, :1])
                nc.vector.tensor_scalar(xk[:], xk[:], wint[:, k:k + 1], None,
                                        A.mult)
                xts.append(xk)
            pw0 = fp.tile([128, FR_PC], dt.float32, tag="pw0")
            pw1 = fp.tile([128, FR_PC], dt.float32, tag="pw1")
            pw2 = fp.tile([1, FR_PC], dt.float32, tag="pw2")
            pwr = [pw0, pw1, pw2]
            fcs = [(0, 128), (128, 256), (256, 257)]
            for fi, (f0, f1) in enumerate(fcs):
                for h in range(2):
                    hs = slice(h * FH, (h + 1) * FH)
                    pc = psp.tile([f1 - f0, FH], dt.float32, tag="pc")
                    ps = psp.tile([f1 - f0, FH], dt.float32, tag="ps")
                    for k in range(4):
                        nc.tensor.matmul(
                            pc[:], cosm[:, 257 * k + f0:257 * k + f1],
                            xts[k][:, hs], start=(k == 0), stop=(k == 3))
                    for k in range(4):
                        nc.tensor.matmul(
                            ps[:], sinm[:, 257 * k + f0:257 * k + f1],
                            xts[k][:, hs], start=(k == 0), stop=(k == 3))
                    t1_ = fp.tile([128, FH], dt.float32, tag="sq1")
                    nc.scalar.activation(t1_[:f1 - f0], pc[:], ACTF.Square)
                    t2_ = fp.tile([128, FH], dt.float32, tag="sq2")
                    nc.scalar.activation(t2_[:f1 - f0], ps[:], ACTF.Square)
                    nc.vector.tensor_tensor(pwr[fi][:, hs], t1_[:f1 - f0],
                                            t2_[:f1 - f0], A.add)
            lm = fp.tile([NMEL, FR_PC], dt.float32, tag="lm")
            for h in range(2):
                hs = slice(h * FH, (h + 1) * FH)
                mm = psp.tile([NMEL, FH], dt.float32, tag="mm")
                nc.tensor.matmul(mm[:], fbm[:, :NMEL], pwr[0][:, hs],
                                 start=True, stop=False)
                nc.tensor.matmul(mm[:], fbm[:, NMEL:], pwr[1][:, hs],
                                 start=False, stop=False)
                nc.tensor.matmul(mm[:], fbm2[:, :], pwr[2][:, hs],
                                 start=False, stop=True)
                xs = fp.tile([NMEL, FH], dt.float32, tag="xs")
                nc.vector.tensor_scalar(xs[:], mm[:], 1e-6, None, A.add)
                bx = fp.tile([NMEL, FH], dt.int32, tag="bx")
                nc.vector.tensor_copy(bx[:], xs[:].bitcast(dt.int32))
                ev = fp.tile([NMEL, FH], dt.int32, tag="ev")
                nc.vector.tensor_scalar(ev[:], bx[:], 23, None,
                                        A.logical_shift_right)
                nc.vector.tensor_scalar(ev[:], ev[:], -127, None, A.add)
                evf = fp.tile([NMEL, FH], dt.float32, tag="evf")
                nc.vector.tensor_copy(evf[:], ev[:])
                nc.vector.tensor_scalar(bx[:], bx[:], 0x7FFFFF, 127 << 23,
                                        A.bitwise_and, A.bitwise_or)
                lnm = fp.tile([NMEL, FH], dt.float32, tag="lnm")
                nc.scalar.activation(lnm[:], bx[:].bitcast(dt.float32), ACTF.Ln)
                nc.vector.scalar_tensor_tensor(
                    lm[:, hs], evf[:], 0.6931471805599453, lnm[:],
                    A.mult, A.add)
            nc.sync.dma_start(out_p[:], lm[:])

    nc.compile()
    return nc


def _constants():
    mask = np.ones((128, 5136), np.float32)
    mask[:, 0::16] = 0.0
    n = np.arange(N_FFT, dtype=np.float64)
    f = np.arange(257, dtype=np.float64)
    ang = 2.0 * np.pi * n[:, None] * f[None, :] / N_FFT
    cosm = np.cos(ang).astype(np.float32)
    sinm = (-np.sin(ang)).astype(np.float32)
    fbm = _mel_fbanks_np()
    t = np.arange(WIN_LEN, dtype=np.float32)
    win = (0.5 * (1.0 - np.cos(2.0 * np.pi * t / WIN_LEN))).astype(np.float32)
    pad_l = (N_FFT - WIN_LEN) // 2
    win_p = np.zeros(N_FFT, np.float32)
    win_p[pad_l:pad_l + WIN_LEN] = win
    winm = win_p.reshape(4, 128).T.copy()
    return mask, cosm, sinm, fbm, winm


def kernel(pdm_bits, taps, scale):
    from concourse.bass_utils import run_bass_kernel_spmd

    pdm = np.asarray(pdm_bits, dtype=np.int32)
    taps_l = [int(x) for x in np.asarray(taps).tolist()]
    key = (tuple(taps_l), int(scale))
    if key not in _COMPILED:
        _COMPILED[key] = _build(taps_l, int(scale))
    nc = _COMPILED[key]

    mask, cosm, sinm, fbm, winm = _constants()
    shards = pdm.reshape(NCORE, P, FREE)
    in_maps = []
    for c in range(NCORE):
        f15 = np.ones((1, L), np.float32)
        if c == 0:
            f15[:] = 0.0
        phi = (4 * c) % 16
        r1 = (472500 * c - phi) // 16
        in_maps.append({
            "pdm": shards[c],
            "mask": mask, "cosm": cosm, "sinm": sinm, "fbm": fbm,
            "winm": winm, "f15": f15,
            "phiv": np.array([[phi]], np.int32),
            "r1v": np.array([[r1]], np.int32),
            "p16v": np.array([[16 - phi]], np.int32),
            "r1m1": np.array([[max(r1 - 1, 0)]], np.int32),
            "v0m": np.array([[1.0 if (phi == 0 and c != 0) else 0.0]],
                            np.float32),
        })
    res = run_bass_kernel_spmd(nc, in_maps, list(range(NCORE)))
    global _LAST_RES
    _LAST_RES = res
    outs = [res.results[c]["out"] for c in range(NCORE)]
    full = np.concatenate(outs, axis=1)[:, :T_FRAMES]
    return full[None, None].astype(np.float32)


# revision 25
# speedup vs baseline: 731.4523x; 8.8218x over previous
"""AudioFrontend Trainium2 kernel: PDM -> CIC(f32 blk16-exact) -> FIR(int64) -> logmel.

Bit-exact replication of jax-CPU float32 cumsum (XLA ReduceWindowRewriter
base-16 blocked scans) through the chaotic CIC stages, exact int64 FIR via
12-bit limbs on gpsimd int32, then matmul STFT/mel/log.
Self-contained: hardcodes all shapes; host code only shards/gathers.
"""
import numpy as np

NCORE = 8
N_PDM = 60_480_000
PERCORE = N_PDM // NCORE          # 7,560,000
P = 125
FREE = PERCORE // P               # 60480
TILE_F = 4032                     # 63*64 = 16*252
NT = FREE // TILE_F               # 15
ROWS_T = TILE_F // 16             # 252
ROWS_P = FREE // 16               # 3780
T0_LOC = P * ROWS_P               # 472500
T0_GLOB = NCORE * T0_LOC          # 3780000
GF = T0_GLOB // P                 # 30240
GCH = 5040
NGC = GF // GCH                   # 6
T1R = GCH // 16                   # 315
T1N = T0_GLOB // 16               # 236250
T1PAD = 236256                    # 123*1920 + 96
T2N = T1PAD // 16                 # 14766
T2PAD = 14768
T3N = T2PAD // 16                 # 923
T3PAD = 928
T4N = T3PAD // 16                 # 58
T4PAD = 64
T5N = T4PAD // 16                 # 4
DECIM = 63
DEC_PC = PERCORE // DECIM         # 120000
DEC_PP = FREE // DECIM            # 960
DEC_T = TILE_F // DECIM           # 64
L = 15
CHALO = 19
NBH = 384
N_FFT = 512
HOP = 160
WIN_LEN = 400
NMEL = 40
FR_PC = 750
T_FRAMES = 1 + (N_PDM // DECIM - N_FFT) // HOP  # 5997
FH = 375
SAT = 9.223372036854775808e18


def _mel_fbanks_np():
    n_freqs = N_FFT // 2 + 1
    all_freqs = np.linspace(0.0, 16000 / 2, n_freqs)
    h2m = lambda f: 2595.0 * np.log10(1.0 + f / 700.0)
    m_pts = np.linspace(h2m(0.0), h2m(8000.0), NMEL + 2)
    f_pts = 700.0 * (10.0 ** (m_pts / 2595.0) - 1.0)
    f_diff = f_pts[1:] - f_pts[:-1]
    slopes = f_pts[None, :] - all_freqs[:, None]
    down = -slopes[:, :-2] / f_diff[:-1]
    up = slopes[:, 2:] / f_diff[1:]
    return np.maximum(0.0, np.minimum(down, up)).astype(np.float32)


_COMPILED = {}
_LAST_RES = None


def _build(taps_list, scale_int):
    import concourse.bass as bass
    import concourse.bacc as bacc
    import concourse.mybir as mybir
    import concourse.tile as tile

    dt = mybir.dt
    A = mybir.AluOpType
    ACTF = mybir.ActivationFunctionType

    nc = bacc.Bacc()
    pdm_in = nc.declare_dram_parameter("pdm", [P, FREE], dt.int32, isOutput=False)
    mask_in = nc.declare_dram_parameter("mask", [128, 5136], dt.float32, isOutput=False)
    cos_in = nc.declare_dram_parameter("cosm", [N_FFT, 257], dt.float32, isOutput=False)
    sin_in = nc.declare_dram_parameter("sinm", [N_FFT, 257], dt.float32, isOutput=False)
    fb_in = nc.declare_dram_parameter("fbm", [257, NMEL], dt.float32, isOutput=False)
    win_in = nc.declare_dram_parameter("winm", [128, 4], dt.float32, isOutput=False)
    f15_in = nc.declare_dram_parameter("f15", [1, L], dt.float32, isOutput=False)
    phi_in = nc.declare_dram_parameter("phiv", [1, 1], dt.int32, isOutput=False)
    r1_in = nc.declare_dram_parameter("r1v", [1, 1], dt.int32, isOutput=False)
    p16_in = nc.declare_dram_parameter("p16v", [1, 1], dt.int32, isOutput=False)
    r1m_in = nc.declare_dram_parameter("r1m1", [1, 1], dt.int32, isOutput=False)
    v0m_in = nc.declare_dram_parameter("v0m", [1, 1], dt.float32, isOutput=False)
    out_p = nc.declare_dram_parameter("out", [NMEL, FR_PC], dt.float32, isOutput=True)

    pA = nc.dram_tensor("pA", [P, FREE], dt.float32)
    pB = nc.dram_tensor("pB", [P, FREE], dt.float32)
    t0loc = nc.dram_tensor("t0loc", [1, T0_LOC], dt.float32)
    e0buf = nc.dram_tensor("e0buf", [1, 16 + T0_LOC + 16], dt.float32)
    h16i = nc.dram_tensor("h16i", [1, 32], dt.float32)
    h16o = nc.dram_tensor("h16o", [NCORE, 32], dt.float32)
    h16p = nc.dram_tensor("h16p", [NCORE + 2, 32], dt.float32)
    t1agi = nc.dram_tensor("t1agi", [1, 29532], dt.float32)
    t1ago = nc.dram_tensor("t1ago", [NCORE, 29532], dt.float32)
    vloc = nc.dram_tensor("vloc", [1, 1 + 472512], dt.float32)
    t1buf = nc.dram_tensor("t1buf", [1, T1PAD], dt.float32)
    t2buf = nc.dram_tensor("t2buf", [1, T2PAD], dt.float32)
    zbuf = nc.dram_tensor("zbuf", [1, 1 + T2PAD], dt.float32)
    zsbuf = nc.dram_tensor("zsbuf", [1, 1 + T1PAD], dt.float32)
    decb = nc.dram_tensor("decb", [1, CHALO + DEC_PC], dt.float32)
    ybuf = nc.dram_tensor("ybuf", [1, DEC_PC + NBH], dt.float32)
    h19i = nc.dram_tensor("h19i", [1, CHALO], dt.float32)
    h19o = nc.dram_tensor("h19o", [NCORE, CHALO], dt.float32)
    h19p = nc.dram_tensor("h19p", [NCORE + 1, CHALO], dt.float32)
    hnbi = nc.dram_tensor("hnbi", [1, NBH], dt.float32)
    hnbo = nc.dram_tensor("hnbo", [NCORE, NBH], dt.float32)
    hnbp = nc.dram_tensor("hnbp", [NCORE + 1, NBH], dt.float32)
    taild = nc.dram_tensor("taild", [1, CHALO + NBH], dt.float32)
    t3d = nc.dram_tensor("t3d", [1, T3N], dt.float32)
    z3buf = nc.dram_tensor("z3buf", [1, 1 + T3PAD], dt.float32)

    RG = [list(range(NCORE))]
    PHIS = [(4 * c) % 16 for c in range(NCORE)]
    R1S = [(T0_LOC * c - PHIS[c]) // 16 for c in range(NCORE)]
    MCS = [(R1S[c + 1] - R1S[c]) if c + 1 < NCORE else (T1N - R1S[c])
           for c in range(NCORE)]
    taps = [int(t) for t in taps_list]
    assert (1 << 15) == int(scale_int)

    with tile.TileContext(nc) as tc:
        pid = nc.gpsimd.partition_id()

        # ============ scan stages ============
        with tc.tile_pool(name="persist", bufs=1) as pp:
            mask = pp.tile([128, 5136], dt.float32)
            nc.sync.dma_start(mask[:], mask_in[:])
            nc.vector.tensor_copy(mask[:1, :1], mask[:1, :1])
            t0sb = pp.tile([P, ROWS_P], dt.float32)
            carry0 = pp.tile([P, ROWS_P], dt.float32)
            decsb = pp.tile([P, DEC_PP], dt.float32)

            for st in range(5):
                src = [None, pA, pB, pA, pB][st]
                dst = [pA, pB, pA, pB, pA][st]
                with tc.tile_pool(name=f"s{st}", bufs=3) as sp:
                    for t in range(NT):
                        fs = slice(t * TILE_F, (t + 1) * TILE_F)
                        if st == 0:
                            raw = sp.tile([P, TILE_F], dt.int32, tag="raw")
                            nc.sync.dma_start(raw[:], pdm_in[:, fs])
                            xt = sp.tile([P, TILE_F], dt.float32, tag="xt")
                            nc.scalar.activation(xt[:], raw[:], ACTF.Copy,
                                                 bias=-1.0, scale=2.0)
                        else:
                            pin = sp.tile([P, TILE_F], dt.float32, tag="pin")
                            nc.sync.dma_start(pin[:], src[:, fs])
                            nc.vector.tensor_copy(pin[:1, :1], pin[:1, :1])
                            xt = sp.tile([P, TILE_F], dt.float32, tag="xt")
                            rs = slice(t * ROWS_T, (t + 1) * ROWS_T)
                            nc.vector.tensor_tensor(
                                xt[:].rearrange("p (r s) -> p r s", s=16),
                                pin[:].rearrange("p (r s) -> p r s", s=16),
                                carry0[:, rs].broadcast_to([P, ROWS_T, 16]),
                                A.add)
                        po = sp.tile([P, TILE_F], dt.float32, tag="po")
                        nc.vector.tensor_tensor_scan(
                            po[:], mask[:P, :TILE_F], xt[:], 0.0, A.mult, A.add)
                        nc.vector.tensor_copy(
                            t0sb[:, t * ROWS_T:(t + 1) * ROWS_T], po[:, 15::16])
                        nc.sync.dma_start(dst[:, fs], po[:])

                nc.sync.dma_start(h16i[0, :16], t0sb[:1, :16])
                nc.sync.dma_start(h16i[0, 16:], t0sb[P - 1:P, ROWS_P - 16:])
                nc.gpsimd.collective_compute(
                    "AllGather", A.bypass, replica_groups=RG,
                    ins=[h16i[:]], outs=[h16o[:]])
                # padded halo rows: row0=AG7, rows1..8=AG0..7, row9=AG0
                nc.sync.dma_start(h16p[0, :], h16o[NCORE - 1, :])
                nc.sync.dma_start(h16p[1:NCORE + 1, :], h16o[:])
                nc.sync.dma_start(h16p[NCORE + 1, :], h16o[0, :])
                # e0: [left16 halo][own t0][right16 halo]
                nc.gpsimd.dma_start(e0buf[0, :16], h16p[:][pid, 16:])
                nc.sync.dma_start(
                    bass.AP(e0buf, 16, [[ROWS_P, P], [1, ROWS_P]]), t0sb[:])
                nc.gpsimd.dma_start(
                    e0buf[0, 16 + T0_LOC:], h16p[:][pid + 2, :16])

                with tc.tile_pool(name=f"g{st}", bufs=1) as gp:
                    phir = nc.gpsimd.alloc_register(f"phir{st}")
                    nc.gpsimd.reg_load(phir, phi_in[:1, :1])
                    r1r = nc.gpsimd.alloc_register(f"r1r{st}")
                    nc.gpsimd.reg_load(r1r, r1_in[:1, :1])
                    p16r = nc.gpsimd.alloc_register(f"p16r{st}")
                    nc.gpsimd.reg_load(p16r, p16_in[:1, :1])
                    r1mr = nc.gpsimd.alloc_register(f"r1mr{st}")
                    nc.gpsimd.reg_load(r1mr, r1m_in[:1, :1])
                    # local extended-t0 scan: [92, 5136] from e0buf
                    ge = gp.tile([92, 5136], dt.float32, tag="ge")
                    nc.gpsimd.dma_start(
                        ge[:],
                        bass.AP(e0buf, bass.make_scalar_value(p16r),
                                [[5136, 92], [1, 5136]]))
                    nc.vector.tensor_copy(ge[:1, :1], ge[:1, :1])
                    gs = gp.tile([92, 5136], dt.float32, tag="gs")
                    nc.vector.tensor_tensor_scan(
                        gs[:], mask[:92, :5136], ge[:], 0.0, A.mult, A.add)
                    tx = gp.tile([92, 321], dt.float32, tag="tx")
                    nc.vector.tensor_copy(tx[:], gs[:, 15::16])
                    nc.sync.dma_start(
                        bass.AP(t1agi, 0, [[321, 92], [1, 321]]), tx[:])
                    nc.gpsimd.collective_compute(
                        "AllGather", A.bypass, replica_groups=RG,
                        ins=[t1agi[:]], outs=[t1ago[:]])
                    # compact ragged t1 via SBUF bounce; full-width copies in
                    # forward order so each overwrites the previous overhang
                    for c in range(NCORE):
                        cb = gp.tile([12, 2461], dt.float32, tag="cb")
                        nc.sync.dma_start(
                            cb[:], bass.AP(t1ago, c * 29532,
                                           [[2461, 12], [1, 2461]]))
                        nc.sync.dma_start(
                            bass.AP(t1buf, R1S[c], [[2461, 12], [1, 2461]]),
                            cb[:])
                    zt = gp.tile([1, 16], dt.float32, tag="zt")
                    nc.vector.memset(zt[:], 0.0)
                    nc.sync.dma_start(t1buf[0, T1N:T1PAD], zt[:1, :T1PAD - T1N])
                    nc.sync.dma_start(zbuf[0, :1], zt[:1, :1])
                    nc.sync.dma_start(zsbuf[0, :1], zt[:1, :1])

                    u1a = gp.tile([123, 1920], dt.float32, tag="u1a")
                    nc.sync.dma_start(
                        u1a[:], bass.AP(t1buf, 0, [[1920, 123], [1, 1920]]))
                    nc.vector.tensor_copy(u1a[:1, :1], u1a[:1, :1])
                    p2a = gp.tile([123, 1920], dt.float32, tag="p2a")
                    nc.vector.tensor_tensor_scan(
                        p2a[:], mask[:123, :1920], u1a[:], 0.0, A.mult, A.add)
                    u1b = gp.tile([1, 96], dt.float32, tag="u1b")
                    nc.sync.dma_start(u1b[:], t1buf[0, 123 * 1920:T1PAD])
                    nc.vector.tensor_copy(u1b[:1, :1], u1b[:1, :1])
                    p2b = gp.tile([1, 96], dt.float32, tag="p2b")
                    nc.vector.tensor_tensor_scan(
                        p2b[:], mask[:1, :96], u1b[:], 0.0, A.mult, A.add)
                    t2a = gp.tile([123, 120], dt.float32, tag="t2a")
                    nc.vector.tensor_copy(t2a[:], p2a[:, 15::16])
                    nc.sync.dma_start(
                        bass.AP(t2buf, 0, [[120, 123], [1, 120]]), t2a[:])
                    t2b = gp.tile([1, 6], dt.float32, tag="t2b")
                    nc.vector.tensor_copy(t2b[:], p2b[:, 15::16])
                    nc.sync.dma_start(t2buf[0, 123 * 120:T2N], t2b[:1, :])
                    nc.sync.dma_start(t2buf[0, T2N:T2PAD], zt[:1, :T2PAD - T2N])

                    u2 = gp.tile([13, 1136], dt.float32, tag="u2")
                    nc.sync.dma_start(
                        u2[:], bass.AP(t2buf, 0, [[1136, 13], [1, 1136]]))
                    nc.vector.tensor_copy(u2[:1, :1], u2[:1, :1])
                    p3 = gp.tile([13, 1136], dt.float32, tag="p3")
                    nc.vector.tensor_tensor_scan(
                        p3[:], mask[:13, :1136], u2[:], 0.0, A.mult, A.add)
                    t3x = gp.tile([13, 71], dt.float32, tag="t3x")
                    nc.vector.tensor_copy(t3x[:], p3[:, 15::16])
                    nc.sync.dma_start(
                        bass.AP(t3d, 0, [[71, 13], [1, 71]]), t3x[:])
                    u3 = gp.tile([1, T3PAD], dt.float32, tag="u3")
                    nc.vector.memset(u3[:], 0.0)
                    nc.sync.dma_start(u3[:1, :T3N], t3d[0, :])
                    nc.vector.tensor_copy(u3[:1, :1], u3[:1, :1])
                    p4 = gp.tile([1, T3PAD], dt.float32, tag="p4")
                    nc.vector.tensor_tensor_scan(
                        p4[:], mask[:1, :T3PAD], u3[:], 0.0, A.mult, A.add)
                    u4 = gp.tile([1, T4PAD], dt.float32, tag="u4")
                    nc.vector.memset(u4[:], 0.0)
                    nc.vector.tensor_copy(u4[:, :T4N], p4[:, 15::16])
                    p5 = gp.tile([1, T4PAD], dt.float32, tag="p5")
                    nc.vector.tensor_tensor_scan(
                        p5[:], mask[:1, :T4PAD], u4[:], 0.0, A.mult, A.add)
                    u5 = gp.tile([1, T5N], dt.float32, tag="u5")
                    nc.vector.tensor_copy(u5[:], p5[:, 15::16])
                    s5 = gp.tile([1, T5N], dt.float32, tag="s5")
                    nc.vector.tensor_tensor_scan(
                        s5[:], mask[:1, :T5N], u5[:], 0.0, A.mult, A.add)
                    nc.vector.tensor_tensor(
                        p5[:, 16:].rearrange("p (r s) -> p r s", s=16),
                        p5[:, 16:].rearrange("p (r s) -> p r s", s=16),
                        s5[:, :3].broadcast_to([1, 3, 16]), A.add)
                    nc.vector.tensor_tensor(
                        p4[:, 16:].rearrange("p (r s) -> p r s", s=16),
                        p4[:, 16:].rearrange("p (r s) -> p r s", s=16),
                        p5[:, :T4N - 1].broadcast_to([1, T4N - 1, 16]), A.add)
                    nc.sync.dma_start(z3buf[0, :1], zt[:1, :1])
                    nc.sync.dma_start(z3buf[0, 1:1 + T3PAD], p4[:1, :])
                    cz3 = gp.tile([13, 71], dt.float32, tag="t3x")
                    nc.sync.dma_start(
                        cz3[:], bass.AP(z3buf, 0, [[71, 13], [1, 71]]))
                    nc.vector.tensor_copy(cz3[:1, :1], cz3[:1, :1])
                    nc.vector.tensor_tensor(
                        p3[:].rearrange("p (r s) -> p r s", s=16),
                        p3[:].rearrange("p (r s) -> p r s", s=16),
                        cz3[:].broadcast_to([13, 71, 16]), A.add)
                    nc.sync.dma_start(
                        bass.AP(zbuf, 1, [[1136, 13], [1, 1136]]), p3[:])
                    cza = gp.tile([123, 120], dt.float32, tag="cza")
                    nc.sync.dma_start(
                        cza[:], bass.AP(zbuf, 0, [[120, 123], [1, 120]]))
                    nc.vector.tensor_copy(cza[:1, :1], cza[:1, :1])
                    nc.vector.tensor_tensor(
                        p2a[:].rearrange("p (r s) -> p r s", s=16),
                        p2a[:].rearrange("p (r s) -> p r s", s=16),
                        cza[:].broadcast_to([123, 120, 16]), A.add)
                    czb = gp.tile([1, 6], dt.float32, tag="czb")
                    nc.sync.dma_start(czb[:], zbuf[0, 123 * 120:123 * 120 + 6])
                    nc.vector.tensor_copy(czb[:1, :1], czb[:1, :1])
                    nc.vector.tensor_tensor(
                        p2b[:].rearrange("p (r s) -> p r s", s=16),
                        p2b[:].rearrange("p (r s) -> p r s", s=16),
                        czb[:].broadcast_to([1, 6, 16]), A.add)
                    nc.sync.dma_start(
                        bass.AP(zsbuf, 1, [[1920, 123], [1, 1920]]), p2a[:])
                    nc.sync.dma_start(
                        zsbuf[0, 1 + 123 * 1920:1 + T1PAD], p2b[:1, :])

                    # own scan_t0: p1_local + bcast(Zs at own rows)
                    ctb = gp.tile([92, 321], dt.float32, tag="tx")
                    nc.gpsimd.dma_start(
                        ctb[:],
                        bass.AP(zsbuf, bass.make_scalar_value(r1r),
                                [[321, 92], [1, 321]]))
                    nc.vector.tensor_copy(ctb[:1, :1], ctb[:1, :1])
                    nc.vector.tensor_tensor(
                        gs[:].rearrange("p (r s) -> p r s", s=16),
                        gs[:].rearrange("p (r s) -> p r s", s=16),
                        ctb[:].broadcast_to([92, 321, 16]), A.add)
                    nc.sync.dma_start(
                        bass.AP(vloc, 1, [[5136, 92], [1, 5136]]), gs[:])
                    # vloc[0]: 0 normally; for the phi==0 mid core (c=4):
                    # scan_t0[A_c - 1] = t1[A_c/16 - 1] + scan_t1[A_c/16 - 2]
                    sv = gp.tile([1, 2], dt.float32, tag="sv")
                    nc.gpsimd.dma_start(
                        sv[:1, :1],
                        bass.AP(t1buf, bass.make_scalar_value(r1mr),
                                [[1, 1], [1, 1]]))
                    nc.gpsimd.dma_start(
                        sv[:1, 1:],
                        bass.AP(zsbuf, bass.make_scalar_value(r1mr),
                                [[1, 1], [1, 1]]))
                    v0t = gp.tile([1, 1], dt.float32, tag="v0t")
                    nc.sync.dma_start(v0t[:], v0m_in[:])
                    nc.vector.tensor_copy(v0t[:1, :1], v0t[:1, :1])
                    sv2 = gp.tile([1, 1], dt.float32, tag="sv2")
                    nc.vector.tensor_tensor(sv2[:], sv[:1, :1], sv[:1, 1:],
                                            A.add)
                    nc.vector.tensor_tensor(sv2[:], sv2[:], v0t[:], A.mult)
                    nc.sync.dma_start(vloc[0, :1], sv2[:1, :])
                    nc.gpsimd.dma_start(
                        carry0[:],
                        bass.AP(vloc, bass.make_scalar_value(phir),
                                [[ROWS_P, P], [1, ROWS_P]]))
                    nc.vector.tensor_copy(carry0[:1, :1], carry0[:1, :1])

            # ============ phase B of stage 5: decimate ============
            with tc.tile_pool(name="pb5", bufs=3) as sp:
                for t in range(NT):
                    fs = slice(t * TILE_F, (t + 1) * TILE_F)
                    pin = sp.tile([P, TILE_F], dt.float32, tag="pin")
                    nc.sync.dma_start(pin[:], pA[:, fs])
                    nc.vector.tensor_copy(pin[:1, :1], pin[:1, :1])
                    xt = sp.tile([P, TILE_F], dt.float32, tag="xt")
                    rs = slice(t * ROWS_T, (t + 1) * ROWS_T)
                    nc.vector.tensor_tensor(
                        xt[:].rearrange("p (r s) -> p r s", s=16),
                        pin[:].rearrange("p (r s) -> p r s", s=16),
                        carry0[:, rs].broadcast_to([P, ROWS_T, 16]), A.add)
                    nc.vector.tensor_copy(
                        decsb[:, t * DEC_T:(t + 1) * DEC_T], xt[:, 0::DECIM])

            nc.sync.dma_start(
                bass.AP(decb, CHALO, [[DEC_PP, P], [1, DEC_PP]]), decsb[:])
            nc.sync.dma_start(h19i[:1, :], decsb[P - 1:P, DEC_PP - CHALO:])
            nc.gpsimd.collective_compute(
                "AllGather", A.bypass, replica_groups=RG,
                ins=[h19i[:]], outs=[h19o[:]])
            nc.sync.dma_start(hnbi[:1, :], decsb[:1, :NBH])
            nc.gpsimd.collective_compute(
                "AllGather", A.bypass, replica_groups=RG,
                ins=[hnbi[:]], outs=[hnbo[:]])
            with tc.tile_pool(name="hx", bufs=1) as hp:
                zt2 = hp.tile([1, NBH], dt.float32)
                nc.vector.memset(zt2[:], 0.0)
                nc.sync.dma_start(h19p[0, :], zt2[:1, :CHALO])
                nc.sync.dma_start(h19p[1:, :], h19o[:])
                nc.sync.dma_start(hnbp[:NCORE, :], hnbo[:])
                nc.sync.dma_start(hnbp[NCORE, :], hnbo[0, :])
                nc.gpsimd.dma_start(decb[0, :CHALO], h19p[:][pid, :])

        # ============ comb + FIR + y ============
        def emit_comb_fir(dp, src_ap, np_, width, ydst, ybase, yrstride):
            A_ = A
            NL = 16
            cmb = dp.tile([np_, width], dt.float32, tag="cmb0")
            nc.sync.dma_start(cmb[:], src_ap)
            nc.vector.tensor_copy(cmb[:1, :1], cmb[:1, :1])
            cur = cmb
            w = width
            for it in range(5):
                nxt = dp.tile([np_, w - 1], dt.float32, tag=f"cmb{1 + it % 2}")
                nc.vector.tensor_tensor(
                    nxt[:], cur[:, 1:w], cur[:, :w - 1], A_.subtract)
                cur = nxt
                w -= 1
            nw_full = w
            ny_full = nw_full - (L - 1)
            CW = 320 if np_ > 1 else ny_full
            nch = ny_full // CW
            assert nch * CW == ny_full
            for ch in range(nch):
                cw = cur[:, ch * CW:ch * CW + CW + (L - 1)]
                nw = CW + (L - 1)
                ny = CW
                satp = dp.tile([np_, nw], dt.float32, tag="satp")
                nc.vector.tensor_scalar(satp[:], cw, SAT, None, A_.is_ge)
                satn = dp.tile([np_, nw], dt.float32, tag="satn")
                nc.vector.tensor_scalar(satn[:], cw, -SAT, None, A_.is_le)
                sgn = dp.tile([np_, nw], dt.float32, tag="sgn")
                nc.vector.tensor_scalar(sgn[:], cw, 0.0, None, A_.is_ge)
                nc.vector.tensor_scalar(sgn[:], sgn[:], 2.0, -1.0, A_.mult,
                                        A_.add)
                mag = dp.tile([np_, nw], dt.float32, tag="mag")
                nc.vector.tensor_tensor(mag[:], cw, sgn[:], A_.mult)
                rs_ = dp.tile([np_, nw], dt.float32, tag="rs")
                nc.vector.tensor_tensor(rs_[:], satp[:], satn[:], A_.add)
                nc.vector.tensor_scalar(rs_[:], rs_[:], -1.0, 1.0, A_.mult,
                                        A_.add)
                nc.vector.tensor_tensor(rs_[:], rs_[:], sgn[:], A_.mult)
                bits = dp.tile([np_, nw], dt.int32, tag="bits")
                nc.vector.tensor_copy(bits[:], mag[:].bitcast(dt.int32))
                ex = dp.tile([np_, nw], dt.int32, tag="ex")
                nc.vector.tensor_scalar(ex[:], bits[:], 23, None,
                                        A_.logical_shift_right)
                nc.vector.tensor_scalar(ex[:], ex[:], 255, None, A_.bitwise_and)
                nc.vector.tensor_scalar(ex[:], ex[:], -150, None, A_.add)
                mant = dp.tile([np_, nw], dt.int32, tag="mant")
                nc.vector.tensor_scalar(mant[:], bits[:], 0x7FFFFF, 0x800000,
                                        A_.bitwise_and, A_.bitwise_or)
                nzm = dp.tile([np_, nw], dt.int32, tag="nzm")
                nc.vector.tensor_scalar(nzm[:], ex[:], -23, None, A_.is_ge)
                nc.vector.tensor_tensor(mant[:], mant[:], nzm[:], A_.mult)
                tmpa = dp.tile([np_, nw], dt.int32, tag="tmpa")
                tmpb = dp.tile([np_, nw], dt.int32, tag="tmpb")
                tmpr = dp.tile([np_, nw], dt.int32, tag="tmpr")
                tmps = dp.tile([np_, nw], dt.int32, tag="tmps")
                sel = dp.tile([np_, nw], dt.int32, tag="sel")
                limbs = []
                for j in range(NL):
                    nc.vector.tensor_scalar(tmpr[:], ex[:], -1, 4 * j, A_.mult,
                                            A_.add)
                    nc.vector.tensor_scalar(tmps[:], tmpr[:], 31, None, A_.min)
                    nc.vector.tensor_scalar(tmps[:], tmps[:], 0, None, A_.max)
                    nc.vector.tensor_tensor(tmpa[:], mant[:], tmps[:],
                                            A_.logical_shift_right)
                    nc.vector.tensor_scalar(tmpa[:], tmpa[:], 15, None,
                                            A_.bitwise_and)
                    nc.vector.tensor_scalar(tmps[:], tmpr[:], -1, 0, A_.mult,
                                            A_.max)
                    nc.vector.tensor_scalar(tmps[:], tmps[:], 3, None, A_.min)
                    nc.vector.tensor_tensor(tmpb[:], mant[:], tmps[:],
                                            A_.logical_shift_left)
                    nc.vector.tensor_scalar(tmpb[:], tmpb[:], 15, None,
                                            A_.bitwise_and)
                    nc.vector.tensor_scalar(sel[:], tmpr[:], 0, None, A_.is_ge)
                    li = dp.tile([np_, nw], dt.int32, tag="li")
                    nc.vector.select(li[:], sel[:], tmpa[:], tmpb[:])
                    lf = dp.tile([np_, nw], dt.float32, tag=f"lf{j}")
                    nc.vector.tensor_copy(lf[:], li[:])
                    nc.vector.tensor_tensor(lf[:], lf[:], rs_[:], A_.mult)
                    limbs.append(lf)
                accA = dp.tile([np_, ny], dt.float32, tag="accA")
                accB = dp.tile([np_, ny], dt.float32, tag="accB")
                for k in range(L):
                    o = L - 1 - k
                    if k == 0:
                        nc.vector.tensor_scalar(accA[:], satp[:, o:o + ny],
                                                float(taps[k]), None, A_.mult)
                        nc.vector.tensor_scalar(accB[:], satn[:, o:o + ny],
                                                float(taps[k]), None, A_.mult)
                    else:
                        nc.vector.scalar_tensor_tensor(
                            accA[:], satp[:, o:o + ny], float(taps[k]),
                            accA[:], A_.mult, A_.add)
                        nc.vector.scalar_tensor_tensor(
                            accB[:], satn[:, o:o + ny], float(taps[k]),
                            accB[:], A_.mult, A_.add)
                cols = []
                for j in range(NL):
                    cj = dp.tile([np_, ny], dt.float32, tag=f"c{j}")
                    nc.vector.tensor_scalar(cj[:],
                                            limbs[j][:, L - 1:L - 1 + ny],
                                            float(taps[0]), None, A_.mult)
                    for k in range(1, L):
                        o = L - 1 - k
                        nc.vector.scalar_tensor_tensor(
                            cj[:], limbs[j][:, o:o + ny], float(taps[k]),
                            cj[:], A_.mult, A_.add)
                    cols.append(cj)
                ai = dp.tile([np_, ny], dt.int32, tag="ai")
                nc.vector.tensor_copy(ai[:], accA[:])
                bi = dp.tile([np_, ny], dt.int32, tag="bi")
                nc.vector.tensor_copy(bi[:], accB[:])
                nc.vector.tensor_tensor(ai[:], ai[:], bi[:], A_.subtract)
                nc.vector.tensor_scalar(ai[:], ai[:], 1, 3, A_.bitwise_and,
                                        A_.logical_shift_left)
                parf = dp.tile([np_, ny], dt.float32, tag="parf")
                nc.vector.tensor_copy(parf[:], ai[:])
                nc.vector.tensor_tensor(cols[0][:], cols[0][:], accA[:],
                                        A_.subtract)
                nc.vector.tensor_tensor(cols[15][:], cols[15][:], parf[:],
                                        A_.add)
                carry = dp.tile([np_, ny], dt.int32, tag="cy")
                vj = dp.tile([np_, ny], dt.int32, tag="vj")
                lmb = [None] * NL
                for j in range(NL):
                    nc.vector.tensor_copy(vj[:], cols[j][:])
                    if j > 0:
                        nc.vector.tensor_tensor(vj[:], vj[:], carry[:], A_.add)
                    if j < NL - 1:
                        nc.vector.tensor_scalar(carry[:], vj[:], 4, None,
                                                A_.arith_shift_right)
                    if j >= 3:
                        mj = dp.tile([np_, ny], dt.int32, tag=f"m{j}")
                        nc.vector.tensor_scalar(mj[:], vj[:], 15, None,
                                                A_.bitwise_and)
                        lmb[j] = mj
                l24 = dp.tile([np_, ny], dt.int32, tag="l24")
                hh = dp.tile([np_, ny], dt.int32, tag="hh")
                def gather_bits(dst, pieces):
                    first = True
                    for (src, shr, andm, shl) in pieces:
                        if andm is not None:
                            nc.vector.tensor_scalar(tmpa[:, :ny], src[:], andm,
                                                    shl, A_.bitwise_and,
                                                    A_.logical_shift_left)
                        elif shr > 0:
                            nc.vector.tensor_scalar(tmpa[:, :ny], src[:], shr,
                                                    None,
                                                    A_.logical_shift_right)
                        else:
                            nc.vector.tensor_scalar(tmpa[:, :ny], src[:], shl,
                                                    None,
                                                    A_.logical_shift_left)
                        if first:
                            nc.vector.tensor_copy(dst[:], tmpa[:, :ny])
                            first = False
                        else:
                            nc.vector.tensor_tensor(dst[:], dst[:],
                                                    tmpa[:, :ny], A_.bitwise_or)
                gather_bits(l24, [(lmb[3], 3, None, 0), (lmb[4], 0, None, 1),
                                  (lmb[5], 0, None, 5), (lmb[6], 0, None, 9),
                                  (lmb[7], 0, None, 13), (lmb[8], 0, None, 17),
                                  (lmb[9], 0, 7, 21)])
                gather_bits(hh, [(lmb[9], 3, None, 0), (lmb[10], 0, None, 1),
                                 (lmb[11], 0, None, 5), (lmb[12], 0, None, 9),
                                 (lmb[13], 0, None, 13),
                                 (lmb[14], 0, None, 17),
                                 (lmb[15], 0, 7, 21)])
                s63 = dp.tile([np_, ny], dt.int32, tag="s63")
                nc.vector.tensor_scalar(s63[:], lmb[15][:], 3, 1,
                                        A_.logical_shift_right, A_.bitwise_and)
                s63f = dp.tile([np_, ny], dt.float32, tag="s63f")
                nc.vector.tensor_copy(s63f[:], s63[:])
                hf = dp.tile([np_, ny], dt.float32, tag="hf")
                nc.vector.tensor_copy(hf[:], hh[:])
                nc.vector.scalar_tensor_tensor(
                    hf[:], s63f[:], -16777216.0, hf[:], A_.mult, A_.add)
                lf24 = dp.tile([np_, ny], dt.float32, tag="lf24")
                nc.vector.tensor_copy(lf24[:], l24[:])
                yv = dp.tile([np_, ny], dt.float32, tag="yv")
                nc.vector.scalar_tensor_tensor(
                    yv[:], hf[:], 16777216.0, lf24[:], A_.mult, A_.add)
                nc.sync.dma_start(
                    bass.AP(ydst, ybase + ch * CW, [[yrstride, np_], [1, ny]]),
                    yv[:])

        with tc.tile_pool(name="fir", bufs=1) as dp:
            emit_comb_fir(
                dp, bass.AP(decb, 0, [[DEC_PP, P], [1, DEC_PP + CHALO]]),
                P, DEC_PP + CHALO, ybuf, 0, DEC_PP)
            f15 = dp.tile([1, L], dt.float32, tag="f15t")
            nc.sync.dma_start(f15[:], f15_in[:])
            y15 = dp.tile([1, L], dt.float32, tag="y15")
            nc.sync.dma_start(y15[:], ybuf[0, :L])
            nc.vector.tensor_copy(y15[:1, :1], y15[:1, :1])
            nc.vector.tensor_tensor(y15[:], y15[:], f15[:], A.mult)
            nc.sync.dma_start(ybuf[0, :L], y15[:1, :])
            tl = dp.tile([1, CHALO + NBH], dt.float32, tag="tl")
            nc.sync.dma_start(tl[:, :CHALO], decb[0, DEC_PC:DEC_PC + CHALO])
            nc.gpsimd.dma_start(tl[:, CHALO:], hnbp[:][pid + 1, :])
            nc.sync.dma_start(taild[:], tl[:])
            emit_comb_fir(dp, taild[:], 1, CHALO + NBH, ybuf, DEC_PC, 1)

        # ============ STFT + mel + log ============
        with (tc.tile_pool(name="stft", bufs=1) as fp,
              tc.tile_pool(name="psum", bufs=1, space="PSUM") as psp):
            wint = fp.tile([128, 4], dt.float32)
            nc.sync.dma_start(wint[:], win_in[:])
            nc.vector.tensor_copy(wint[:1, :1], wint[:1, :1])
            cosm = fp.tile([128, 4 * 257], dt.float32)
            nc.sync.dma_start(
                cosm[:].rearrange("p (k f) -> p k f", f=257),
                bass.AP(cos_in, 0, [[257, 128], [128 * 257, 4], [1, 257]]))
            nc.vector.tensor_copy(cosm[:1, :1], cosm[:1, :1])
            sinm = fp.tile([128, 4 * 257], dt.float32)
            nc.sync.dma_start(
                sinm[:].rearrange("p (k f) -> p k f", f=257),
                bass.AP(sin_in, 0, [[257, 128], [128 * 257, 4], [1, 257]]))
            nc.vector.tensor_copy(sinm[:1, :1], sinm[:1, :1])
            fbm = fp.tile([128, 2 * NMEL], dt.float32)
            nc.sync.dma_start(
                fbm[:].rearrange("p (k f) -> p k f", f=NMEL),
                bass.AP(fb_in, 0, [[NMEL, 128], [128 * NMEL, 2], [1, NMEL]]))
            nc.vector.tensor_copy(fbm[:1, :1], fbm[:1, :1])
            fbm2 = fp.tile([1, NMEL], dt.float32)
            nc.sync.dma_start(fbm2[:], bass.AP(fb_in, 256 * NMEL, [[NMEL, 1], [1, NMEL]]))
            nc.vector.tensor_copy(fbm2[:1, :1], fbm2[:1, :1])
            xts = []
            for k in range(4):
                xk = fp.tile([128, FR_PC], dt.float32, tag=f"xk{k}")
                nc.sync.dma_start(
                    xk[:], bass.AP(ybuf, 128 * k, [[1, 128], [HOP, FR_PC]]))
                nc.vector.tensor_copy(xk[:1, :1], xk[:1, :1])
                nc.vector.tensor_scalar(xk[:], xk[:], wint[:, k:k + 1], None,
                                        A.mult)
                xts.append(xk)
            pw0 = fp.tile([128, FR_PC], dt.float32, tag="pw0")
            pw1 = fp.tile([128, FR_PC], dt.float32, tag="pw1")
            pw2 = fp.tile([1, FR_PC], dt.float32, tag="pw2")
            pwr = [pw0, pw1, pw2]
            fcs = [(0, 128), (128, 256), (256, 257)]
            for fi, (f0, f1) in enumerate(fcs):
                for h in range(2):
                    hs = slice(h * FH, (h + 1) * FH)
                    pc = psp.tile([f1 - f0, FH], dt.float32, tag="pc")
                    ps = psp.tile([f1 - f0, FH], dt.float32, tag="ps")
                    for k in range(4):
                        nc.tensor.matmul(
                            pc[:], cosm[:, 257 * k + f0:257 * k + f1],
                            xts[k][:, hs], start=(k == 0), stop=(k == 3))
                    for k in range(4):
                        nc.tensor.matmul(
                            ps[:], sinm[:, 257 * k + f0:257 * k + f1],
                            xts[k][:, hs], start=(k == 0), stop=(k == 3))
                    t1_ = fp.tile([128, FH], dt.float32, tag="sq1")
                    nc.scalar.activation(t1_[:f1 - f0], pc[:], ACTF.Square)
                    t2_ = fp.tile([128, FH], dt.float32, tag="sq2")
                    nc.scalar.activation(t2_[:f1 - f0], ps[:], ACTF.Square)
                    nc.vector.tensor_tensor(pwr[fi][:, hs], t1_[:f1 - f0],
                                            t2_[:f1 - f0], A.add)
            lm = fp.tile([NMEL, FR_PC], dt.float32, tag="lm")
            for h in range(2):
                hs = slice(h * FH, (h + 1) * FH)
                mm = psp.tile([NMEL, FH], dt.float32, tag="mm")
                nc.tensor.matmul(mm[:], fbm[:, :NMEL], pwr[0][:, hs],
                                 start=True, stop=False)
                nc.tensor.matmul(mm[:], fbm[:, NMEL:], pwr[1][:, hs],
                                 start=False, stop=False)
                nc.tensor.matmul(mm[:], fbm2[:, :], pwr[2][:, hs],
                                 start=False, stop=True)
                xs = fp.tile([NMEL, FH], dt.float32, tag="xs")
                nc.vector.tensor_scalar(xs[:], mm[:], 1e-6, None, A.add)
                bx = fp.tile([NMEL, FH], dt.int32, tag="bx")
                nc.vector.tensor_copy(bx[:], xs[:].bitcast(dt.int32))
                ev = fp.tile([NMEL, FH], dt.int32, tag="ev")
                nc.vector.tensor_scalar(ev[:], bx[:], 23, None,
                                        A.logical_shift_right)
                nc.vector.tensor_scalar(ev[:], ev[:], -127, None, A.add)
                evf = fp.tile([NMEL, FH], dt.float32, tag="evf")
                nc.vector.tensor_copy(evf[:], ev[:])
                nc.vector.tensor_scalar(bx[:], bx[:], 0x7FFFFF, 127 << 23,
                                        A.bitwise_and, A.bitwise_or)
                lnm = fp.tile([NMEL, FH], dt.float32, tag="lnm")
                nc.scalar.activation(lnm[:], bx[:].bitcast(dt.float32), ACTF.Ln)
                nc.vector.scalar_tensor_tensor(
                    lm[:, hs], evf[:], 0.6931471805599453, lnm[:],
                    A.mult, A.add)
            nc.sync.dma_start(out_p[:], lm[:])

    nc.compile()
    return nc


def _constants():
    mask = np.ones((128, 5136), np.float32)
    mask[:, 0::16] = 0.0
    n = np.arange(N_FFT, dtype=np.float64)
    f = np.arange(257, dtype=np.float64)
    ang = 2.0 * np.pi * n[:, None] * f[None, :] / N_FFT
    cosm = np.cos(ang).astype(np.float32)
    sinm = (-np.sin(ang)).astype(np.float32)
    fbm = _mel_fbanks_np()
    t = np.arange(WIN_LEN, dtype=np.float32)
    win = (0.5 * (1.0 - np.cos(2.0 * np.pi * t / WIN_LEN))).astype(np.float32)
    pad_l = (N_FFT - WIN_LEN) // 2
    win_p = np.zeros(N_FFT, np.float32)
    win_p[pad_l:pad_l + WIN_LEN] = win
    winm = win_p.reshape(4, 128).T.copy()
    return mask, cosm, sinm, fbm, winm


def kernel(pdm_bits, taps, scale):
    from concourse.bass_utils import run_bass_kernel_spmd

    pdm = np.asarray(pdm_bits, dtype=np.int32)
    taps_l = [int(x) for x in np.asarray(taps).tolist()]
    key = (tuple(taps_l), int(scale))
    if key not in _COMPILED:
        _COMPILED[key] = _build(taps_l, int(scale))
    nc = _COMPILED[key]

    mask, cosm, sinm, fbm, winm = _constants()
    shards = pdm.reshape(NCORE, P, FREE)
    in_maps = []
    for c in range(NCORE):
        f15 = np.ones((1, L), np.float32)
        if c == 0:
            f15[:] = 0.0
        phi = (4 * c) % 16
        r1 = (472500 * c - phi) // 16
        in_maps.append({
            "pdm": shards[c],
            "mask": mask, "cosm": cosm, "sinm": sinm, "fbm": fbm,
            "winm": winm, "f15": f15,
            "phiv": np.array([[phi]], np.int32),
            "r1v": np.array([[r1]], np.int32),
            "p16v": np.array([[16 - phi]], np.int32),
            "r1m1": np.array([[max(r1 - 1, 0)]], np.int32),
            "v0m": np.array([[1.0 if (phi == 0 and c != 0) else 0.0]],
                            np.float32),
        })
    res = run_bass_kernel_spmd(nc, in_maps, list(range(NCORE)))
    global _LAST_RES
    _LAST_RES = res
    outs = [res.results[c]["out"] for c in range(NCORE)]
    full = np.concatenate(outs, axis=1)[:, :T_FRAMES]
    return full[None, None].astype(np.float32)


# revision 28
# speedup vs baseline: 732.4262x; 1.0013x over previous
"""AudioFrontend Trainium2 kernel: PDM -> CIC(f32 blk16-exact) -> FIR(int64) -> logmel.

Bit-exact replication of jax-CPU float32 cumsum (XLA ReduceWindowRewriter
base-16 blocked scans) through the chaotic CIC stages, exact int64 FIR via
12-bit limbs on gpsimd int32, then matmul STFT/mel/log.
Self-contained: hardcodes all shapes; host code only shards/gathers.
"""
import numpy as np

NCORE = 8
N_PDM = 60_480_000
PERCORE = N_PDM // NCORE          # 7,560,000
P = 125
FREE = PERCORE // P               # 60480
TILE_F = 4032                     # 63*64 = 16*252
NT = FREE // TILE_F               # 15
ROWS_T = TILE_F // 16             # 252
ROWS_P = FREE // 16               # 3780
T0_LOC = P * ROWS_P               # 472500
T0_GLOB = NCORE * T0_LOC          # 3780000
GF = T0_GLOB // P                 # 30240
GCH = 5040
NGC = GF // GCH                   # 6
T1R = GCH // 16                   # 315
T1N = T0_GLOB // 16               # 236250
T1PAD = 236256                    # 123*1920 + 96
T2N = T1PAD // 16                 # 14766
T2PAD = 14768
T3N = T2PAD // 16                 # 923
T3PAD = 928
T4N = T3PAD // 16                 # 58
T4PAD = 64
T5N = T4PAD // 16                 # 4
DECIM = 63
DEC_PC = PERCORE // DECIM         # 120000
DEC_PP = FREE // DECIM            # 960
DEC_T = TILE_F // DECIM           # 64
L = 15
CHALO = 19
NBH = 384
N_FFT = 512
HOP = 160
WIN_LEN = 400
NMEL = 40
FR_PC = 750
T_FRAMES = 1 + (N_PDM // DECIM - N_FFT) // HOP  # 5997
FH = 375
SAT = 9.223372036854775808e18


def _mel_fbanks_np():
    n_freqs = N_FFT // 2 + 1
    all_freqs = np.linspace(0.0, 16000 / 2, n_freqs)
    h2m = lambda f: 2595.0 * np.log10(1.0 + f / 700.0)
    m_pts = np.linspace(h2m(0.0), h2m(8000.0), NMEL + 2)
    f_pts = 700.0 * (10.0 ** (m_pts / 2595.0) - 1.0)
    f_diff = f_pts[1:] - f_pts[:-1]
    slopes = f_pts[None, :] - all_freqs[:, None]
    down = -slopes[:, :-2] / f_diff[:-1]
    up = slopes[:, 2:] / f_diff[1:]
    return np.maximum(0.0, np.minimum(down, up)).astype(np.float32)


_COMPILED = {}
_LAST_RES = None


def _build(taps_list, scale_int):
    import concourse.bass as bass
    import concourse.bacc as bacc
    import concourse.mybir as mybir
    import concourse.tile as tile

    dt = mybir.dt
    A = mybir.AluOpType
    ACTF = mybir.ActivationFunctionType

    nc = bacc.Bacc()
    pdm_in = nc.declare_dram_parameter("pdm", [P, FREE], dt.int32, isOutput=False)
    mask_in = nc.declare_dram_parameter("mask", [128, 5136], dt.float32, isOutput=False)
    cos_in = nc.declare_dram_parameter("cosm", [N_FFT, 257], dt.float32, isOutput=False)
    sin_in = nc.declare_dram_parameter("sinm", [N_FFT, 257], dt.float32, isOutput=False)
    fb_in = nc.declare_dram_parameter("fbm", [257, NMEL], dt.float32, isOutput=False)
    win_in = nc.declare_dram_parameter("winm", [128, 4], dt.float32, isOutput=False)
    f15_in = nc.declare_dram_parameter("f15", [1, L], dt.float32, isOutput=False)
    phi_in = nc.declare_dram_parameter("phiv", [1, 1], dt.int32, isOutput=False)
    r1_in = nc.declare_dram_parameter("r1v", [1, 1], dt.int32, isOutput=False)
    p16_in = nc.declare_dram_parameter("p16v", [1, 1], dt.int32, isOutput=False)
    r1m_in = nc.declare_dram_parameter("r1m1", [1, 1], dt.int32, isOutput=False)
    v0m_in = nc.declare_dram_parameter("v0m", [1, 1], dt.float32, isOutput=False)
    out_p = nc.declare_dram_parameter("out", [NMEL, FR_PC], dt.float32, isOutput=True)

    pA = nc.dram_tensor("pA", [P, FREE], dt.float32)
    pB = nc.dram_tensor("pB", [P, FREE], dt.float32)
    t0loc = nc.dram_tensor("t0loc", [1, T0_LOC], dt.float32)
    e0buf = nc.dram_tensor("e0buf", [1, 16 + T0_LOC + 16], dt.float32)
    h16i = nc.dram_tensor("h16i", [1, 32], dt.float32)
    h16o = nc.dram_tensor("h16o", [NCORE, 32], dt.float32)
    h16p = nc.dram_tensor("h16p", [NCORE + 2, 32], dt.float32)
    t1agi = nc.dram_tensor("t1agi", [1, 29532], dt.float32)
    t1ago = nc.dram_tensor("t1ago", [NCORE, 29532], dt.float32)
    vloc = nc.dram_tensor("vloc", [1, 1 + 472512], dt.float32)
    t1buf = nc.dram_tensor("t1buf", [1, T1PAD], dt.float32)
    t2buf = nc.dram_tensor("t2buf", [1, T2PAD], dt.float32)
    zbuf = nc.dram_tensor("zbuf", [1, 1 + T2PAD], dt.float32)
    zsbuf = nc.dram_tensor("zsbuf", [1, 1 + T1PAD], dt.float32)
    decb = nc.dram_tensor("decb", [1, CHALO + DEC_PC], dt.float32)
    ybuf = nc.dram_tensor("ybuf", [1, DEC_PC + NBH], dt.float32)
    h19i = nc.dram_tensor("h19i", [1, CHALO], dt.float32)
    h19o = nc.dram_tensor("h19o", [NCORE, CHALO], dt.float32)
    h19p = nc.dram_tensor("h19p", [NCORE + 1, CHALO], dt.float32)
    hnbi = nc.dram_tensor("hnbi", [1, NBH], dt.float32)
    hnbo = nc.dram_tensor("hnbo", [NCORE, NBH], dt.float32)
    hnbp = nc.dram_tensor("hnbp", [NCORE + 1, NBH], dt.float32)
    taild = nc.dram_tensor("taild", [1, CHALO + NBH], dt.float32)
    t3d = nc.dram_tensor("t3d", [1, T3N], dt.float32)
    z3buf = nc.dram_tensor("z3buf", [1, 1 + T3PAD], dt.float32)

    RG = [list(range(NCORE))]
    PHIS = [(4 * c) % 16 for c in range(NCORE)]
    R1S = [(T0_LOC * c - PHIS[c]) // 16 for c in range(NCORE)]
    MCS = [(R1S[c + 1] - R1S[c]) if c + 1 < NCORE else (T1N - R1S[c])
           for c in range(NCORE)]
    taps = [int(t) for t in taps_list]
    assert (1 << 15) == int(scale_int)

    with tile.TileContext(nc) as tc:
        pid = nc.gpsimd.partition_id()

        # ============ scan stages ============
        with tc.tile_pool(name="persist", bufs=1) as pp:
            mask = pp.tile([128, 5136], dt.float32)
            nc.sync.dma_start(mask[:], mask_in[:])
            nc.vector.tensor_copy(mask[:1, :1], mask[:1, :1])
            t0sb = pp.tile([P, ROWS_P], dt.float32)
            carry0 = pp.tile([P, ROWS_P], dt.float32)
            decsb = pp.tile([P, DEC_PP], dt.float32)

            for st in range(5):
                src = [None, pA, pB, pA, pB][st]
                dst = [pA, pB, pA, pB, pA][st]
                with tc.tile_pool(name=f"s{st}", bufs=3) as sp:
                    for t in range(NT):
                        fs = slice(t * TILE_F, (t + 1) * TILE_F)
                        if st == 0:
                            raw = sp.tile([P, TILE_F], dt.int32, tag="raw")
                            nc.sync.dma_start(raw[:], pdm_in[:, fs])
                            xt = sp.tile([P, TILE_F], dt.float32, tag="xt")
                            nc.scalar.activation(xt[:], raw[:], ACTF.Copy,
                                                 bias=-1.0, scale=2.0)
                        else:
                            pin = sp.tile([P, TILE_F], dt.float32, tag="pin")
                            nc.sync.dma_start(pin[:], src[:, fs])
                            nc.vector.tensor_copy(pin[:1, :1], pin[:1, :1])
                            xt = sp.tile([P, TILE_F], dt.float32, tag="xt")
                            rs = slice(t * ROWS_T, (t + 1) * ROWS_T)
                            nc.vector.tensor_tensor(
                                xt[:].rearrange("p (r s) -> p r s", s=16),
                                pin[:].rearrange("p (r s) -> p r s", s=16),
                                carry0[:, rs].broadcast_to([P, ROWS_T, 16]),
                                A.add)
                        po = sp.tile([P, TILE_F], dt.float32, tag="po")
                        nc.vector.tensor_tensor_scan(
                            po[:], mask[:P, :TILE_F], xt[:], 0.0, A.mult, A.add)
                        nc.vector.tensor_copy(
                            t0sb[:, t * ROWS_T:(t + 1) * ROWS_T], po[:, 15::16])
                        nc.scalar.dma_start(dst[:, fs], po[:])

                nc.sync.dma_start(h16i[0, :16], t0sb[:1, :16])
                nc.sync.dma_start(h16i[0, 16:], t0sb[P - 1:P, ROWS_P - 16:])
                nc.gpsimd.collective_compute(
                    "AllGather", A.bypass, replica_groups=RG,
                    ins=[h16i[:]], outs=[h16o[:]])
                # padded halo rows: row0=AG7, rows1..8=AG0..7, row9=AG0
                nc.sync.dma_start(h16p[0, :], h16o[NCORE - 1, :])
                nc.sync.dma_start(h16p[1:NCORE + 1, :], h16o[:])
                nc.sync.dma_start(h16p[NCORE + 1, :], h16o[0, :])
                # e0: [left16 halo][own t0][right16 halo]
                nc.gpsimd.dma_start(e0buf[0, :16], h16p[:][pid, 16:])
                nc.sync.dma_start(
                    bass.AP(e0buf, 16, [[ROWS_P, P], [1, ROWS_P]]), t0sb[:])
                nc.gpsimd.dma_start(
                    e0buf[0, 16 + T0_LOC:], h16p[:][pid + 2, :16])

                with tc.tile_pool(name=f"g{st}", bufs=1) as gp:
                    phir = nc.gpsimd.alloc_register(f"phir{st}")
                    nc.gpsimd.reg_load(phir, phi_in[:1, :1])
                    r1r = nc.gpsimd.alloc_register(f"r1r{st}")
                    nc.gpsimd.reg_load(r1r, r1_in[:1, :1])
                    p16r = nc.gpsimd.alloc_register(f"p16r{st}")
                    nc.gpsimd.reg_load(p16r, p16_in[:1, :1])
                    r1mr = nc.gpsimd.alloc_register(f"r1mr{st}")
                    nc.gpsimd.reg_load(r1mr, r1m_in[:1, :1])
                    # local extended-t0 scan: [92, 5136] from e0buf
                    ge = gp.tile([92, 5136], dt.float32, tag="ge")
                    nc.gpsimd.dma_start(
                        ge[:],
                        bass.AP(e0buf, bass.make_scalar_value(p16r),
                                [[5136, 92], [1, 5136]]))
                    nc.vector.tensor_copy(ge[:1, :1], ge[:1, :1])
                    gs = gp.tile([92, 5136], dt.float32, tag="gs")
                    nc.vector.tensor_tensor_scan(
                        gs[:], mask[:92, :5136], ge[:], 0.0, A.mult, A.add)
                    tx = gp.tile([92, 321], dt.float32, tag="tx")
                    nc.vector.tensor_copy(tx[:], gs[:, 15::16])
                    nc.sync.dma_start(
                        bass.AP(t1agi, 0, [[321, 92], [1, 321]]), tx[:])
                    nc.gpsimd.collective_compute(
                        "AllGather", A.bypass, replica_groups=RG,
                        ins=[t1agi[:]], outs=[t1ago[:]])
                    # compact ragged t1 via SBUF bounce; full-width copies in
                    # forward order so each overwrites the previous overhang
                    for c in range(NCORE):
                        cb = gp.tile([12, 2461], dt.float32, tag="cb")
                        nc.sync.dma_start(
                            cb[:], bass.AP(t1ago, c * 29532,
                                           [[2461, 12], [1, 2461]]))
                        nc.sync.dma_start(
                            bass.AP(t1buf, R1S[c], [[2461, 12], [1, 2461]]),
                            cb[:])
                    zt = gp.tile([1, 16], dt.float32, tag="zt")
                    nc.vector.memset(zt[:], 0.0)
                    nc.sync.dma_start(t1buf[0, T1N:T1PAD], zt[:1, :T1PAD - T1N])
                    nc.sync.dma_start(zbuf[0, :1], zt[:1, :1])
                    nc.sync.dma_start(zsbuf[0, :1], zt[:1, :1])

                    u1a = gp.tile([123, 1920], dt.float32, tag="u1a")
                    nc.sync.dma_start(
                        u1a[:], bass.AP(t1buf, 0, [[1920, 123], [1, 1920]]))
                    nc.vector.tensor_copy(u1a[:1, :1], u1a[:1, :1])
                    p2a = gp.tile([123, 1920], dt.float32, tag="p2a")
                    nc.vector.tensor_tensor_scan(
                        p2a[:], mask[:123, :1920], u1a[:], 0.0, A.mult, A.add)
                    u1b = gp.tile([1, 96], dt.float32, tag="u1b")
                    nc.sync.dma_start(u1b[:], t1buf[0, 123 * 1920:T1PAD])
                    nc.vector.tensor_copy(u1b[:1, :1], u1b[:1, :1])
                    p2b = gp.tile([1, 96], dt.float32, tag="p2b")
                    nc.vector.tensor_tensor_scan(
                        p2b[:], mask[:1, :96], u1b[:], 0.0, A.mult, A.add)
                    t2a = gp.tile([123, 120], dt.float32, tag="t2a")
                    nc.vector.tensor_copy(t2a[:], p2a[:, 15::16])
                    nc.sync.dma_start(
                        bass.AP(t2buf, 0, [[120, 123], [1, 120]]), t2a[:])
                    t2b = gp.tile([1, 6], dt.float32, tag="t2b")
                    nc.vector.tensor_copy(t2b[:], p2b[:, 15::16])
                    nc.sync.dma_start(t2buf[0, 123 * 120:T2N], t2b[:1, :])
                    nc.sync.dma_start(t2buf[0, T2N:T2PAD], zt[:1, :T2PAD - T2N])

                    u2 = gp.tile([13, 1136], dt.float32, tag="u2")
                    nc.sync.dma_start(
                        u2[:], bass.AP(t2buf, 0, [[1136, 13], [1, 1136]]))
                    nc.vector.tensor_copy(u2[:1, :1], u2[:1, :1])
                    p3 = gp.tile([13, 1136], dt.float32, tag="p3")
                    nc.vector.tensor_tensor_scan(
                        p3[:], mask[:13, :1136], u2[:], 0.0, A.mult, A.add)
                    t3x = gp.tile([13, 71], dt.float32, tag="t3x")
                    nc.vector.tensor_copy(t3x[:], p3[:, 15::16])
                    nc.sync.dma_start(
                        bass.AP(t3d, 0, [[71, 13], [1, 71]]), t3x[:])
                    u3 = gp.tile([1, T3PAD], dt.float32, tag="u3")
                    nc.vector.memset(u3[:], 0.0)
                    nc.sync.dma_start(u3[:1, :T3N], t3d[0, :])
                    nc.vector.tensor_copy(u3[:1, :1], u3[:1, :1])
                    p4 = gp.tile([1, T3PAD], dt.float32, tag="p4")
                    nc.vector.tensor_tensor_scan(
                        p4[:], mask[:1, :T3PAD], u3[:], 0.0, A.mult, A.add)
                    u4 = gp.tile([1, T4PAD], dt.float32, tag="u4")
                    nc.vector.memset(u4[:], 0.0)
                    nc.vector.tensor_copy(u4[:, :T4N], p4[:, 15::16])
                    p5 = gp.tile([1, T4PAD], dt.float32, tag="p5")
                    nc.vector.tensor_tensor_scan(
                        p5[:], mask[:1, :T4PAD], u4[:], 0.0, A.mult, A.add)
                    u5 = gp.tile([1, T5N], dt.float32, tag="u5")
                    nc.vector.tensor_copy(u5[:], p5[:, 15::16])
                    s5 = gp.tile([1, T5N], dt.float32, tag="s5")
                    nc.vector.tensor_tensor_scan(
                        s5[:], mask[:1, :T5N], u5[:], 0.0, A.mult, A.add)
                    nc.vector.tensor_tensor(
                        p5[:, 16:].rearrange("p (r s) -> p r s", s=16),
                        p5[:, 16:].rearrange("p (r s) -> p r s", s=16),
                        s5[:, :3].broadcast_to([1, 3, 16]), A.add)
                    nc.vector.tensor_tensor(
                        p4[:, 16:].rearrange("p (r s) -> p r s", s=16),
                        p4[:, 16:].rearrange("p (r s) -> p r s", s=16),
                        p5[:, :T4N - 1].broadcast_to([1, T4N - 1, 16]), A.add)
                    nc.sync.dma_start(z3buf[0, :1], zt[:1, :1])
                    nc.sync.dma_start(z3buf[0, 1:1 + T3PAD], p4[:1, :])
                    cz3 = gp.tile([13, 71], dt.float32, tag="t3x")
                    nc.sync.dma_start(
                        cz3[:], bass.AP(z3buf, 0, [[71, 13], [1, 71]]))
                    nc.vector.tensor_copy(cz3[:1, :1], cz3[:1, :1])
                    nc.vector.tensor_tensor(
                        p3[:].rearrange("p (r s) -> p r s", s=16),
                        p3[:].rearrange("p (r s) -> p r s", s=16),
                        cz3[:].broadcast_to([13, 71, 16]), A.add)
                    nc.sync.dma_start(
                        bass.AP(zbuf, 1, [[1136, 13], [1, 1136]]), p3[:])
                    cza = gp.tile([123, 120], dt.float32, tag="cza")
                    nc.sync.dma_start(
                        cza[:], bass.AP(zbuf, 0, [[120, 123], [1, 120]]))
                    nc.vector.tensor_copy(cza[:1, :1], cza[:1, :1])
                    nc.vector.tensor_tensor(
                        p2a[:].rearrange("p (r s) -> p r s", s=16),
                        p2a[:].rearrange("p (r s) -> p r s", s=16),
                        cza[:].broadcast_to([123, 120, 16]), A.add)
                    czb = gp.tile([1, 6], dt.float32, tag="czb")
                    nc.sync.dma_start(czb[:], zbuf[0, 123 * 120:123 * 120 + 6])
                    nc.vector.tensor_copy(czb[:1, :1], czb[:1, :1])
                    nc.vector.tensor_tensor(
                        p2b[:].rearrange("p (r s) -> p r s", s=16),
                        p2b[:].rearrange("p (r s) -> p r s", s=16),
                        czb[:].broadcast_to([1, 6, 16]), A.add)
                    nc.sync.dma_start(
                        bass.AP(zsbuf, 1, [[1920, 123], [1, 1920]]), p2a[:])
                    nc.sync.dma_start(
                        zsbuf[0, 1 + 123 * 1920:1 + T1PAD], p2b[:1, :])

                    # own scan_t0: p1_local + bcast(Zs at own rows)
                    ctb = gp.tile([92, 321], dt.float32, tag="tx")
                    nc.gpsimd.dma_start(
                        ctb[:],
                        bass.AP(zsbuf, bass.make_scalar_value(r1r),
                                [[321, 92], [1, 321]]))
                    nc.vector.tensor_copy(ctb[:1, :1], ctb[:1, :1])
                    nc.vector.tensor_tensor(
                        gs[:].rearrange("p (r s) -> p r s", s=16),
                        gs[:].rearrange("p (r s) -> p r s", s=16),
                        ctb[:].broadcast_to([92, 321, 16]), A.add)
                    nc.sync.dma_start(
                        bass.AP(vloc, 1, [[5136, 92], [1, 5136]]), gs[:])
                    # vloc[0]: 0 normally; for the phi==0 mid core (c=4):
                    # scan_t0[A_c - 1] = t1[A_c/16 - 1] + scan_t1[A_c/16 - 2]
                    sv = gp.tile([1, 2], dt.float32, tag="sv")
                    nc.gpsimd.dma_start(
                        sv[:1, :1],
                        bass.AP(t1buf, bass.make_scalar_value(r1mr),
                                [[1, 1], [1, 1]]))
                    nc.gpsimd.dma_start(
                        sv[:1, 1:],
                        bass.AP(zsbuf, bass.make_scalar_value(r1mr),
                                [[1, 1], [1, 1]]))
                    v0t = gp.tile([1, 1], dt.float32, tag="v0t")
                    nc.sync.dma_start(v0t[:], v0m_in[:])
                    nc.vector.tensor_copy(v0t[:1, :1], v0t[:1, :1])
                    sv2 = gp.tile([1, 1], dt.float32, tag="sv2")
                    nc.vector.tensor_tensor(sv2[:], sv[:1, :1], sv[:1, 1:],
                                            A.add)
                    nc.vector.tensor_tensor(sv2[:], sv2[:], v0t[:], A.mult)
                    nc.sync.dma_start(vloc[0, :1], sv2[:1, :])
                    nc.gpsimd.dma_start(
                        carry0[:],
                        bass.AP(vloc, bass.make_scalar_value(phir),
                                [[ROWS_P, P], [1, ROWS_P]]))
                    nc.vector.tensor_copy(carry0[:1, :1], carry0[:1, :1])

            # ============ phase B of stage 5: decimate ============
            with tc.tile_pool(name="pb5", bufs=3) as sp:
                for t in range(NT):
                    fs = slice(t * TILE_F, (t + 1) * TILE_F)
                    pin = sp.tile([P, TILE_F], dt.float32, tag="pin")
                    nc.sync.dma_start(pin[:], pA[:, fs])
                    nc.vector.tensor_copy(pin[:1, :1], pin[:1, :1])
                    xt = sp.tile([P, TILE_F], dt.float32, tag="xt")
                    rs = slice(t * ROWS_T, (t + 1) * ROWS_T)
                    nc.vector.tensor_tensor(
                        xt[:].rearrange("p (r s) -> p r s", s=16),
                        pin[:].rearrange("p (r s) -> p r s", s=16),
                        carry0[:, rs].broadcast_to([P, ROWS_T, 16]), A.add)
                    nc.vector.tensor_copy(
                        decsb[:, t * DEC_T:(t + 1) * DEC_T], xt[:, 0::DECIM])

            nc.sync.dma_start(
                bass.AP(decb, CHALO, [[DEC_PP, P], [1, DEC_PP]]), decsb[:])
            nc.sync.dma_start(h19i[:1, :], decsb[P - 1:P, DEC_PP - CHALO:])
            nc.gpsimd.collective_compute(
                "AllGather", A.bypass, replica_groups=RG,
                ins=[h19i[:]], outs=[h19o[:]])
            nc.sync.dma_start(hnbi[:1, :], decsb[:1, :NBH])
            nc.gpsimd.collective_compute(
                "AllGather", A.bypass, replica_groups=RG,
                ins=[hnbi[:]], outs=[hnbo[:]])
            with tc.tile_pool(name="hx", bufs=1) as hp:
                zt2 = hp.tile([1, NBH], dt.float32)
                nc.vector.memset(zt2[:], 0.0)
                nc.sync.dma_start(h19p[0, :], zt2[:1, :CHALO])
                nc.sync.dma_start(h19p[1:, :], h19o[:])
                nc.sync.dma_start(hnbp[:NCORE, :], hnbo[:])
                nc.sync.dma_start(hnbp[NCORE, :], hnbo[0, :])
                nc.gpsimd.dma_start(decb[0, :CHALO], h19p[:][pid, :])

        # ============ comb + FIR + y ============
        def emit_comb_fir(dp, src_ap, np_, width, ydst, ybase, yrstride):
            A_ = A
            NL = 16
            cmb = dp.tile([np_, width], dt.float32, tag="cmb0")
            nc.sync.dma_start(cmb[:], src_ap)
            nc.vector.tensor_copy(cmb[:1, :1], cmb[:1, :1])
            cur = cmb
            w = width
            for it in range(5):
                nxt = dp.tile([np_, w - 1], dt.float32, tag=f"cmb{1 + it % 2}")
                nc.vector.tensor_tensor(
                    nxt[:], cur[:, 1:w], cur[:, :w - 1], A_.subtract)
                cur = nxt
                w -= 1
            nw_full = w
            ny_full = nw_full - (L - 1)
            CW = 320 if np_ > 1 else ny_full
            nch = ny_full // CW
            assert nch * CW == ny_full
            for ch in range(nch):
                cw = cur[:, ch * CW:ch * CW + CW + (L - 1)]
                nw = CW + (L - 1)
                ny = CW
                satp = dp.tile([np_, nw], dt.float32, tag="satp")
                nc.vector.tensor_scalar(satp[:], cw, SAT, None, A_.is_ge)
                satn = dp.tile([np_, nw], dt.float32, tag="satn")
                nc.vector.tensor_scalar(satn[:], cw, -SAT, None, A_.is_le)
                sgn = dp.tile([np_, nw], dt.float32, tag="sgn")
                nc.vector.tensor_scalar(sgn[:], cw, 0.0, None, A_.is_ge)
                nc.vector.tensor_scalar(sgn[:], sgn[:], 2.0, -1.0, A_.mult,
                                        A_.add)
                mag = dp.tile([np_, nw], dt.float32, tag="mag")
                nc.vector.tensor_tensor(mag[:], cw, sgn[:], A_.mult)
                rs_ = dp.tile([np_, nw], dt.float32, tag="rs")
                nc.vector.tensor_tensor(rs_[:], satp[:], satn[:], A_.add)
                nc.vector.tensor_scalar(rs_[:], rs_[:], -1.0, 1.0, A_.mult,
                                        A_.add)
                nc.vector.tensor_tensor(rs_[:], rs_[:], sgn[:], A_.mult)
                bits = dp.tile([np_, nw], dt.int32, tag="bits")
                nc.vector.tensor_copy(bits[:], mag[:].bitcast(dt.int32))
                ex = dp.tile([np_, nw], dt.int32, tag="ex")
                nc.vector.tensor_scalar(ex[:], bits[:], 23, None,
                                        A_.logical_shift_right)
                nc.vector.tensor_scalar(ex[:], ex[:], 255, None, A_.bitwise_and)
                nc.vector.tensor_scalar(ex[:], ex[:], -150, None, A_.add)
                mant = dp.tile([np_, nw], dt.int32, tag="mant")
                nc.vector.tensor_scalar(mant[:], bits[:], 0x7FFFFF, 0x800000,
                                        A_.bitwise_and, A_.bitwise_or)
                nzm = dp.tile([np_, nw], dt.int32, tag="nzm")
                nc.vector.tensor_scalar(nzm[:], ex[:], -23, None, A_.is_ge)
                nc.vector.tensor_tensor(mant[:], mant[:], nzm[:], A_.mult)
                tmpa = dp.tile([np_, nw], dt.int32, tag="tmpa")
                tmpb = dp.tile([np_, nw], dt.int32, tag="tmpb")
                tmpr = dp.tile([np_, nw], dt.int32, tag="tmpr")
                tmps = dp.tile([np_, nw], dt.int32, tag="tmps")
                sel = dp.tile([np_, nw], dt.int32, tag="sel")
                limbs = []
                for j in range(NL):
                    nc.vector.tensor_scalar(tmpr[:], ex[:], -1, 4 * j, A_.mult,
                                            A_.add)
                    nc.vector.tensor_scalar(tmps[:], tmpr[:], 31, None, A_.min)
                    nc.vector.tensor_scalar(tmps[:], tmps[:], 0, None, A_.max)
                    nc.vector.tensor_tensor(tmpa[:], mant[:], tmps[:],
                                            A_.logical_shift_right)
                    nc.vector.tensor_scalar(tmpa[:], tmpa[:], 15, None,
                                            A_.bitwise_and)
                    nc.vector.tensor_scalar(tmps[:], tmpr[:], -1, 0, A_.mult,
                                            A_.max)
                    nc.vector.tensor_scalar(tmps[:], tmps[:], 3, None, A_.min)
                    nc.vector.tensor_tensor(tmpb[:], mant[:], tmps[:],
                                            A_.logical_shift_left)
                    nc.vector.tensor_scalar(tmpb[:], tmpb[:], 15, None,
                                            A_.bitwise_and)
                    nc.vector.tensor_scalar(sel[:], tmpr[:], 0, None, A_.is_ge)
                    li = dp.tile([np_, nw], dt.int32, tag="li")
                    nc.vector.select(li[:], sel[:], tmpa[:], tmpb[:])
                    lf = dp.tile([np_, nw], dt.float32, tag=f"lf{j}")
                    nc.vector.tensor_copy(lf[:], li[:])
                    nc.vector.tensor_tensor(lf[:], lf[:], rs_[:], A_.mult)
                    limbs.append(lf)
                accA = dp.tile([np_, ny], dt.float32, tag="accA")
                accB = dp.tile([np_, ny], dt.float32, tag="accB")
                for k in range(L):
                    o = L - 1 - k
                    if k == 0:
                        nc.vector.tensor_scalar(accA[:], satp[:, o:o + ny],
                                                float(taps[k]), None, A_.mult)
                        nc.vector.tensor_scalar(accB[:], satn[:, o:o + ny],
                                                float(taps[k]), None, A_.mult)
                    else:
                        nc.vector.scalar_tensor_tensor(
                            accA[:], satp[:, o:o + ny], float(taps[k]),
                            accA[:], A_.mult, A_.add)
                        nc.vector.scalar_tensor_tensor(
                            accB[:], satn[:, o:o + ny], float(taps[k]),
                            accB[:], A_.mult, A_.add)
                cols = []
                for j in range(NL):
                    cj = dp.tile([np_, ny], dt.float32, tag=f"c{j}")
                    nc.vector.tensor_scalar(cj[:],
                                            limbs[j][:, L - 1:L - 1 + ny],
                                            float(taps[0]), None, A_.mult)
                    for k in range(1, L):
                        o = L - 1 - k
                        nc.vector.scalar_tensor_tensor(
                            cj[:], limbs[j][:, o:o + ny], float(taps[k]),
                            cj[:], A_.mult, A_.add)
                    cols.append(cj)
                ai = dp.tile([np_, ny], dt.int32, tag="ai")
                nc.vector.tensor_copy(ai[:], accA[:])
                bi = dp.tile([np_, ny], dt.int32, tag="bi")
                nc.vector.tensor_copy(bi[:], accB[:])
                nc.vector.tensor_tensor(ai[:], ai[:], bi[:], A_.subtract)
                nc.vector.tensor_scalar(ai[:], ai[:], 1, 3, A_.bitwise_and,
                                        A_.logical_shift_left)
                parf = dp.tile([np_, ny], dt.float32, tag="parf")
                nc.vector.tensor_copy(parf[:], ai[:])
                nc.vector.tensor_tensor(cols[0][:], cols[0][:], accA[:],
                                        A_.subtract)
                nc.vector.tensor_tensor(cols[15][:], cols[15][:], parf[:],
                                        A_.add)
                carry = dp.tile([np_, ny], dt.int32, tag="cy")
                vj = dp.tile([np_, ny], dt.int32, tag="vj")
                lmb = [None] * NL
                for j in range(NL):
                    nc.vector.tensor_copy(vj[:], cols[j][:])
                    if j > 0:
                        nc.vector.tensor_tensor(vj[:], vj[:], carry[:], A_.add)
                    if j < NL - 1:
                        nc.vector.tensor_scalar(carry[:], vj[:], 4, None,
                                                A_.arith_shift_right)
                    if j >= 3:
                        mj = dp.tile([np_, ny], dt.int32, tag=f"m{j}")
                        nc.vector.tensor_scalar(mj[:], vj[:], 15, None,
                                                A_.bitwise_and)
                        lmb[j] = mj
                l24 = dp.tile([np_, ny], dt.int32, tag="l24")
                hh = dp.tile([np_, ny], dt.int32, tag="hh")
                def gather_bits(dst, pieces):
                    first = True
                    for (src, shr, andm, shl) in pieces:
                        if andm is not None:
                            nc.vector.tensor_scalar(tmpa[:, :ny], src[:], andm,
                                                    shl, A_.bitwise_and,
                                                    A_.logical_shift_left)
                        elif shr > 0:
                            nc.vector.tensor_scalar(tmpa[:, :ny], src[:], shr,
                                                    None,
                                                    A_.logical_shift_right)
                        else:
                            nc.vector.tensor_scalar(tmpa[:, :ny], src[:], shl,
                                                    None,
                                                    A_.logical_shift_left)
                        if first:
                            nc.vector.tensor_copy(dst[:], tmpa[:, :ny])
                            first = False
                        else:
                            nc.vector.tensor_tensor(dst[:], dst[:],
                                                    tmpa[:, :ny], A_.bitwise_or)
                gather_bits(l24, [(lmb[3], 3, None, 0), (lmb[4], 0, None, 1),
                                  (lmb[5], 0, None, 5), (lmb[6], 0, None, 9),
                                  (lmb[7], 0, None, 13), (lmb[8], 0, None, 17),
                                  (lmb[9], 0, 7, 21)])
                gather_bits(hh, [(lmb[9], 3, None, 0), (lmb[10], 0, None, 1),
                                 (lmb[11], 0, None, 5), (lmb[12], 0, None, 9),
                                 (lmb[13], 0, None, 13),
                                 (lmb[14], 0, None, 17),
                                 (lmb[15], 0, 7, 21)])
                s63 = dp.tile([np_, ny], dt.int32, tag="s63")
                nc.vector.tensor_scalar(s63[:], lmb[15][:], 3, 1,
                                        A_.logical_shift_right, A_.bitwise_and)
                s63f = dp.tile([np_, ny], dt.float32, tag="s63f")
                nc.vector.tensor_copy(s63f[:], s63[:])
                hf = dp.tile([np_, ny], dt.float32, tag="hf")
                nc.vector.tensor_copy(hf[:], hh[:])
                nc.vector.scalar_tensor_tensor(
                    hf[:], s63f[:], -16777216.0, hf[:], A_.mult, A_.add)
                lf24 = dp.tile([np_, ny], dt.float32, tag="lf24")
                nc.vector.tensor_copy(lf24[:], l24[:])
                yv = dp.tile([np_, ny], dt.float32, tag="yv")
                nc.vector.scalar_tensor_tensor(
                    yv[:], hf[:], 16777216.0, lf24[:], A_.mult, A_.add)
                nc.sync.dma_start(
                    bass.AP(ydst, ybase + ch * CW, [[yrstride, np_], [1, ny]]),
                    yv[:])

        with tc.tile_pool(name="fir", bufs=1) as dp:
            emit_comb_fir(
                dp, bass.AP(decb, 0, [[DEC_PP, P], [1, DEC_PP + CHALO]]),
                P, DEC_PP + CHALO, ybuf, 0, DEC_PP)
            f15 = dp.tile([1, L], dt.float32, tag="f15t")
            nc.sync.dma_start(f15[:], f15_in[:])
            y15 = dp.tile([1, L], dt.float32, tag="y15")
            nc.sync.dma_start(y15[:], ybuf[0, :L])
            nc.vector.tensor_copy(y15[:1, :1], y15[:1, :1])
            nc.vector.tensor_tensor(y15[:], y15[:], f15[:], A.mult)
            nc.sync.dma_start(ybuf[0, :L], y15[:1, :])
            tl = dp.tile([1, CHALO + NBH], dt.float32, tag="tl")
            nc.sync.dma_start(tl[:, :CHALO], decb[0, DEC_PC:DEC_PC + CHALO])
            nc.gpsimd.dma_start(tl[:, CHALO:], hnbp[:][pid + 1, :])
            nc.sync.dma_start(taild[:], tl[:])
            emit_comb_fir(dp, taild[:], 1, CHALO + NBH, ybuf, DEC_PC, 1)

        # ============ STFT + mel + log ============
        with (tc.tile_pool(name="stft", bufs=1) as fp,
              tc.tile_pool(name="psum", bufs=1, space="PSUM") as psp):
            wint = fp.tile([128, 4], dt.float32)
            nc.sync.dma_start(wint[:], win_in[:])
            nc.vector.tensor_copy(wint[:1, :1], wint[:1, :1])
            cosm = fp.tile([128, 4 * 257], dt.float32)
            nc.sync.dma_start(
                cosm[:].rearrange("p (k f) -> p k f", f=257),
                bass.AP(cos_in, 0, [[257, 128], [128 * 257, 4], [1, 257]]))
            nc.vector.tensor_copy(cosm[:1, :1], cosm[:1, :1])
            sinm = fp.tile([128, 4 * 257], dt.float32)
            nc.sync.dma_start(
                sinm[:].rearrange("p (k f) -> p k f", f=257),
                bass.AP(sin_in, 0, [[257, 128], [128 * 257, 4], [1, 257]]))
            nc.vector.tensor_copy(sinm[:1, :1], sinm[:1, :1])
            fbm = fp.tile([128, 2 * NMEL], dt.float32)
            nc.sync.dma_start(
                fbm[:].rearrange("p (k f) -> p k f", f=NMEL),
                bass.AP(fb_in, 0, [[NMEL, 128], [128 * NMEL, 2], [1, NMEL]]))
            nc.vector.tensor_copy(fbm[:1, :1], fbm[:1, :1])
            fbm2 = fp.tile([1, NMEL], dt.float32)
            nc.sync.dma_start(fbm2[:], bass.AP(fb_in, 256 * NMEL, [[NMEL, 1], [1, NMEL]]))
            nc.vector.tensor_copy(fbm2[:1, :1], fbm2[:1, :1])
            xts = []
            for k in range(4):
                xk = fp.tile([128, FR_PC], dt.float32, tag=f"xk{k}")
                eng = [nc.sync, nc.scalar, nc.sync, nc.scalar][k]
                eng.dma_start(
                    xk[:], bass.AP(ybuf, 128 * k, [[1, 128], [HOP, FR_PC]]))
                nc.vector.tensor_copy(xk[:1, :1], xk[:1, :1])
                nc.vector.tensor_scalar(xk[:], xk[:], wint[:, k:k + 1], None,
                                        A.mult)
                xts.append(xk)
            pw0 = fp.tile([128, FR_PC], dt.float32, tag="pw0")
            pw1 = fp.tile([128, FR_PC], dt.float32, tag="pw1")
            pw2 = fp.tile([1, FR_PC], dt.float32, tag="pw2")
            pwr = [pw0, pw1, pw2]
            fcs = [(0, 128), (128, 256), (256, 257)]
            for fi, (f0, f1) in enumerate(fcs):
                for h in range(2):
                    hs = slice(h * FH, (h + 1) * FH)
                    pc = psp.tile([f1 - f0, FH], dt.float32, tag="pc")
                    ps = psp.tile([f1 - f0, FH], dt.float32, tag="ps")
                    for k in range(4):
                        nc.tensor.matmul(
                            pc[:], cosm[:, 257 * k + f0:257 * k + f1],
                            xts[k][:, hs], start=(k == 0), stop=(k == 3))
                    for k in range(4):
                        nc.tensor.matmul(
                            ps[:], sinm[:, 257 * k + f0:257 * k + f1],
                            xts[k][:, hs], start=(k == 0), stop=(k == 3))
                    t1_ = fp.tile([128, FH], dt.float32, tag="sq1")
                    nc.scalar.activation(t1_[:f1 - f0], pc[:], ACTF.Square)
                    t2_ = fp.tile([128, FH], dt.float32, tag="sq2")
                    nc.scalar.activation(t2_[:f1 - f0], ps[:], ACTF.Square)
                    nc.vector.tensor_tensor(pwr[fi][:, hs], t1_[:f1 - f0],
                                            t2_[:f1 - f0], A.add)
            lm = fp.tile([NMEL, FR_PC], dt.float32, tag="lm")
            for h in range(2):
                hs = slice(h * FH, (h + 1) * FH)
                mm = psp.tile([NMEL, FH], dt.float32, tag="mm")
                nc.tensor.matmul(mm[:], fbm[:, :NMEL], pwr[0][:, hs],
                                 start=True, stop=False)
                nc.tensor.matmul(mm[:], fbm[:, NMEL:], pwr[1][:, hs],
                                 start=False, stop=False)
                nc.tensor.matmul(mm[:], fbm2[:, :], pwr[2][:, hs],
                                 start=False, stop=True)
                xs = fp.tile([NMEL, FH], dt.float32, tag="xs")
                nc.vector.tensor_scalar(xs[:], mm[:], 1e-6, None, A.add)
                bx = fp.tile([NMEL, FH], dt.int32, tag="bx")
                nc.vector.tensor_copy(bx[:], xs[:].bitcast(dt.int32))
                ev = fp.tile([NMEL, FH], dt.int32, tag="ev")
                nc.vector.tensor_scalar(ev[:], bx[:], 23, None,
                                        A.logical_shift_right)
                nc.vector.tensor_scalar(ev[:], ev[:], -127, None, A.add)
                evf = fp.tile([NMEL, FH], dt.float32, tag="evf")
                nc.vector.tensor_copy(evf[:], ev[:])
                nc.vector.tensor_scalar(bx[:], bx[:], 0x7FFFFF, 127 << 23,
                                        A.bitwise_and, A.bitwise_or)
                lnm = fp.tile([NMEL, FH], dt.float32, tag="lnm")
                nc.scalar.activation(lnm[:], bx[:].bitcast(dt.float32), ACTF.Ln)
                nc.vector.scalar_tensor_tensor(
                    lm[:, hs], evf[:], 0.6931471805599453, lnm[:],
                    A.mult, A.add)
            nc.sync.dma_start(out_p[:], lm[:])

    nc.compile()
    return nc


def _constants():
    mask = np.ones((128, 5136), np.float32)
    mask[:, 0::16] = 0.0
    n = np.arange(N_FFT, dtype=np.float64)
    f = np.arange(257, dtype=np.float64)
    ang = 2.0 * np.pi * n[:, None] * f[None, :] / N_FFT
    cosm = np.cos(ang).astype(np.float32)
    sinm = (-np.sin(ang)).astype(np.float32)
    fbm = _mel_fbanks_np()
    t = np.arange(WIN_LEN, dtype=np.float32)
    win = (0.5 * (1.0 - np.cos(2.0 * np.pi * t / WIN_LEN))).astype(np.float32)
    pad_l = (N_FFT - WIN_LEN) // 2
    win_p = np.zeros(N_FFT, np.float32)
    win_p[pad_l:pad_l + WIN_LEN] = win
    winm = win_p.reshape(4, 128).T.copy()
    return mask, cosm, sinm, fbm, winm


def kernel(pdm_bits, taps, scale):
    from concourse.bass_utils import run_bass_kernel_spmd

    pdm = np.asarray(pdm_bits, dtype=np.int32)
    taps_l = [int(x) for x in np.asarray(taps).tolist()]
    key = (tuple(taps_l), int(scale))
    if key not in _COMPILED:
        _COMPILED[key] = _build(taps_l, int(scale))
    nc = _COMPILED[key]

    mask, cosm, sinm, fbm, winm = _constants()
    shards = pdm.reshape(NCORE, P, FREE)
    in_maps = []
    for c in range(NCORE):
        f15 = np.ones((1, L), np.float32)
        if c == 0:
            f15[:] = 0.0
        phi = (4 * c) % 16
        r1 = (472500 * c - phi) // 16
        in_maps.append({
            "pdm": shards[c],
            "mask": mask, "cosm": cosm, "sinm": sinm, "fbm": fbm,
            "winm": winm, "f15": f15,
            "phiv": np.array([[phi]], np.int32),
            "r1v": np.array([[r1]], np.int32),
            "p16v": np.array([[16 - phi]], np.int32),
            "r1m1": np.array([[max(r1 - 1, 0)]], np.int32),
            "v0m": np.array([[1.0 if (phi == 0 and c != 0) else 0.0]],
                            np.float32),
        })
    res = run_bass_kernel_spmd(nc, in_maps, list(range(NCORE)))
    global _LAST_RES
    _LAST_RES = res
    outs = [res.results[c]["out"] for c in range(NCORE)]
    full = np.concatenate(outs, axis=1)[:, :T_FRAMES]
    return full[None, None].astype(np.float32)


# revision 30
# speedup vs baseline: 746.0076x; 1.0185x over previous
"""AudioFrontend Trainium2 kernel: PDM -> CIC(f32 blk16-exact) -> FIR(int64) -> logmel.

Bit-exact replication of jax-CPU float32 cumsum (XLA ReduceWindowRewriter
base-16 blocked scans) through the chaotic CIC stages, exact int64 FIR via
12-bit limbs on gpsimd int32, then matmul STFT/mel/log.
Self-contained: hardcodes all shapes; host code only shards/gathers.
"""
import numpy as np

NCORE = 8
N_PDM = 60_480_000
PERCORE = N_PDM // NCORE          # 7,560,000
P = 125
FREE = PERCORE // P               # 60480
TILE_F = 4032                     # 63*64 = 16*252
NT = FREE // TILE_F               # 15
ROWS_T = TILE_F // 16             # 252
ROWS_P = FREE // 16               # 3780
T0_LOC = P * ROWS_P               # 472500
T0_GLOB = NCORE * T0_LOC          # 3780000
GF = T0_GLOB // P                 # 30240
GCH = 5040
NGC = GF // GCH                   # 6
T1R = GCH // 16                   # 315
T1N = T0_GLOB // 16               # 236250
T1PAD = 236256                    # 123*1920 + 96
T2N = T1PAD // 16                 # 14766
T2PAD = 14768
T3N = T2PAD // 16                 # 923
T3PAD = 928
T4N = T3PAD // 16                 # 58
T4PAD = 64
T5N = T4PAD // 16                 # 4
DECIM = 63
DEC_PC = PERCORE // DECIM         # 120000
DEC_PP = FREE // DECIM            # 960
DEC_T = TILE_F // DECIM           # 64
L = 15
CHALO = 19
NBH = 384
N_FFT = 512
HOP = 160
WIN_LEN = 400
NMEL = 40
FR_PC = 750
T_FRAMES = 1 + (N_PDM // DECIM - N_FFT) // HOP  # 5997
FH = 375
SAT = 9.223372036854775808e18


def _mel_fbanks_np():
    n_freqs = N_FFT // 2 + 1
    all_freqs = np.linspace(0.0, 16000 / 2, n_freqs)
    h2m = lambda f: 2595.0 * np.log10(1.0 + f / 700.0)
    m_pts = np.linspace(h2m(0.0), h2m(8000.0), NMEL + 2)
    f_pts = 700.0 * (10.0 ** (m_pts / 2595.0) - 1.0)
    f_diff = f_pts[1:] - f_pts[:-1]
    slopes = f_pts[None, :] - all_freqs[:, None]
    down = -slopes[:, :-2] / f_diff[:-1]
    up = slopes[:, 2:] / f_diff[1:]
    return np.maximum(0.0, np.minimum(down, up)).astype(np.float32)


_COMPILED = {}
_LAST_RES = None


def _build(taps_list, scale_int):
    import concourse.bass as bass
    import concourse.bacc as bacc
    import concourse.mybir as mybir
    import concourse.tile as tile

    dt = mybir.dt
    A = mybir.AluOpType
    ACTF = mybir.ActivationFunctionType

    nc = bacc.Bacc()
    pdm_in = nc.declare_dram_parameter("pdm", [P, FREE], dt.int32, isOutput=False)
    mask_in = nc.declare_dram_parameter("mask", [128, 5136], dt.float32, isOutput=False)
    cos_in = nc.declare_dram_parameter("cosm", [N_FFT, 257], dt.float32, isOutput=False)
    sin_in = nc.declare_dram_parameter("sinm", [N_FFT, 257], dt.float32, isOutput=False)
    fb_in = nc.declare_dram_parameter("fbm", [257, NMEL], dt.float32, isOutput=False)
    win_in = nc.declare_dram_parameter("winm", [128, 4], dt.float32, isOutput=False)
    f15_in = nc.declare_dram_parameter("f15", [1, L], dt.float32, isOutput=False)
    phi_in = nc.declare_dram_parameter("phiv", [1, 1], dt.int32, isOutput=False)
    r1_in = nc.declare_dram_parameter("r1v", [1, 1], dt.int32, isOutput=False)
    p16_in = nc.declare_dram_parameter("p16v", [1, 1], dt.int32, isOutput=False)
    r1m_in = nc.declare_dram_parameter("r1m1", [1, 1], dt.int32, isOutput=False)
    v0m_in = nc.declare_dram_parameter("v0m", [1, 1], dt.float32, isOutput=False)
    out_p = nc.declare_dram_parameter("out", [NMEL, FR_PC], dt.float32, isOutput=True)

    pA = nc.dram_tensor("pA", [P, FREE], dt.float32)
    pB = nc.dram_tensor("pB", [P, FREE], dt.float32)
    t0loc = nc.dram_tensor("t0loc", [1, T0_LOC], dt.float32)
    e0buf = nc.dram_tensor("e0buf", [1, 16 + T0_LOC + 16], dt.float32)
    h16i = nc.dram_tensor("h16i", [1, 32], dt.float32)
    h16o = nc.dram_tensor("h16o", [NCORE, 32], dt.float32)
    h16p = nc.dram_tensor("h16p", [NCORE + 2, 32], dt.float32)
    t1agi = nc.dram_tensor("t1agi", [1, 29532], dt.float32)
    t1ago = nc.dram_tensor("t1ago", [NCORE, 29532], dt.float32)
    vloc = nc.dram_tensor("vloc", [1, 1 + 472512], dt.float32)
    t1buf = nc.dram_tensor("t1buf", [1, T1PAD], dt.float32)
    t2buf = nc.dram_tensor("t2buf", [1, T2PAD], dt.float32)
    zbuf = nc.dram_tensor("zbuf", [1, 1 + T2PAD], dt.float32)
    zsbuf = nc.dram_tensor("zsbuf", [1, 1 + T1PAD], dt.float32)
    decb = nc.dram_tensor("decb", [1, CHALO + DEC_PC], dt.float32)
    ybuf = nc.dram_tensor("ybuf", [1, DEC_PC + NBH], dt.float32)
    h19i = nc.dram_tensor("h19i", [1, CHALO], dt.float32)
    h19o = nc.dram_tensor("h19o", [NCORE, CHALO], dt.float32)
    h19p = nc.dram_tensor("h19p", [NCORE + 1, CHALO], dt.float32)
    hnbi = nc.dram_tensor("hnbi", [1, NBH], dt.float32)
    hnbo = nc.dram_tensor("hnbo", [NCORE, NBH], dt.float32)
    hnbp = nc.dram_tensor("hnbp", [NCORE + 1, NBH], dt.float32)
    taild = nc.dram_tensor("taild", [1, CHALO + NBH], dt.float32)
    t3d = nc.dram_tensor("t3d", [1, T3N], dt.float32)
    z3buf = nc.dram_tensor("z3buf", [1, 1 + T3PAD], dt.float32)

    RG = [list(range(NCORE))]
    PHIS = [(4 * c) % 16 for c in range(NCORE)]
    R1S = [(T0_LOC * c - PHIS[c]) // 16 for c in range(NCORE)]
    MCS = [(R1S[c + 1] - R1S[c]) if c + 1 < NCORE else (T1N - R1S[c])
           for c in range(NCORE)]
    taps = [int(t) for t in taps_list]
    assert (1 << 15) == int(scale_int)

    with tile.TileContext(nc) as tc:
        pid = nc.gpsimd.partition_id()

        # ============ scan stages ============
        with tc.tile_pool(name="persist", bufs=1) as pp:
            mask = pp.tile([128, 5136], dt.float32)
            nc.sync.dma_start(mask[:], mask_in[:])
            nc.vector.tensor_copy(mask[:1, :1], mask[:1, :1])
            t0sb = pp.tile([P, ROWS_P], dt.float32)
            carry0 = pp.tile([P, ROWS_P], dt.float32)
            decsb = pp.tile([P, DEC_PP], dt.float32)

            for st in range(5):
                src = [None, pA, pB, pA, pB][st]
                dst = [pA, pB, pA, pB, pA][st]
                with tc.tile_pool(name=f"s{st}", bufs=3) as sp:
                    for t in range(NT):
                        fs = slice(t * TILE_F, (t + 1) * TILE_F)
                        if st == 0:
                            raw = sp.tile([P, TILE_F], dt.int32, tag="raw")
                            nc.sync.dma_start(raw[:], pdm_in[:, fs])
                            xt = sp.tile([P, TILE_F], dt.float32, tag="xt")
                            nc.scalar.activation(xt[:], raw[:], ACTF.Copy,
                                                 bias=-1.0, scale=2.0)
                        else:
                            pin = sp.tile([P, TILE_F], dt.float32, tag="pin")
                            nc.sync.dma_start(pin[:], src[:, fs])
                            nc.vector.tensor_copy(pin[:1, :1], pin[:1, :1])
                            xt = sp.tile([P, TILE_F], dt.float32, tag="xt")
                            rs = slice(t * ROWS_T, (t + 1) * ROWS_T)
                            nc.vector.tensor_tensor(
                                xt[:].rearrange("p (r s) -> p r s", s=16),
                                pin[:].rearrange("p (r s) -> p r s", s=16),
                                carry0[:, rs].broadcast_to([P, ROWS_T, 16]),
                                A.add)
                        po = sp.tile([P, TILE_F], dt.float32, tag="po")
                        nc.vector.tensor_tensor_scan(
                            po[:], mask[:P, :TILE_F], xt[:], 0.0, A.mult, A.add)
                        nc.vector.tensor_copy(
                            t0sb[:, t * ROWS_T:(t + 1) * ROWS_T], po[:, 15::16])
                        nc.scalar.dma_start(dst[:, fs], po[:])

                nc.sync.dma_start(h16i[0, :16], t0sb[:1, :16])
                nc.sync.dma_start(h16i[0, 16:], t0sb[P - 1:P, ROWS_P - 16:])
                nc.gpsimd.collective_compute(
                    "AllGather", A.bypass, replica_groups=RG,
                    ins=[h16i[:]], outs=[h16o[:]])
                # padded halo rows: row0=AG7, rows1..8=AG0..7, row9=AG0
                nc.sync.dma_start(h16p[0, :], h16o[NCORE - 1, :])
                nc.sync.dma_start(h16p[1:NCORE + 1, :], h16o[:])
                nc.sync.dma_start(h16p[NCORE + 1, :], h16o[0, :])
                # e0: [left16 halo][own t0][right16 halo]
                nc.gpsimd.dma_start(e0buf[0, :16], h16p[:][pid, 16:])
                nc.sync.dma_start(
                    bass.AP(e0buf, 16, [[ROWS_P, P], [1, ROWS_P]]), t0sb[:])
                nc.gpsimd.dma_start(
                    e0buf[0, 16 + T0_LOC:], h16p[:][pid + 2, :16])

                with tc.tile_pool(name=f"g{st}", bufs=1) as gp:
                    phir = nc.gpsimd.alloc_register(f"phir{st}")
                    nc.gpsimd.reg_load(phir, phi_in[:1, :1])
                    r1r = nc.gpsimd.alloc_register(f"r1r{st}")
                    nc.gpsimd.reg_load(r1r, r1_in[:1, :1])
                    p16r = nc.gpsimd.alloc_register(f"p16r{st}")
                    nc.gpsimd.reg_load(p16r, p16_in[:1, :1])
                    r1mr = nc.gpsimd.alloc_register(f"r1mr{st}")
                    nc.gpsimd.reg_load(r1mr, r1m_in[:1, :1])
                    # local extended-t0 scan: [92, 5136] from e0buf
                    ge = gp.tile([92, 5136], dt.float32, tag="ge")
                    nc.gpsimd.dma_start(
                        ge[:],
                        bass.AP(e0buf, bass.make_scalar_value(p16r),
                                [[5136, 92], [1, 5136]]))
                    nc.vector.tensor_copy(ge[:1, :1], ge[:1, :1])
                    gs = gp.tile([92, 5136], dt.float32, tag="gs")
                    nc.vector.tensor_tensor_scan(
                        gs[:], mask[:92, :5136], ge[:], 0.0, A.mult, A.add)
                    tx = gp.tile([92, 321], dt.float32, tag="tx")
                    nc.vector.tensor_copy(tx[:], gs[:, 15::16])
                    nc.sync.dma_start(
                        bass.AP(t1agi, 0, [[321, 92], [1, 321]]), tx[:])
                    nc.gpsimd.collective_compute(
                        "AllGather", A.bypass, replica_groups=RG,
                        ins=[t1agi[:]], outs=[t1ago[:]])
                    # compact ragged t1 via SBUF bounce; full-width copies in
                    # forward order so each overwrites the previous overhang
                    for c in range(NCORE):
                        cb = gp.tile([12, 2461], dt.float32, tag="cb")
                        nc.sync.dma_start(
                            cb[:], bass.AP(t1ago, c * 29532,
                                           [[2461, 12], [1, 2461]]))
                        nc.sync.dma_start(
                            bass.AP(t1buf, R1S[c], [[2461, 12], [1, 2461]]),
                            cb[:])
                    zt = gp.tile([1, 16], dt.float32, tag="zt")
                    nc.vector.memset(zt[:], 0.0)
                    nc.sync.dma_start(t1buf[0, T1N:T1PAD], zt[:1, :T1PAD - T1N])
                    nc.sync.dma_start(zbuf[0, :1], zt[:1, :1])
                    nc.sync.dma_start(zsbuf[0, :1], zt[:1, :1])

                    u1a = gp.tile([123, 1920], dt.float32, tag="u1a")
                    nc.sync.dma_start(
                        u1a[:], bass.AP(t1buf, 0, [[1920, 123], [1, 1920]]))
                    nc.vector.tensor_copy(u1a[:1, :1], u1a[:1, :1])
                    p2a = gp.tile([123, 1920], dt.float32, tag="p2a")
                    nc.vector.tensor_tensor_scan(
                        p2a[:], mask[:123, :1920], u1a[:], 0.0, A.mult, A.add)
                    u1b = gp.tile([1, 96], dt.float32, tag="u1b")
                    nc.sync.dma_start(u1b[:], t1buf[0, 123 * 1920:T1PAD])
                    nc.vector.tensor_copy(u1b[:1, :1], u1b[:1, :1])
                    p2b = gp.tile([1, 96], dt.float32, tag="p2b")
                    nc.vector.tensor_tensor_scan(
                        p2b[:], mask[:1, :96], u1b[:], 0.0, A.mult, A.add)
                    t2a = gp.tile([123, 120], dt.float32, tag="t2a")
                    nc.vector.tensor_copy(t2a[:], p2a[:, 15::16])
                    nc.sync.dma_start(
                        bass.AP(t2buf, 0, [[120, 123], [1, 120]]), t2a[:])
                    t2b = gp.tile([1, 6], dt.float32, tag="t2b")
                    nc.vector.tensor_copy(t2b[:], p2b[:, 15::16])
                    nc.sync.dma_start(t2buf[0, 123 * 120:T2N], t2b[:1, :])
                    nc.sync.dma_start(t2buf[0, T2N:T2PAD], zt[:1, :T2PAD - T2N])

                    u2 = gp.tile([13, 1136], dt.float32, tag="u2")
                    nc.sync.dma_start(
                        u2[:], bass.AP(t2buf, 0, [[1136, 13], [1, 1136]]))
                    nc.vector.tensor_copy(u2[:1, :1], u2[:1, :1])
                    p3 = gp.tile([13, 1136], dt.float32, tag="p3")
                    nc.vector.tensor_tensor_scan(
                        p3[:], mask[:13, :1136], u2[:], 0.0, A.mult, A.add)
                    t3x = gp.tile([13, 71], dt.float32, tag="t3x")
                    nc.vector.tensor_copy(t3x[:], p3[:, 15::16])
                    nc.sync.dma_start(
                        bass.AP(t3d, 0, [[71, 13], [1, 71]]), t3x[:])
                    u3 = gp.tile([1, T3PAD], dt.float32, tag="u3")
                    nc.vector.memset(u3[:], 0.0)
                    nc.sync.dma_start(u3[:1, :T3N], t3d[0, :])
                    nc.vector.tensor_copy(u3[:1, :1], u3[:1, :1])
                    p4 = gp.tile([1, T3PAD], dt.float32, tag="p4")
                    nc.vector.tensor_tensor_scan(
                        p4[:], mask[:1, :T3PAD], u3[:], 0.0, A.mult, A.add)
                    u4 = gp.tile([1, T4PAD], dt.float32, tag="u4")
                    nc.vector.memset(u4[:], 0.0)
                    nc.vector.tensor_copy(u4[:, :T4N], p4[:, 15::16])
                    p5 = gp.tile([1, T4PAD], dt.float32, tag="p5")
                    nc.vector.tensor_tensor_scan(
                        p5[:], mask[:1, :T4PAD], u4[:], 0.0, A.mult, A.add)
                    u5 = gp.tile([1, T5N], dt.float32, tag="u5")
                    nc.vector.tensor_copy(u5[:], p5[:, 15::16])
                    s5 = gp.tile([1, T5N], dt.float32, tag="s5")
                    nc.vector.tensor_tensor_scan(
                        s5[:], mask[:1, :T5N], u5[:], 0.0, A.mult, A.add)
                    nc.vector.tensor_tensor(
                        p5[:, 16:].rearrange("p (r s) -> p r s", s=16),
                        p5[:, 16:].rearrange("p (r s) -> p r s", s=16),
                        s5[:, :3].broadcast_to([1, 3, 16]), A.add)
                    nc.vector.tensor_tensor(
                        p4[:, 16:].rearrange("p (r s) -> p r s", s=16),
                        p4[:, 16:].rearrange("p (r s) -> p r s", s=16),
                        p5[:, :T4N - 1].broadcast_to([1, T4N - 1, 16]), A.add)
                    nc.sync.dma_start(z3buf[0, :1], zt[:1, :1])
                    nc.sync.dma_start(z3buf[0, 1:1 + T3PAD], p4[:1, :])
                    cz3 = gp.tile([13, 71], dt.float32, tag="t3x")
                    nc.sync.dma_start(
                        cz3[:], bass.AP(z3buf, 0, [[71, 13], [1, 71]]))
                    nc.vector.tensor_copy(cz3[:1, :1], cz3[:1, :1])
                    nc.vector.tensor_tensor(
                        p3[:].rearrange("p (r s) -> p r s", s=16),
                        p3[:].rearrange("p (r s) -> p r s", s=16),
                        cz3[:].broadcast_to([13, 71, 16]), A.add)
                    nc.sync.dma_start(
                        bass.AP(zbuf, 1, [[1136, 13], [1, 1136]]), p3[:])
                    cza = gp.tile([123, 120], dt.float32, tag="cza")
                    nc.sync.dma_start(
                        cza[:], bass.AP(zbuf, 0, [[120, 123], [1, 120]]))
                    nc.vector.tensor_copy(cza[:1, :1], cza[:1, :1])
                    nc.vector.tensor_tensor(
                        p2a[:].rearrange("p (r s) -> p r s", s=16),
                        p2a[:].rearrange("p (r s) -> p r s", s=16),
                        cza[:].broadcast_to([123, 120, 16]), A.add)
                    czb = gp.tile([1, 6], dt.float32, tag="czb")
                    nc.sync.dma_start(czb[:], zbuf[0, 123 * 120:123 * 120 + 6])
                    nc.vector.tensor_copy(czb[:1, :1], czb[:1, :1])
                    nc.vector.tensor_tensor(
                        p2b[:].rearrange("p (r s) -> p r s", s=16),
                        p2b[:].rearrange("p (r s) -> p r s", s=16),
                        czb[:].broadcast_to([1, 6, 16]), A.add)
                    nc.sync.dma_start(
                        bass.AP(zsbuf, 1, [[1920, 123], [1, 1920]]), p2a[:])
                    nc.sync.dma_start(
                        zsbuf[0, 1 + 123 * 1920:1 + T1PAD], p2b[:1, :])

                    # own scan_t0: p1_local + bcast(Zs at own rows)
                    ctb = gp.tile([92, 321], dt.float32, tag="tx")
                    nc.gpsimd.dma_start(
                        ctb[:],
                        bass.AP(zsbuf, bass.make_scalar_value(r1r),
                                [[321, 92], [1, 321]]))
                    nc.vector.tensor_copy(ctb[:1, :1], ctb[:1, :1])
                    nc.vector.tensor_tensor(
                        gs[:].rearrange("p (r s) -> p r s", s=16),
                        gs[:].rearrange("p (r s) -> p r s", s=16),
                        ctb[:].broadcast_to([92, 321, 16]), A.add)
                    nc.sync.dma_start(
                        bass.AP(vloc, 1, [[5136, 92], [1, 5136]]), gs[:])
                    # vloc[0]: 0 normally; for the phi==0 mid core (c=4):
                    # scan_t0[A_c - 1] = t1[A_c/16 - 1] + scan_t1[A_c/16 - 2]
                    sv = gp.tile([1, 2], dt.float32, tag="sv")
                    nc.gpsimd.dma_start(
                        sv[:1, :1],
                        bass.AP(t1buf, bass.make_scalar_value(r1mr),
                                [[1, 1], [1, 1]]))
                    nc.gpsimd.dma_start(
                        sv[:1, 1:],
                        bass.AP(zsbuf, bass.make_scalar_value(r1mr),
                                [[1, 1], [1, 1]]))
                    v0t = gp.tile([1, 1], dt.float32, tag="v0t")
                    nc.sync.dma_start(v0t[:], v0m_in[:])
                    nc.vector.tensor_copy(v0t[:1, :1], v0t[:1, :1])
                    sv2 = gp.tile([1, 1], dt.float32, tag="sv2")
                    nc.vector.tensor_tensor(sv2[:], sv[:1, :1], sv[:1, 1:],
                                            A.add)
                    nc.vector.tensor_tensor(sv2[:], sv2[:], v0t[:], A.mult)
                    nc.sync.dma_start(vloc[0, :1], sv2[:1, :])
                    nc.gpsimd.dma_start(
                        carry0[:],
                        bass.AP(vloc, bass.make_scalar_value(phir),
                                [[ROWS_P, P], [1, ROWS_P]]))
                    nc.vector.tensor_copy(carry0[:1, :1], carry0[:1, :1])

            # ============ phase B of stage 5: decimate ============
            with tc.tile_pool(name="pb5", bufs=3) as sp:
                for t in range(NT):
                    fs = slice(t * TILE_F, (t + 1) * TILE_F)
                    pin = sp.tile([P, TILE_F], dt.float32, tag="pin")
                    nc.sync.dma_start(pin[:], pA[:, fs])
                    nc.vector.tensor_copy(pin[:1, :1], pin[:1, :1])
                    xt = sp.tile([P, TILE_F], dt.float32, tag="xt")
                    rs = slice(t * ROWS_T, (t + 1) * ROWS_T)
                    nc.vector.tensor_tensor(
                        xt[:].rearrange("p (r s) -> p r s", s=16),
                        pin[:].rearrange("p (r s) -> p r s", s=16),
                        carry0[:, rs].broadcast_to([P, ROWS_T, 16]), A.add)
                    nc.vector.tensor_copy(
                        decsb[:, t * DEC_T:(t + 1) * DEC_T], xt[:, 0::DECIM])

            nc.sync.dma_start(
                bass.AP(decb, CHALO, [[DEC_PP, P], [1, DEC_PP]]), decsb[:])
            nc.sync.dma_start(h19i[:1, :], decsb[P - 1:P, DEC_PP - CHALO:])
            nc.gpsimd.collective_compute(
                "AllGather", A.bypass, replica_groups=RG,
                ins=[h19i[:]], outs=[h19o[:]])
            nc.sync.dma_start(hnbi[:1, :], decsb[:1, :NBH])
            nc.gpsimd.collective_compute(
                "AllGather", A.bypass, replica_groups=RG,
                ins=[hnbi[:]], outs=[hnbo[:]])
            with tc.tile_pool(name="hx", bufs=1) as hp:
                zt2 = hp.tile([1, NBH], dt.float32)
                nc.vector.memset(zt2[:], 0.0)
                nc.sync.dma_start(h19p[0, :], zt2[:1, :CHALO])
                nc.sync.dma_start(h19p[1:, :], h19o[:])
                nc.sync.dma_start(hnbp[:NCORE, :], hnbo[:])
                nc.sync.dma_start(hnbp[NCORE, :], hnbo[0, :])
                nc.gpsimd.dma_start(decb[0, :CHALO], h19p[:][pid, :])

        # ============ comb + FIR + y ============
        def emit_comb_fir(dp, src_ap, np_, width, ydst, ybase, yrstride):
            A_ = A
            NL = 6  # 12-bit limbs; MAC on gpsimd int32 (exact mod 2^32)
            cmb = dp.tile([np_, width], dt.float32, tag="cmb0")
            nc.sync.dma_start(cmb[:], src_ap)
            nc.vector.tensor_copy(cmb[:1, :1], cmb[:1, :1])
            cur = cmb
            w = width
            for it in range(5):
                nxt = dp.tile([np_, w - 1], dt.float32, tag=f"cmb{1 + it % 2}")
                nc.vector.tensor_tensor(
                    nxt[:], cur[:, 1:w], cur[:, :w - 1], A_.subtract)
                cur = nxt
                w -= 1
            nw = w
            ny = nw - (L - 1)
            satp = dp.tile([np_, nw], dt.float32, tag="satp")
            nc.vector.tensor_scalar(satp[:], cur[:], SAT, None, A_.is_ge)
            satn = dp.tile([np_, nw], dt.float32, tag="satn")
            nc.vector.tensor_scalar(satn[:], cur[:], -SAT, None, A_.is_le)
            sgn = dp.tile([np_, nw], dt.float32, tag="sgn")
            nc.vector.tensor_scalar(sgn[:], cur[:], 0.0, None, A_.is_ge)
            nc.vector.tensor_scalar(sgn[:], sgn[:], 2.0, -1.0, A_.mult, A_.add)
            mag = dp.tile([np_, nw], dt.float32, tag="mag")
            nc.vector.tensor_tensor(mag[:], cur[:], sgn[:], A_.mult)
            rs_ = dp.tile([np_, nw], dt.float32, tag="rs")
            nc.vector.tensor_tensor(rs_[:], satp[:], satn[:], A_.add)
            nc.vector.tensor_scalar(rs_[:], rs_[:], -1.0, 1.0, A_.mult, A_.add)
            nc.vector.tensor_tensor(rs_[:], rs_[:], sgn[:], A_.mult)
            rsi = dp.tile([np_, nw], dt.int32, tag="rsi")
            nc.vector.tensor_copy(rsi[:], rs_[:])
            bits = dp.tile([np_, nw], dt.int32, tag="bits")
            nc.vector.tensor_copy(bits[:], mag[:].bitcast(dt.int32))
            ex = dp.tile([np_, nw], dt.int32, tag="ex")
            nc.vector.tensor_scalar(ex[:], bits[:], 23, None,
                                    A_.logical_shift_right)
            nc.vector.tensor_scalar(ex[:], ex[:], 255, None, A_.bitwise_and)
            nc.vector.tensor_scalar(ex[:], ex[:], -150, None, A_.add)
            mant = dp.tile([np_, nw], dt.int32, tag="mant")
            nc.vector.tensor_scalar(mant[:], bits[:], 0x7FFFFF, 0x800000,
                                    A_.bitwise_and, A_.bitwise_or)
            nzm = dp.tile([np_, nw], dt.int32, tag="nzm")
            nc.vector.tensor_scalar(nzm[:], ex[:], -23, None, A_.is_ge)
            nc.vector.tensor_tensor(mant[:], mant[:], nzm[:], A_.mult)
            tmpa = dp.tile([np_, nw], dt.int32, tag="tmpa")
            tmpb = dp.tile([np_, nw], dt.int32, tag="tmpb")
            tmpr = dp.tile([np_, nw], dt.int32, tag="tmpr")
            tmps = dp.tile([np_, nw], dt.int32, tag="tmps")
            sel = dp.tile([np_, nw], dt.int32, tag="sel")
            dgp = dp.tile([1, 1], dt.int32, tag="dgp")
            nc.gpsimd.tensor_copy(dgp[:], rsi[:1, :1])  # touch rsi on gpsimd
            limbs = []
            for j in range(NL):
                # r = 12j - ex; limb = r>=0 ? (mant>>min(r,31))&4095
                #                          : (mant<<min(-r,11))&4095
                nc.vector.tensor_scalar(tmpr[:], ex[:], -1, 12 * j, A_.mult,
                                        A_.add)
                nc.vector.tensor_scalar(tmps[:], tmpr[:], 31, None, A_.min)
                nc.vector.tensor_scalar(tmps[:], tmps[:], 0, None, A_.max)
                nc.vector.tensor_tensor(tmpa[:], mant[:], tmps[:],
                                        A_.logical_shift_right)
                nc.vector.tensor_scalar(tmpa[:], tmpa[:], 4095, None,
                                        A_.bitwise_and)
                nc.vector.tensor_scalar(tmps[:], tmpr[:], -1, 0, A_.mult,
                                        A_.max)
                nc.vector.tensor_scalar(tmps[:], tmps[:], 11, None, A_.min)
                nc.vector.tensor_tensor(tmpb[:], mant[:], tmps[:],
                                        A_.logical_shift_left)
                nc.vector.tensor_scalar(tmpb[:], tmpb[:], 4095, None,
                                        A_.bitwise_and)
                nc.vector.tensor_scalar(sel[:], tmpr[:], 0, None, A_.is_ge)
                lj = dp.tile([np_, nw], dt.int32, tag=f"l{j}")
                nc.vector.select(lj[:], sel[:], tmpa[:], tmpb[:])
                # signed/masked limb on gpsimd (int, exact)
                nc.gpsimd.tensor_mul(lj[:], lj[:], rsi[:])
                limbs.append(lj)
            accA = dp.tile([np_, ny], dt.float32, tag="accA")
            accB = dp.tile([np_, ny], dt.float32, tag="accB")
            for k in range(L):
                o = L - 1 - k
                if k == 0:
                    nc.vector.tensor_scalar(accA[:], satp[:, o:o + ny],
                                            float(taps[k]), None, A_.mult)
                    nc.vector.tensor_scalar(accB[:], satn[:, o:o + ny],
                                            float(taps[k]), None, A_.mult)
                else:
                    nc.vector.scalar_tensor_tensor(
                        accA[:], satp[:, o:o + ny], float(taps[k]), accA[:],
                        A_.mult, A_.add)
                    nc.vector.scalar_tensor_tensor(
                        accB[:], satn[:, o:o + ny], float(taps[k]), accB[:],
                        A_.mult, A_.add)
            # MAC on gpsimd int32: k-outer with a memset tap tile
            cols = []
            for j in range(NL):
                cj = dp.tile([np_, ny], dt.int32, tag=f"c{j}")
                cols.append(cj)
            tapt = dp.tile([np_, ny], dt.int32, tag="tapt")
            tmpg = dp.tile([np_, ny], dt.int32, tag="tmpg")
            for k in range(L):
                o = L - 1 - k
                nc.gpsimd.memset(tapt[:], taps[k])
                for j in range(NL):
                    if k == 0:
                        nc.gpsimd.tensor_mul(cols[j][:],
                                             limbs[j][:, o:o + ny], tapt[:])
                    else:
                        nc.gpsimd.tensor_mul(tmpg[:],
                                             limbs[j][:, o:o + ny], tapt[:])
                        nc.gpsimd.tensor_add(cols[j][:], cols[j][:], tmpg[:])
            ai = dp.tile([np_, ny], dt.int32, tag="ai")
            nc.vector.tensor_copy(ai[:], accA[:])
            bi = dp.tile([np_, ny], dt.int32, tag="bi")
            nc.vector.tensor_copy(bi[:], accB[:])
            nc.gpsimd.tensor_copy(dgp[:], ai[:1, :1])  # touch ai on gpsimd
            par = dp.tile([np_, ny], dt.int32, tag="par")
            nc.gpsimd.tensor_sub(par[:], ai[:], bi[:])
            nc.vector.tensor_scalar(par[:], par[:], 1, 3, A_.bitwise_and,
                                    A_.logical_shift_left)
            nc.gpsimd.tensor_sub(cols[0][:], cols[0][:], ai[:])
            nc.gpsimd.tensor_add(cols[5][:], cols[5][:], par[:])
            # ripple: adds on gpsimd (values < 2^31 exact), shifts/masks on DVE
            carry = dp.tile([np_, ny], dt.int32, tag="cy")
            lmb = [None] * NL
            for j in range(NL):
                if j > 0:
                    nc.gpsimd.tensor_add(cols[j][:], cols[j][:], carry[:])
                if j < NL - 1:
                    nc.vector.tensor_scalar(carry[:], cols[j][:], 12, None,
                                            A_.arith_shift_right)
                if 1 <= j <= 4:
                    mj = dp.tile([np_, ny], dt.int32, tag=f"m{j}")
                    nc.vector.tensor_scalar(mj[:], cols[j][:], 4095, None,
                                            A_.bitwise_and)
                    lmb[j] = mj
            lmb[5] = cols[5]
            # y = acc >> 15: L24 = bits 15..38, H = bits 39..62 + sign bit 63
            l24 = dp.tile([np_, ny], dt.int32, tag="l24")
            nc.vector.tensor_scalar(l24[:], lmb[1][:], 3, None,
                                    A_.logical_shift_right)
            nc.vector.tensor_scalar(tmpa[:, :ny], lmb[2][:], 9, None,
                                    A_.logical_shift_left)
            nc.vector.tensor_tensor(l24[:], l24[:], tmpa[:, :ny], A_.bitwise_or)
            nc.vector.tensor_scalar(tmpa[:, :ny], lmb[3][:], 7, 21,
                                    A_.bitwise_and, A_.logical_shift_left)
            nc.vector.tensor_tensor(l24[:], l24[:], tmpa[:, :ny], A_.bitwise_or)
            hh = dp.tile([np_, ny], dt.int32, tag="hh")
            nc.vector.tensor_scalar(hh[:], lmb[3][:], 3, None,
                                    A_.logical_shift_right)
            nc.vector.tensor_scalar(tmpa[:, :ny], lmb[4][:], 9, None,
                                    A_.logical_shift_left)
            nc.vector.tensor_tensor(hh[:], hh[:], tmpa[:, :ny], A_.bitwise_or)
            nc.vector.tensor_scalar(tmpa[:, :ny], lmb[5][:], 7, 21,
                                    A_.bitwise_and, A_.logical_shift_left)
            nc.vector.tensor_tensor(hh[:], hh[:], tmpa[:, :ny], A_.bitwise_or)
            s63 = dp.tile([np_, ny], dt.int32, tag="s63")
            nc.vector.tensor_scalar(s63[:], lmb[5][:], 3, 1,
                                    A_.logical_shift_right, A_.bitwise_and)
            s63f = dp.tile([np_, ny], dt.float32, tag="s63f")
            nc.vector.tensor_copy(s63f[:], s63[:])
            hf = dp.tile([np_, ny], dt.float32, tag="hf")
            nc.vector.tensor_copy(hf[:], hh[:])
            nc.vector.scalar_tensor_tensor(
                hf[:], s63f[:], -16777216.0, hf[:], A_.mult, A_.add)
            lf24 = dp.tile([np_, ny], dt.float32, tag="lf24")
            nc.vector.tensor_copy(lf24[:], l24[:])
            yv = dp.tile([np_, ny], dt.float32, tag="yv")
            nc.vector.scalar_tensor_tensor(
                yv[:], hf[:], 16777216.0, lf24[:], A_.mult, A_.add)
            nc.sync.dma_start(
                bass.AP(ydst, ybase, [[yrstride, np_], [1, ny]]), yv[:])

        with tc.tile_pool(name="fir", bufs=1) as dp:
            emit_comb_fir(
                dp, bass.AP(decb, 0, [[DEC_PP, P], [1, DEC_PP + CHALO]]),
                P, DEC_PP + CHALO, ybuf, 0, DEC_PP)
            f15 = dp.tile([1, L], dt.float32, tag="f15t")
            nc.sync.dma_start(f15[:], f15_in[:])
            y15 = dp.tile([1, L], dt.float32, tag="y15")
            nc.sync.dma_start(y15[:], ybuf[0, :L])
            nc.vector.tensor_copy(y15[:1, :1], y15[:1, :1])
            nc.vector.tensor_tensor(y15[:], y15[:], f15[:], A.mult)
            nc.sync.dma_start(ybuf[0, :L], y15[:1, :])
            tl = dp.tile([1, CHALO + NBH], dt.float32, tag="tl")
            nc.sync.dma_start(tl[:, :CHALO], decb[0, DEC_PC:DEC_PC + CHALO])
            nc.gpsimd.dma_start(tl[:, CHALO:], hnbp[:][pid + 1, :])
            nc.sync.dma_start(taild[:], tl[:])
            emit_comb_fir(dp, taild[:], 1, CHALO + NBH, ybuf, DEC_PC, 1)

        # ============ STFT + mel + log ============
        with (tc.tile_pool(name="stft", bufs=1) as fp,
              tc.tile_pool(name="psum", bufs=1, space="PSUM") as psp):
            wint = fp.tile([128, 4], dt.float32)
            nc.sync.dma_start(wint[:], win_in[:])
            nc.vector.tensor_copy(wint[:1, :1], wint[:1, :1])
            cosm = fp.tile([128, 4 * 257], dt.float32)
            nc.sync.dma_start(
                cosm[:].rearrange("p (k f) -> p k f", f=257),
                bass.AP(cos_in, 0, [[257, 128], [128 * 257, 4], [1, 257]]))
            nc.vector.tensor_copy(cosm[:1, :1], cosm[:1, :1])
            sinm = fp.tile([128, 4 * 257], dt.float32)
            nc.sync.dma_start(
                sinm[:].rearrange("p (k f) -> p k f", f=257),
                bass.AP(sin_in, 0, [[257, 128], [128 * 257, 4], [1, 257]]))
            nc.vector.tensor_copy(sinm[:1, :1], sinm[:1, :1])
            fbm = fp.tile([128, 2 * NMEL], dt.float32)
            nc.sync.dma_start(
                fbm[:].rearrange("p (k f) -> p k f", f=NMEL),
                bass.AP(fb_in, 0, [[NMEL, 128], [128 * NMEL, 2], [1, NMEL]]))
            nc.vector.tensor_copy(fbm[:1, :1], fbm[:1, :1])
            fbm2 = fp.tile([1, NMEL], dt.float32)
            nc.sync.dma_start(fbm2[:], bass.AP(fb_in, 256 * NMEL, [[NMEL, 1], [1, NMEL]]))
            nc.vector.tensor_copy(fbm2[:1, :1], fbm2[:1, :1])
            xts = []
            for k in range(4):
                xk = fp.tile([128, FR_PC], dt.float32, tag=f"xk{k}")
                eng = [nc.sync, nc.scalar, nc.sync, nc.scalar][k]
                eng.dma_start(
                    xk[:], bass.AP(ybuf, 128 * k, [[1, 128], [HOP, FR_PC]]))
                nc.vector.tensor_copy(xk[:1, :1], xk[:1, :1])
                nc.vector.tensor_scalar(xk[:], xk[:], wint[:, k:k + 1], None,
                                        A.mult)
                xts.append(xk)
            pw0 = fp.tile([128, FR_PC], dt.float32, tag="pw0")
            pw1 = fp.tile([128, FR_PC], dt.float32, tag="pw1")
            pw2 = fp.tile([1, FR_PC], dt.float32, tag="pw2")
            pwr = [pw0, pw1, pw2]
            fcs = [(0, 128), (128, 256), (256, 257)]
            for fi, (f0, f1) in enumerate(fcs):
                for h in range(2):
                    hs = slice(h * FH, (h + 1) * FH)
                    pc = psp.tile([f1 - f0, FH], dt.float32, tag="pc")
                    ps = psp.tile([f1 - f0, FH], dt.float32, tag="ps")
                    for k in range(4):
                        nc.tensor.matmul(
                            pc[:], cosm[:, 257 * k + f0:257 * k + f1],
                            xts[k][:, hs], start=(k == 0), stop=(k == 3))
                    for k in range(4):
                        nc.tensor.matmul(
                            ps[:], sinm[:, 257 * k + f0:257 * k + f1],
                            xts[k][:, hs], start=(k == 0), stop=(k == 3))
                    t1_ = fp.tile([128, FH], dt.float32, tag="sq1")
                    nc.scalar.activation(t1_[:f1 - f0], pc[:], ACTF.Square)
                    t2_ = fp.tile([128, FH], dt.float32, tag="sq2")
                    nc.scalar.activation(t2_[:f1 - f0], ps[:], ACTF.Square)
                    nc.vector.tensor_tensor(pwr[fi][:, hs], t1_[:f1 - f0],
                                            t2_[:f1 - f0], A.add)
            lm = fp.tile([NMEL, FR_PC], dt.float32, tag="lm")
            for h in range(2):
                hs = slice(h * FH, (h + 1) * FH)
                mm = psp.tile([NMEL, FH], dt.float32, tag="mm")
                nc.tensor.matmul(mm[:], fbm[:, :NMEL], pwr[0][:, hs],
                                 start=True, stop=False)
                nc.tensor.matmul(mm[:], fbm[:, NMEL:], pwr[1][:, hs],
                                 start=False, stop=False)
                nc.tensor.matmul(mm[:], fbm2[:, :], pwr[2][:, hs],
                                 start=False, stop=True)
                xs = fp.tile([NMEL, FH], dt.float32, tag="xs")
                nc.vector.tensor_scalar(xs[:], mm[:], 1e-6, None, A.add)
                bx = fp.tile([NMEL, FH], dt.int32, tag="bx")
                nc.vector.tensor_copy(bx[:], xs[:].bitcast(dt.int32))
                ev = fp.tile([NMEL, FH], dt.int32, tag="ev")
                nc.vector.tensor_scalar(ev[:], bx[:], 23, None,
                                        A.logical_shift_right)
                nc.vector.tensor_scalar(ev[:], ev[:], -127, None, A.add)
                evf = fp.tile([NMEL, FH], dt.float32, tag="evf")
                nc.vector.tensor_copy(evf[:], ev[:])
                nc.vector.tensor_scalar(bx[:], bx[:], 0x7FFFFF, 127 << 23,
                                        A.bitwise_and, A.bitwise_or)
                lnm = fp.tile([NMEL, FH], dt.float32, tag="lnm")
                nc.scalar.activation(lnm[:], bx[:].bitcast(dt.float32), ACTF.Ln)
                nc.vector.scalar_tensor_tensor(
                    lm[:, hs], evf[:], 0.6931471805599453, lnm[:],
                    A.mult, A.add)
            nc.sync.dma_start(out_p[:], lm[:])

    nc.compile()
    return nc


def _constants():
    mask = np.ones((128, 5136), np.float32)
    mask[:, 0::16] = 0.0
    n = np.arange(N_FFT, dtype=np.float64)
    f = np.arange(257, dtype=np.float64)
    ang = 2.0 * np.pi * n[:, None] * f[None, :] / N_FFT
    cosm = np.cos(ang).astype(np.float32)
    sinm = (-np.sin(ang)).astype(np.float32)
    fbm = _mel_fbanks_np()
    t = np.arange(WIN_LEN, dtype=np.float32)
    win = (0.5 * (1.0 - np.cos(2.0 * np.pi * t / WIN_LEN))).astype(np.float32)
    pad_l = (N_FFT - WIN_LEN) // 2
    win_p = np.zeros(N_FFT, np.float32)
    win_p[pad_l:pad_l + WIN_LEN] = win
    winm = win_p.reshape(4, 128).T.copy()
    return mask, cosm, sinm, fbm, winm


def kernel(pdm_bits, taps, scale):
    from concourse.bass_utils import run_bass_kernel_spmd

    pdm = np.asarray(pdm_bits, dtype=np.int32)
    taps_l = [int(x) for x in np.asarray(taps).tolist()]
    key = (tuple(taps_l), int(scale))
    if key not in _COMPILED:
        _COMPILED[key] = _build(taps_l, int(scale))
    nc = _COMPILED[key]

    mask, cosm, sinm, fbm, winm = _constants()
    shards = pdm.reshape(NCORE, P, FREE)
    in_maps = []
    for c in range(NCORE):
        f15 = np.ones((1, L), np.float32)
        if c == 0:
            f15[:] = 0.0
        phi = (4 * c) % 16
        r1 = (472500 * c - phi) // 16
        in_maps.append({
            "pdm": shards[c],
            "mask": mask, "cosm": cosm, "sinm": sinm, "fbm": fbm,
            "winm": winm, "f15": f15,
            "phiv": np.array([[phi]], np.int32),
            "r1v": np.array([[r1]], np.int32),
            "p16v": np.array([[16 - phi]], np.int32),
            "r1m1": np.array([[max(r1 - 1, 0)]], np.int32),
            "v0m": np.array([[1.0 if (phi == 0 and c != 0) else 0.0]],
                            np.float32),
        })
    res = run_bass_kernel_spmd(nc, in_maps, list(range(NCORE)))
    global _LAST_RES
    _LAST_RES = res
    outs = [res.results[c]["out"] for c in range(NCORE)]
    full = np.concatenate(outs, axis=1)[:, :T_FRAMES]
    return full[None, None].astype(np.float32)


# revision 32
# speedup vs baseline: 778.4207x; 1.0434x over previous
"""AudioFrontend Trainium2 kernel: PDM -> CIC(f32 blk16-exact) -> FIR(int64) -> logmel.

Bit-exact replication of jax-CPU float32 cumsum (XLA ReduceWindowRewriter
base-16 blocked scans) through the chaotic CIC stages, exact int64 FIR via
12-bit limbs on gpsimd int32, then matmul STFT/mel/log.
Self-contained: hardcodes all shapes; host code only shards/gathers.
"""
import numpy as np

NCORE = 8
N_PDM = 60_480_000
PERCORE = N_PDM // NCORE          # 7,560,000
P = 125
FREE = PERCORE // P               # 60480
TILE_F = 4032                     # 63*64 = 16*252
NT = FREE // TILE_F               # 15
ROWS_T = TILE_F // 16             # 252
ROWS_P = FREE // 16               # 3780
T0_LOC = P * ROWS_P               # 472500
T0_GLOB = NCORE * T0_LOC          # 3780000
GF = T0_GLOB // P                 # 30240
GCH = 5040
NGC = GF // GCH                   # 6
T1R = GCH // 16                   # 315
T1N = T0_GLOB // 16               # 236250
T1PAD = 236256                    # 123*1920 + 96
T2N = T1PAD // 16                 # 14766
T2PAD = 14768
T3N = T2PAD // 16                 # 923
T3PAD = 928
T4N = T3PAD // 16                 # 58
T4PAD = 64
T5N = T4PAD // 16                 # 4
DECIM = 63
DEC_PC = PERCORE // DECIM         # 120000
DEC_PP = FREE // DECIM            # 960
DEC_T = TILE_F // DECIM           # 64
L = 15
CHALO = 19
NBH = 384
N_FFT = 512
HOP = 160
WIN_LEN = 400
NMEL = 40
FR_PC = 750
T_FRAMES = 1 + (N_PDM // DECIM - N_FFT) // HOP  # 5997
FH = 375
SAT = 9.223372036854775808e18


def _mel_fbanks_np():
    n_freqs = N_FFT // 2 + 1
    all_freqs = np.linspace(0.0, 16000 / 2, n_freqs)
    h2m = lambda f: 2595.0 * np.log10(1.0 + f / 700.0)
    m_pts = np.linspace(h2m(0.0), h2m(8000.0), NMEL + 2)
    f_pts = 700.0 * (10.0 ** (m_pts / 2595.0) - 1.0)
    f_diff = f_pts[1:] - f_pts[:-1]
    slopes = f_pts[None, :] - all_freqs[:, None]
    down = -slopes[:, :-2] / f_diff[:-1]
    up = slopes[:, 2:] / f_diff[1:]
    return np.maximum(0.0, np.minimum(down, up)).astype(np.float32)


_COMPILED = {}
_LAST_RES = None


def _build(taps_list, scale_int):
    import concourse.bass as bass
    import concourse.bacc as bacc
    import concourse.mybir as mybir
    import concourse.tile as tile

    dt = mybir.dt
    A = mybir.AluOpType
    ACTF = mybir.ActivationFunctionType

    nc = bacc.Bacc()
    pdm_in = nc.declare_dram_parameter("pdm", [P, FREE], dt.int32, isOutput=False)
    mask_in = nc.declare_dram_parameter("mask", [128, 5136], dt.float32, isOutput=False)
    cos_in = nc.declare_dram_parameter("cosm", [N_FFT, 257], dt.float32, isOutput=False)
    sin_in = nc.declare_dram_parameter("sinm", [N_FFT, 257], dt.float32, isOutput=False)
    fb_in = nc.declare_dram_parameter("fbm", [257, NMEL], dt.float32, isOutput=False)
    win_in = nc.declare_dram_parameter("winm", [128, 4], dt.float32, isOutput=False)
    f15_in = nc.declare_dram_parameter("f15", [1, L], dt.float32, isOutput=False)
    phi_in = nc.declare_dram_parameter("phiv", [1, 1], dt.int32, isOutput=False)
    r1_in = nc.declare_dram_parameter("r1v", [1, 1], dt.int32, isOutput=False)
    p16_in = nc.declare_dram_parameter("p16v", [1, 1], dt.int32, isOutput=False)
    r1m_in = nc.declare_dram_parameter("r1m1", [1, 1], dt.int32, isOutput=False)
    v0m_in = nc.declare_dram_parameter("v0m", [1, 1], dt.float32, isOutput=False)
    out_p = nc.declare_dram_parameter("out", [NMEL, FR_PC], dt.float32, isOutput=True)

    pA = nc.dram_tensor("pA", [P, FREE], dt.float32)
    pB = nc.dram_tensor("pB", [P, FREE], dt.float32)
    t0loc = nc.dram_tensor("t0loc", [1, T0_LOC], dt.float32)
    e0buf = nc.dram_tensor("e0buf", [1, 16 + T0_LOC + 16], dt.float32)
    h16i = nc.dram_tensor("h16i", [1, 32], dt.float32)
    h16o = nc.dram_tensor("h16o", [NCORE, 32], dt.float32)
    h16p = nc.dram_tensor("h16p", [NCORE + 2, 32], dt.float32)
    t1agi = nc.dram_tensor("t1agi", [1, 29532], dt.float32)
    t1ago = nc.dram_tensor("t1ago", [NCORE, 29532], dt.float32)
    vloc = nc.dram_tensor("vloc", [1, 1 + 472512], dt.float32)
    t1buf = nc.dram_tensor("t1buf", [1, T1PAD], dt.float32)
    t2buf = nc.dram_tensor("t2buf", [1, T2PAD], dt.float32)
    zbuf = nc.dram_tensor("zbuf", [1, 1 + T2PAD], dt.float32)
    zsbuf = nc.dram_tensor("zsbuf", [1, 1 + T1PAD], dt.float32)
    decb = nc.dram_tensor("decb", [1, CHALO + DEC_PC], dt.float32)
    ybuf = nc.dram_tensor("ybuf", [1, DEC_PC + NBH], dt.float32)
    h19i = nc.dram_tensor("h19i", [1, CHALO], dt.float32)
    h19o = nc.dram_tensor("h19o", [NCORE, CHALO], dt.float32)
    h19p = nc.dram_tensor("h19p", [NCORE + 1, CHALO], dt.float32)
    hnbi = nc.dram_tensor("hnbi", [1, NBH], dt.float32)
    hnbo = nc.dram_tensor("hnbo", [NCORE, NBH], dt.float32)
    hnbp = nc.dram_tensor("hnbp", [NCORE + 1, NBH], dt.float32)
    taild = nc.dram_tensor("taild", [1, CHALO + NBH], dt.float32)
    t3d = nc.dram_tensor("t3d", [1, T3N], dt.float32)
    z3buf = nc.dram_tensor("z3buf", [1, 1 + T3PAD], dt.float32)

    RG = [list(range(NCORE))]
    PHIS = [(4 * c) % 16 for c in range(NCORE)]
    R1S = [(T0_LOC * c - PHIS[c]) // 16 for c in range(NCORE)]
    MCS = [(R1S[c + 1] - R1S[c]) if c + 1 < NCORE else (T1N - R1S[c])
           for c in range(NCORE)]
    taps = [int(t) for t in taps_list]
    assert (1 << 15) == int(scale_int)

    with tile.TileContext(nc) as tc:
        pid = nc.gpsimd.partition_id()

        # ============ scan stages ============
        with tc.tile_pool(name="persist", bufs=1) as pp:
            mask = pp.tile([128, 5136], dt.float32)
            nc.sync.dma_start(mask[:], mask_in[:])
            nc.vector.tensor_copy(mask[:1, :1], mask[:1, :1])
            t0sb = pp.tile([P, ROWS_P], dt.float32)
            carry0 = pp.tile([P, ROWS_P], dt.float32)
            decsb = pp.tile([P, DEC_PP], dt.float32)

            for st in range(5):
                src = [None, pA, pB, pA, pB][st]
                dst = [pA, pB, pA, pB, pA][st]
                with tc.tile_pool(name=f"s{st}", bufs=3) as sp:
                    for t in range(NT):
                        fs = slice(t * TILE_F, (t + 1) * TILE_F)
                        if st == 0:
                            raw = sp.tile([P, TILE_F], dt.int32, tag="raw")
                            nc.sync.dma_start(raw[:], pdm_in[:, fs])
                            xt = sp.tile([P, TILE_F], dt.float32, tag="xt")
                            nc.scalar.activation(xt[:], raw[:], ACTF.Copy,
                                                 bias=-1.0, scale=2.0)
                        else:
                            pin = sp.tile([P, TILE_F], dt.float32, tag="pin")
                            nc.sync.dma_start(pin[:], src[:, fs])
                            nc.vector.tensor_copy(pin[:1, :1], pin[:1, :1])
                            xt = sp.tile([P, TILE_F], dt.float32, tag="xt")
                            rs = slice(t * ROWS_T, (t + 1) * ROWS_T)
                            nc.vector.tensor_tensor(
                                xt[:].rearrange("p (r s) -> p r s", s=16),
                                pin[:].rearrange("p (r s) -> p r s", s=16),
                                carry0[:, rs].broadcast_to([P, ROWS_T, 16]),
                                A.add)
                        po = sp.tile([P, TILE_F], dt.float32, tag="po")
                        nc.vector.tensor_tensor_scan(
                            po[:], mask[:P, :TILE_F], xt[:], 0.0, A.mult, A.add)
                        nc.vector.tensor_copy(
                            t0sb[:, t * ROWS_T:(t + 1) * ROWS_T], po[:, 15::16])
                        nc.scalar.dma_start(dst[:, fs], po[:])

                nc.sync.dma_start(h16i[0, :16], t0sb[:1, :16])
                nc.sync.dma_start(h16i[0, 16:], t0sb[P - 1:P, ROWS_P - 16:])
                nc.gpsimd.collective_compute(
                    "AllGather", A.bypass, replica_groups=RG,
                    ins=[h16i[:]], outs=[h16o[:]])
                # padded halo rows: row0=AG7, rows1..8=AG0..7, row9=AG0
                nc.sync.dma_start(h16p[0, :], h16o[NCORE - 1, :])
                nc.sync.dma_start(h16p[1:NCORE + 1, :], h16o[:])
                nc.sync.dma_start(h16p[NCORE + 1, :], h16o[0, :])
                # e0: [left16 halo][own t0][right16 halo]
                nc.gpsimd.dma_start(e0buf[0, :16], h16p[:][pid, 16:])
                nc.sync.dma_start(
                    bass.AP(e0buf, 16, [[ROWS_P, P], [1, ROWS_P]]), t0sb[:])
                nc.gpsimd.dma_start(
                    e0buf[0, 16 + T0_LOC:], h16p[:][pid + 2, :16])

                with tc.tile_pool(name=f"g{st}", bufs=1) as gp:
                    phir = nc.gpsimd.alloc_register(f"phir{st}")
                    nc.gpsimd.reg_load(phir, phi_in[:1, :1])
                    r1r = nc.gpsimd.alloc_register(f"r1r{st}")
                    nc.gpsimd.reg_load(r1r, r1_in[:1, :1])
                    p16r = nc.gpsimd.alloc_register(f"p16r{st}")
                    nc.gpsimd.reg_load(p16r, p16_in[:1, :1])
                    r1mr = nc.gpsimd.alloc_register(f"r1mr{st}")
                    nc.gpsimd.reg_load(r1mr, r1m_in[:1, :1])
                    # local extended-t0 scan: [92, 5136] from e0buf
                    ge = gp.tile([92, 5136], dt.float32, tag="ge")
                    nc.gpsimd.dma_start(
                        ge[:],
                        bass.AP(e0buf, bass.make_scalar_value(p16r),
                                [[5136, 92], [1, 5136]]))
                    nc.vector.tensor_copy(ge[:1, :1], ge[:1, :1])
                    gs = gp.tile([92, 5136], dt.float32, tag="gs")
                    nc.vector.tensor_tensor_scan(
                        gs[:], mask[:92, :5136], ge[:], 0.0, A.mult, A.add)
                    tx = gp.tile([92, 321], dt.float32, tag="tx")
                    nc.vector.tensor_copy(tx[:], gs[:, 15::16])
                    nc.sync.dma_start(
                        bass.AP(t1agi, 0, [[321, 92], [1, 321]]), tx[:])
                    nc.gpsimd.collective_compute(
                        "AllGather", A.bypass, replica_groups=RG,
                        ins=[t1agi[:]], outs=[t1ago[:]])
                    # compact ragged t1 via SBUF bounce; full-width copies in
                    # forward order so each overwrites the previous overhang
                    for c in range(NCORE):
                        cb = gp.tile([12, 2461], dt.float32, tag="cb")
                        nc.sync.dma_start(
                            cb[:], bass.AP(t1ago, c * 29532,
                                           [[2461, 12], [1, 2461]]))
                        nc.sync.dma_start(
                            bass.AP(t1buf, R1S[c], [[2461, 12], [1, 2461]]),
                            cb[:])
                    zt = gp.tile([1, 16], dt.float32, tag="zt")
                    nc.vector.memset(zt[:], 0.0)
                    nc.sync.dma_start(t1buf[0, T1N:T1PAD], zt[:1, :T1PAD - T1N])
                    nc.sync.dma_start(zbuf[0, :1], zt[:1, :1])
                    nc.sync.dma_start(zsbuf[0, :1], zt[:1, :1])

                    u1a = gp.tile([123, 1920], dt.float32, tag="u1a")
                    nc.sync.dma_start(
                        u1a[:], bass.AP(t1buf, 0, [[1920, 123], [1, 1920]]))
                    nc.vector.tensor_copy(u1a[:1, :1], u1a[:1, :1])
                    p2a = gp.tile([123, 1920], dt.float32, tag="p2a")
                    nc.vector.tensor_tensor_scan(
                        p2a[:], mask[:123, :1920], u1a[:], 0.0, A.mult, A.add)
                    u1b = gp.tile([1, 96], dt.float32, tag="u1b")
                    nc.sync.dma_start(u1b[:], t1buf[0, 123 * 1920:T1PAD])
                    nc.vector.tensor_copy(u1b[:1, :1], u1b[:1, :1])
                    p2b = gp.tile([1, 96], dt.float32, tag="p2b")
                    nc.vector.tensor_tensor_scan(
                        p2b[:], mask[:1, :96], u1b[:], 0.0, A.mult, A.add)
                    t2a = gp.tile([123, 120], dt.float32, tag="t2a")
                    nc.vector.tensor_copy(t2a[:], p2a[:, 15::16])
                    nc.sync.dma_start(
                        bass.AP(t2buf, 0, [[120, 123], [1, 120]]), t2a[:])
                    t2b = gp.tile([1, 6], dt.float32, tag="t2b")
                    nc.vector.tensor_copy(t2b[:], p2b[:, 15::16])
                    nc.sync.dma_start(t2buf[0, 123 * 120:T2N], t2b[:1, :])
                    nc.sync.dma_start(t2buf[0, T2N:T2PAD], zt[:1, :T2PAD - T2N])

                    u2 = gp.tile([13, 1136], dt.float32, tag="u2")
                    nc.sync.dma_start(
                        u2[:], bass.AP(t2buf, 0, [[1136, 13], [1, 1136]]))
                    nc.vector.tensor_copy(u2[:1, :1], u2[:1, :1])
                    p3 = gp.tile([13, 1136], dt.float32, tag="p3")
                    nc.vector.tensor_tensor_scan(
                        p3[:], mask[:13, :1136], u2[:], 0.0, A.mult, A.add)
                    t3x = gp.tile([13, 71], dt.float32, tag="t3x")
                    nc.vector.tensor_copy(t3x[:], p3[:, 15::16])
                    nc.sync.dma_start(
                        bass.AP(t3d, 0, [[71, 13], [1, 71]]), t3x[:])
                    u3 = gp.tile([1, T3PAD], dt.float32, tag="u3")
                    nc.vector.memset(u3[:], 0.0)
                    nc.sync.dma_start(u3[:1, :T3N], t3d[0, :])
                    nc.vector.tensor_copy(u3[:1, :1], u3[:1, :1])
                    p4 = gp.tile([1, T3PAD], dt.float32, tag="p4")
                    nc.vector.tensor_tensor_scan(
                        p4[:], mask[:1, :T3PAD], u3[:], 0.0, A.mult, A.add)
                    u4 = gp.tile([1, T4PAD], dt.float32, tag="u4")
                    nc.vector.memset(u4[:], 0.0)
                    nc.vector.tensor_copy(u4[:, :T4N], p4[:, 15::16])
                    p5 = gp.tile([1, T4PAD], dt.float32, tag="p5")
                    nc.vector.tensor_tensor_scan(
                        p5[:], mask[:1, :T4PAD], u4[:], 0.0, A.mult, A.add)
                    u5 = gp.tile([1, T5N], dt.float32, tag="u5")
                    nc.vector.tensor_copy(u5[:], p5[:, 15::16])
                    s5 = gp.tile([1, T5N], dt.float32, tag="s5")
                    nc.vector.tensor_tensor_scan(
                        s5[:], mask[:1, :T5N], u5[:], 0.0, A.mult, A.add)
                    nc.vector.tensor_tensor(
                        p5[:, 16:].rearrange("p (r s) -> p r s", s=16),
                        p5[:, 16:].rearrange("p (r s) -> p r s", s=16),
                        s5[:, :3].broadcast_to([1, 3, 16]), A.add)
                    nc.vector.tensor_tensor(
                        p4[:, 16:].rearrange("p (r s) -> p r s", s=16),
                        p4[:, 16:].rearrange("p (r s) -> p r s", s=16),
                        p5[:, :T4N - 1].broadcast_to([1, T4N - 1, 16]), A.add)
                    nc.sync.dma_start(z3buf[0, :1], zt[:1, :1])
                    nc.sync.dma_start(z3buf[0, 1:1 + T3PAD], p4[:1, :])
                    cz3 = gp.tile([13, 71], dt.float32, tag="t3x")
                    nc.sync.dma_start(
                        cz3[:], bass.AP(z3buf, 0, [[71, 13], [1, 71]]))
                    nc.vector.tensor_copy(cz3[:1, :1], cz3[:1, :1])
                    nc.vector.tensor_tensor(
                        p3[:].rearrange("p (r s) -> p r s", s=16),
                        p3[:].rearrange("p (r s) -> p r s", s=16),
                        cz3[:].broadcast_to([13, 71, 16]), A.add)
                    nc.sync.dma_start(
                        bass.AP(zbuf, 1, [[1136, 13], [1, 1136]]), p3[:])
                    cza = gp.tile([123, 120], dt.float32, tag="cza")
                    nc.sync.dma_start(
                        cza[:], bass.AP(zbuf, 0, [[120, 123], [1, 120]]))
                    nc.vector.tensor_copy(cza[:1, :1], cza[:1, :1])
                    nc.vector.tensor_tensor(
                        p2a[:].rearrange("p (r s) -> p r s", s=16),
                        p2a[:].rearrange("p (r s) -> p r s", s=16),
                        cza[:].broadcast_to([123, 120, 16]), A.add)
                    czb = gp.tile([1, 6], dt.float32, tag="czb")
                    nc.sync.dma_start(czb[:], zbuf[0, 123 * 120:123 * 120 + 6])
                    nc.vector.tensor_copy(czb[:1, :1], czb[:1, :1])
                    nc.vector.tensor_tensor(
                        p2b[:].rearrange("p (r s) -> p r s", s=16),
                        p2b[:].rearrange("p (r s) -> p r s", s=16),
                        czb[:].broadcast_to([1, 6, 16]), A.add)
                    nc.sync.dma_start(
                        bass.AP(zsbuf, 1, [[1920, 123], [1, 1920]]), p2a[:])
                    nc.sync.dma_start(
                        zsbuf[0, 1 + 123 * 1920:1 + T1PAD], p2b[:1, :])

                    # own scan_t0: p1_local + bcast(Zs at own rows)
                    ctb = gp.tile([92, 321], dt.float32, tag="tx")
                    nc.gpsimd.dma_start(
                        ctb[:],
                        bass.AP(zsbuf, bass.make_scalar_value(r1r),
                                [[321, 92], [1, 321]]))
                    nc.vector.tensor_copy(ctb[:1, :1], ctb[:1, :1])
                    nc.vector.tensor_tensor(
                        gs[:].rearrange("p (r s) -> p r s", s=16),
                        gs[:].rearrange("p (r s) -> p r s", s=16),
                        ctb[:].broadcast_to([92, 321, 16]), A.add)
                    nc.sync.dma_start(
                        bass.AP(vloc, 1, [[5136, 92], [1, 5136]]), gs[:])
                    # vloc[0]: 0 normally; for the phi==0 mid core (c=4):
                    # scan_t0[A_c - 1] = t1[A_c/16 - 1] + scan_t1[A_c/16 - 2]
                    sv = gp.tile([1, 2], dt.float32, tag="sv")
                    nc.gpsimd.dma_start(
                        sv[:1, :1],
                        bass.AP(t1buf, bass.make_scalar_value(r1mr),
                                [[1, 1], [1, 1]]))
                    nc.gpsimd.dma_start(
                        sv[:1, 1:],
                        bass.AP(zsbuf, bass.make_scalar_value(r1mr),
                                [[1, 1], [1, 1]]))
                    v0t = gp.tile([1, 1], dt.float32, tag="v0t")
                    nc.sync.dma_start(v0t[:], v0m_in[:])
                    nc.vector.tensor_copy(v0t[:1, :1], v0t[:1, :1])
                    sv2 = gp.tile([1, 1], dt.float32, tag="sv2")
                    nc.vector.tensor_tensor(sv2[:], sv[:1, :1], sv[:1, 1:],
                                            A.add)
                    nc.vector.tensor_tensor(sv2[:], sv2[:], v0t[:], A.mult)
                    nc.sync.dma_start(vloc[0, :1], sv2[:1, :])
                    nc.gpsimd.dma_start(
                        carry0[:],
                        bass.AP(vloc, bass.make_scalar_value(phir),
                                [[ROWS_P, P], [1, ROWS_P]]))
                    nc.vector.tensor_copy(carry0[:1, :1], carry0[:1, :1])

            # ============ phase B of stage 5: decimate ============
            with tc.tile_pool(name="pb5", bufs=3) as sp:
                for t in range(NT):
                    fs = slice(t * TILE_F, (t + 1) * TILE_F)
                    pin = sp.tile([P, TILE_F], dt.float32, tag="pin")
                    nc.sync.dma_start(pin[:], pA[:, fs])
                    nc.vector.tensor_copy(pin[:1, :1], pin[:1, :1])
                    xt = sp.tile([P, TILE_F], dt.float32, tag="xt")
                    rs = slice(t * ROWS_T, (t + 1) * ROWS_T)
                    nc.vector.tensor_tensor(
                        xt[:].rearrange("p (r s) -> p r s", s=16),
                        pin[:].rearrange("p (r s) -> p r s", s=16),
                        carry0[:, rs].broadcast_to([P, ROWS_T, 16]), A.add)
                    nc.vector.tensor_copy(
                        decsb[:, t * DEC_T:(t + 1) * DEC_T], xt[:, 0::DECIM])

            nc.sync.dma_start(
                bass.AP(decb, CHALO, [[DEC_PP, P], [1, DEC_PP]]), decsb[:])
            nc.sync.dma_start(h19i[:1, :], decsb[P - 1:P, DEC_PP - CHALO:])
            nc.gpsimd.collective_compute(
                "AllGather", A.bypass, replica_groups=RG,
                ins=[h19i[:]], outs=[h19o[:]])
            nc.sync.dma_start(hnbi[:1, :], decsb[:1, :NBH])
            nc.gpsimd.collective_compute(
                "AllGather", A.bypass, replica_groups=RG,
                ins=[hnbi[:]], outs=[hnbo[:]])
            with tc.tile_pool(name="hx", bufs=1) as hp:
                zt2 = hp.tile([1, NBH], dt.float32)
                nc.vector.memset(zt2[:], 0.0)
                nc.sync.dma_start(h19p[0, :], zt2[:1, :CHALO])
                nc.sync.dma_start(h19p[1:, :], h19o[:])
                nc.sync.dma_start(hnbp[:NCORE, :], hnbo[:])
                nc.sync.dma_start(hnbp[NCORE, :], hnbo[0, :])
                nc.gpsimd.dma_start(decb[0, :CHALO], h19p[:][pid, :])

        # ============ comb + FIR + y ============
        def emit_comb_fir(dp, src_ap, np_, width, ydst, ybase, yrstride):
            A_ = A
            NL = 6  # 12-bit limbs; MAC on gpsimd int32 (exact mod 2^32)
            cmb = dp.tile([np_, width], dt.float32, tag="cmb0")
            nc.sync.dma_start(cmb[:], src_ap)
            nc.vector.tensor_copy(cmb[:1, :1], cmb[:1, :1])
            cur = cmb
            w = width
            for it in range(5):
                nxt = dp.tile([np_, w - 1], dt.float32, tag=f"cmb{1 + it % 2}")
                nc.vector.tensor_tensor(
                    nxt[:], cur[:, 1:w], cur[:, :w - 1], A_.subtract)
                cur = nxt
                w -= 1
            nw = w
            ny = nw - (L - 1)
            satp = dp.tile([np_, nw], dt.float32, tag="satp")
            nc.vector.tensor_scalar(satp[:], cur[:], SAT, None, A_.is_ge)
            satn = dp.tile([np_, nw], dt.float32, tag="satn")
            nc.vector.tensor_scalar(satn[:], cur[:], -SAT, None, A_.is_le)
            sgn = dp.tile([np_, nw], dt.float32, tag="sgn")
            nc.vector.tensor_scalar(sgn[:], cur[:], 0.0, None, A_.is_ge)
            nc.vector.tensor_scalar(sgn[:], sgn[:], 2.0, -1.0, A_.mult, A_.add)
            mag = dp.tile([np_, nw], dt.float32, tag="mag")
            nc.vector.tensor_tensor(mag[:], cur[:], sgn[:], A_.mult)
            rs_ = dp.tile([np_, nw], dt.float32, tag="rs")
            nc.vector.tensor_tensor(rs_[:], satp[:], satn[:], A_.add)
            nc.vector.tensor_scalar(rs_[:], rs_[:], -1.0, 1.0, A_.mult, A_.add)
            nc.vector.tensor_tensor(rs_[:], rs_[:], sgn[:], A_.mult)
            rsi = dp.tile([np_, nw], dt.int32, tag="rsi")
            nc.vector.tensor_copy(rsi[:], rs_[:])
            bits = dp.tile([np_, nw], dt.int32, tag="bits")
            nc.vector.tensor_copy(bits[:], mag[:].bitcast(dt.int32))
            ex = dp.tile([np_, nw], dt.int32, tag="ex")
            nc.vector.tensor_scalar(ex[:], bits[:], 23, None,
                                    A_.logical_shift_right)
            nc.vector.tensor_scalar(ex[:], ex[:], 255, None, A_.bitwise_and)
            nc.vector.tensor_scalar(ex[:], ex[:], -150, None, A_.add)
            mant = dp.tile([np_, nw], dt.int32, tag="mant")
            nc.vector.tensor_scalar(mant[:], bits[:], 0x7FFFFF, 0x800000,
                                    A_.bitwise_and, A_.bitwise_or)
            nzm = dp.tile([np_, nw], dt.int32, tag="nzm")
            nc.vector.tensor_scalar(nzm[:], ex[:], -23, None, A_.is_ge)
            nc.vector.tensor_tensor(mant[:], mant[:], nzm[:], A_.mult)
            tmpa = dp.tile([np_, nw], dt.int32, tag="tmpa")
            tmpb = dp.tile([np_, nw], dt.int32, tag="tmpb")
            tmpr = dp.tile([np_, nw], dt.int32, tag="tmpr")
            tmps = dp.tile([np_, nw], dt.int32, tag="tmps")
            sel = dp.tile([np_, nw], dt.int32, tag="sel")
            dgp = dp.tile([1, 1], dt.int32, tag="dgp")
            nc.gpsimd.tensor_copy(dgp[:], rsi[:1, :1])  # touch rsi on gpsimd
            limbs = []
            for j in range(NL):
                # r = 12j - ex; limb = r>=0 ? (mant>>min(r,31))&4095
                #                          : (mant<<min(-r,11))&4095
                nc.vector.tensor_scalar(tmpr[:], ex[:], -1, 12 * j, A_.mult,
                                        A_.add)
                nc.vector.tensor_scalar(tmps[:], tmpr[:], 31, None, A_.min)
                nc.vector.tensor_scalar(tmps[:], tmps[:], 0, None, A_.max)
                nc.vector.tensor_tensor(tmpa[:], mant[:], tmps[:],
                                        A_.logical_shift_right)
                nc.vector.tensor_scalar(tmpa[:], tmpa[:], 4095, None,
                                        A_.bitwise_and)
                nc.vector.tensor_scalar(tmps[:], tmpr[:], -1, 0, A_.mult,
                                        A_.max)
                nc.vector.tensor_scalar(tmps[:], tmps[:], 11, None, A_.min)
                nc.vector.tensor_tensor(tmpb[:], mant[:], tmps[:],
                                        A_.logical_shift_left)
                nc.vector.tensor_scalar(tmpb[:], tmpb[:], 4095, None,
                                        A_.bitwise_and)
                nc.vector.tensor_scalar(sel[:], tmpr[:], 0, None, A_.is_ge)
                lj = dp.tile([np_, nw], dt.int32, tag=f"l{j}")
                nc.vector.select(lj[:], sel[:], tmpa[:], tmpb[:])
                nc.vector.tensor_tensor(lj[:], lj[:], rsi[:], A_.mult)
                limbs.append(lj)
            accA = dp.tile([np_, ny], dt.float32, tag="accA")
            accB = dp.tile([np_, ny], dt.float32, tag="accB")
            for k in range(L):
                o = L - 1 - k
                if k == 0:
                    nc.vector.tensor_scalar(accA[:], satp[:, o:o + ny],
                                            float(taps[k]), None, A_.mult)
                    nc.vector.tensor_scalar(accB[:], satn[:, o:o + ny],
                                            float(taps[k]), None, A_.mult)
                else:
                    nc.vector.scalar_tensor_tensor(
                        accA[:], satp[:, o:o + ny], float(taps[k]), accA[:],
                        A_.mult, A_.add)
                    nc.vector.scalar_tensor_tensor(
                        accB[:], satn[:, o:o + ny], float(taps[k]), accB[:],
                        A_.mult, A_.add)
            # symmetric taps: taps[k] == taps[14-k]. Pair-sums on DVE
            # (<= 8190 so fp32-exact), products + accumulate on gpsimd int32.
            cols = []
            for j in range(NL):
                cj = dp.tile([np_, ny], dt.int32, tag=f"c{j}")
                cols.append(cj)
            tapt = dp.tile([np_, ny], dt.int32, tag="tapt")
            tmpg = dp.tile([np_, ny], dt.int32, tag="tmpg")
            pr0 = dp.tile([np_, ny], dt.int32, tag="pr0")
            pr1 = dp.tile([np_, ny], dt.int32, tag="pr1")
            prt = [pr0, pr1]
            assert all(taps[k] == taps[L - 1 - k] for k in range(L))
            for k in range(8):
                o1 = L - 1 - k
                o2 = k
                nc.gpsimd.memset(tapt[:], taps[k])
                for j in range(NL):
                    if k == 7:
                        nc.gpsimd.tensor_mul(tmpg[:],
                                             limbs[j][:, 7:7 + ny], tapt[:])
                        nc.gpsimd.tensor_add(cols[j][:], cols[j][:], tmpg[:])
                        continue
                    pr = prt[j % 2]
                    nc.vector.tensor_tensor(pr[:], limbs[j][:, o1:o1 + ny],
                                            limbs[j][:, o2:o2 + ny], A_.add)
                    if k == 0:
                        nc.gpsimd.tensor_mul(cols[j][:], pr[:], tapt[:])
                    else:
                        nc.gpsimd.tensor_mul(tmpg[:], pr[:], tapt[:])
                        nc.gpsimd.tensor_add(cols[j][:], cols[j][:], tmpg[:])
            ai = dp.tile([np_, ny], dt.int32, tag="ai")
            nc.vector.tensor_copy(ai[:], accA[:])
            bi = dp.tile([np_, ny], dt.int32, tag="bi")
            nc.vector.tensor_copy(bi[:], accB[:])
            nc.gpsimd.tensor_copy(dgp[:], ai[:1, :1])  # touch ai on gpsimd
            par = dp.tile([np_, ny], dt.int32, tag="par")
            nc.gpsimd.tensor_sub(par[:], ai[:], bi[:])
            nc.vector.tensor_scalar(par[:], par[:], 1, 3, A_.bitwise_and,
                                    A_.logical_shift_left)
            nc.gpsimd.tensor_sub(cols[0][:], cols[0][:], ai[:])
            nc.gpsimd.tensor_add(cols[5][:], cols[5][:], par[:])
            # ripple: adds on gpsimd (values < 2^31 exact), shifts/masks on DVE
            carry = dp.tile([np_, ny], dt.int32, tag="cy")
            lmb = [None] * NL
            for j in range(NL):
                if j > 0:
                    nc.gpsimd.tensor_add(cols[j][:], cols[j][:], carry[:])
                if j < NL - 1:
                    nc.vector.tensor_scalar(carry[:], cols[j][:], 12, None,
                                            A_.arith_shift_right)
                if 1 <= j <= 4:
                    mj = dp.tile([np_, ny], dt.int32, tag=f"m{j}")
                    nc.vector.tensor_scalar(mj[:], cols[j][:], 4095, None,
                                            A_.bitwise_and)
                    lmb[j] = mj
            lmb[5] = cols[5]
            # y = acc >> 15: L24 = bits 15..38, H = bits 39..62 + sign bit 63
            l24 = dp.tile([np_, ny], dt.int32, tag="l24")
            nc.vector.tensor_scalar(l24[:], lmb[1][:], 3, None,
                                    A_.logical_shift_right)
            nc.vector.tensor_scalar(tmpa[:, :ny], lmb[2][:], 9, None,
                                    A_.logical_shift_left)
            nc.vector.tensor_tensor(l24[:], l24[:], tmpa[:, :ny], A_.bitwise_or)
            nc.vector.tensor_scalar(tmpa[:, :ny], lmb[3][:], 7, 21,
                                    A_.bitwise_and, A_.logical_shift_left)
            nc.vector.tensor_tensor(l24[:], l24[:], tmpa[:, :ny], A_.bitwise_or)
            hh = dp.tile([np_, ny], dt.int32, tag="hh")
            nc.vector.tensor_scalar(hh[:], lmb[3][:], 3, None,
                                    A_.logical_shift_right)
            nc.vector.tensor_scalar(tmpa[:, :ny], lmb[4][:], 9, None,
                                    A_.logical_shift_left)
            nc.vector.tensor_tensor(hh[:], hh[:], tmpa[:, :ny], A_.bitwise_or)
            nc.vector.tensor_scalar(tmpa[:, :ny], lmb[5][:], 7, 21,
                                    A_.bitwise_and, A_.logical_shift_left)
            nc.vector.tensor_tensor(hh[:], hh[:], tmpa[:, :ny], A_.bitwise_or)
            s63 = dp.tile([np_, ny], dt.int32, tag="s63")
            nc.vector.tensor_scalar(s63[:], lmb[5][:], 3, 1,
                                    A_.logical_shift_right, A_.bitwise_and)
            s63f = dp.tile([np_, ny], dt.float32, tag="s63f")
            nc.vector.tensor_copy(s63f[:], s63[:])
            hf = dp.tile([np_, ny], dt.float32, tag="hf")
            nc.vector.tensor_copy(hf[:], hh[:])
            nc.vector.scalar_tensor_tensor(
                hf[:], s63f[:], -16777216.0, hf[:], A_.mult, A_.add)
            lf24 = dp.tile([np_, ny], dt.float32, tag="lf24")
            nc.vector.tensor_copy(lf24[:], l24[:])
            yv = dp.tile([np_, ny], dt.float32, tag="yv")
            nc.vector.scalar_tensor_tensor(
                yv[:], hf[:], 16777216.0, lf24[:], A_.mult, A_.add)
            nc.sync.dma_start(
                bass.AP(ydst, ybase, [[yrstride, np_], [1, ny]]), yv[:])

        with tc.tile_pool(name="fir", bufs=1) as dp:
            emit_comb_fir(
                dp, bass.AP(decb, 0, [[DEC_PP, P], [1, DEC_PP + CHALO]]),
                P, DEC_PP + CHALO, ybuf, 0, DEC_PP)
            f15 = dp.tile([1, L], dt.float32, tag="f15t")
            nc.sync.dma_start(f15[:], f15_in[:])
            y15 = dp.tile([1, L], dt.float32, tag="y15")
            nc.sync.dma_start(y15[:], ybuf[0, :L])
            nc.vector.tensor_copy(y15[:1, :1], y15[:1, :1])
            nc.vector.tensor_tensor(y15[:], y15[:], f15[:], A.mult)
            nc.sync.dma_start(ybuf[0, :L], y15[:1, :])
            tl = dp.tile([1, CHALO + NBH], dt.float32, tag="tl")
            nc.sync.dma_start(tl[:, :CHALO], decb[0, DEC_PC:DEC_PC + CHALO])
            nc.gpsimd.dma_start(tl[:, CHALO:], hnbp[:][pid + 1, :])
            nc.sync.dma_start(taild[:], tl[:])
            emit_comb_fir(dp, taild[:], 1, CHALO + NBH, ybuf, DEC_PC, 1)

        # ============ STFT + mel + log ============
        with (tc.tile_pool(name="stft", bufs=1) as fp,
              tc.tile_pool(name="psum", bufs=1, space="PSUM") as psp):
            wint = fp.tile([128, 4], dt.float32)
            nc.sync.dma_start(wint[:], win_in[:])
            nc.vector.tensor_copy(wint[:1, :1], wint[:1, :1])
            cosm = fp.tile([128, 4 * 257], dt.float32)
            nc.sync.dma_start(
                cosm[:].rearrange("p (k f) -> p k f", f=257),
                bass.AP(cos_in, 0, [[257, 128], [128 * 257, 4], [1, 257]]))
            nc.vector.tensor_copy(cosm[:1, :1], cosm[:1, :1])
            sinm = fp.tile([128, 4 * 257], dt.float32)
            nc.sync.dma_start(
                sinm[:].rearrange("p (k f) -> p k f", f=257),
                bass.AP(sin_in, 0, [[257, 128], [128 * 257, 4], [1, 257]]))
            nc.vector.tensor_copy(sinm[:1, :1], sinm[:1, :1])
            fbm = fp.tile([128, 2 * NMEL], dt.float32)
            nc.sync.dma_start(
                fbm[:].rearrange("p (k f) -> p k f", f=NMEL),
                bass.AP(fb_in, 0, [[NMEL, 128], [128 * NMEL, 2], [1, NMEL]]))
            nc.vector.tensor_copy(fbm[:1, :1], fbm[:1, :1])
            fbm2 = fp.tile([1, NMEL], dt.float32)
            nc.sync.dma_start(fbm2[:], bass.AP(fb_in, 256 * NMEL, [[NMEL, 1], [1, NMEL]]))
            nc.vector.tensor_copy(fbm2[:1, :1], fbm2[:1, :1])
            xts = []
            for k in range(4):
                xk = fp.tile([128, FR_PC], dt.float32, tag=f"xk{k}")
                eng = [nc.sync, nc.scalar, nc.sync, nc.scalar][k]
                eng.dma_start(
                    xk[:], bass.AP(ybuf, 128 * k, [[1, 128], [HOP, FR_PC]]))
                nc.vector.tensor_copy(xk[:1, :1], xk[:1, :1])
                nc.vector.tensor_scalar(xk[:], xk[:], wint[:, k:k + 1], None,
                                        A.mult)
                xts.append(xk)
            pw0 = fp.tile([128, FR_PC], dt.float32, tag="pw0")
            pw1 = fp.tile([128, FR_PC], dt.float32, tag="pw1")
            pw2 = fp.tile([1, FR_PC], dt.float32, tag="pw2")
            pwr = [pw0, pw1, pw2]
            fcs = [(0, 128), (128, 256), (256, 257)]
            for fi, (f0, f1) in enumerate(fcs):
                for h in range(2):
                    hs = slice(h * FH, (h + 1) * FH)
                    pc = psp.tile([f1 - f0, FH], dt.float32, tag="pc")
                    ps = psp.tile([f1 - f0, FH], dt.float32, tag="ps")
                    for k in range(4):
                        nc.tensor.matmul(
                            pc[:], cosm[:, 257 * k + f0:257 * k + f1],
                            xts[k][:, hs], start=(k == 0), stop=(k == 3))
                    for k in range(4):
                        nc.tensor.matmul(
                            ps[:], sinm[:, 257 * k + f0:257 * k + f1],
                            xts[k][:, hs], start=(k == 0), stop=(k == 3))
                    t1_ = fp.tile([128, FH], dt.float32, tag="sq1")
                    nc.scalar.activation(t1_[:f1 - f0], pc[:], ACTF.Square)
                    t2_ = fp.tile([128, FH], dt.float32, tag="sq2")
                    nc.scalar.activation(t2_[:f1 - f0], ps[:], ACTF.Square)
                    nc.vector.tensor_tensor(pwr[fi][:, hs], t1_[:f1 - f0],
                                            t2_[:f1 - f0], A.add)
            lm = fp.tile([NMEL, FR_PC], dt.float32, tag="lm")
            for h in range(2):
                hs = slice(h * FH, (h + 1) * FH)
                mm = psp.tile([NMEL, FH], dt.float32, tag="mm")
                nc.tensor.matmul(mm[:], fbm[:, :NMEL], pwr[0][:, hs],
                                 start=True, stop=False)
                nc.tensor.matmul(mm[:], fbm[:, NMEL:], pwr[1][:, hs],
                                 start=False, stop=False)
                nc.tensor.matmul(mm[:], fbm2[:, :], pwr[2][:, hs],
                                 start=False, stop=True)
                xs = fp.tile([NMEL, FH], dt.float32, tag="xs")
                nc.vector.tensor_scalar(xs[:], mm[:], 1e-6, None, A.add)
                bx = fp.tile([NMEL, FH], dt.int32, tag="bx")
                nc.vector.tensor_copy(bx[:], xs[:].bitcast(dt.int32))
                ev = fp.tile([NMEL, FH], dt.int32, tag="ev")
                nc.vector.tensor_scalar(ev[:], bx[:], 23, None,
                                        A.logical_shift_right)
                nc.vector.tensor_scalar(ev[:], ev[:], -127, None, A.add)
                evf = fp.tile([NMEL, FH], dt.float32, tag="evf")
                nc.vector.tensor_copy(evf[:], ev[:])
                nc.vector.tensor_scalar(bx[:], bx[:], 0x7FFFFF, 127 << 23,
                                        A.bitwise_and, A.bitwise_or)
                lnm = fp.tile([NMEL, FH], dt.float32, tag="lnm")
                nc.scalar.activation(lnm[:], bx[:].bitcast(dt.float32), ACTF.Ln)
                nc.vector.scalar_tensor_tensor(
                    lm[:, hs], evf[:], 0.6931471805599453, lnm[:],
                    A.mult, A.add)
            nc.sync.dma_start(out_p[:], lm[:])

    nc.compile()
    return nc


def _constants():
    mask = np.ones((128, 5136), np.float32)
    mask[:, 0::16] = 0.0
    n = np.arange(N_FFT, dtype=np.float64)
    f = np.arange(257, dtype=np.float64)
    ang = 2.0 * np.pi * n[:, None] * f[None, :] / N_FFT
    cosm = np.cos(ang).astype(np.float32)
    sinm = (-np.sin(ang)).astype(np.float32)
    fbm = _mel_fbanks_np()
    t = np.arange(WIN_LEN, dtype=np.float32)
    win = (0.5 * (1.0 - np.cos(2.0 * np.pi * t / WIN_LEN))).astype(np.float32)
    pad_l = (N_FFT - WIN_LEN) // 2
    win_p = np.zeros(N_FFT, np.float32)
    win_p[pad_l:pad_l + WIN_LEN] = win
    winm = win_p.reshape(4, 128).T.copy()
    return mask, cosm, sinm, fbm, winm


def kernel(pdm_bits, taps, scale):
    from concourse.bass_utils import run_bass_kernel_spmd

    pdm = np.asarray(pdm_bits, dtype=np.int32)
    taps_l = [int(x) for x in np.asarray(taps).tolist()]
    key = (tuple(taps_l), int(scale))
    if key not in _COMPILED:
        _COMPILED[key] = _build(taps_l, int(scale))
    nc = _COMPILED[key]

    mask, cosm, sinm, fbm, winm = _constants()
    shards = pdm.reshape(NCORE, P, FREE)
    in_maps = []
    for c in range(NCORE):
        f15 = np.ones((1, L), np.float32)
        if c == 0:
            f15[:] = 0.0
        phi = (4 * c) % 16
        r1 = (472500 * c - phi) // 16
        in_maps.append({
            "pdm": shards[c],
            "mask": mask, "cosm": cosm, "sinm": sinm, "fbm": fbm,
            "winm": winm, "f15": f15,
            "phiv": np.array([[phi]], np.int32),
            "r1v": np.array([[r1]], np.int32),
            "p16v": np.array([[16 - phi]], np.int32),
            "r1m1": np.array([[max(r1 - 1, 0)]], np.int32),
            "v0m": np.array([[1.0 if (phi == 0 and c != 0) else 0.0]],
                            np.float32),
        })
    res = run_bass_kernel_spmd(nc, in_maps, list(range(NCORE)))
    global _LAST_RES
    _LAST_RES = res
    outs = [res.results[c]["out"] for c in range(NCORE)]
    full = np.concatenate(outs, axis=1)[:, :T_FRAMES]
    return full[None, None].astype(np.float32)


# revision 35
# speedup vs baseline: 782.9122x; 1.0058x over previous
"""AudioFrontend Trainium2 kernel: PDM -> CIC(f32 blk16-exact) -> FIR(int64) -> logmel.

Bit-exact replication of jax-CPU float32 cumsum (XLA ReduceWindowRewriter
base-16 blocked scans) through the chaotic CIC stages, exact int64 FIR via
12-bit limbs on gpsimd int32, then matmul STFT/mel/log.
Self-contained: hardcodes all shapes; host code only shards/gathers.
"""
import numpy as np

NCORE = 8
N_PDM = 60_480_000
PERCORE = N_PDM // NCORE          # 7,560,000
P = 125
FREE = PERCORE // P               # 60480
TILE_F = 4032                     # 63*64 = 16*252
NT = FREE // TILE_F               # 15
ROWS_T = TILE_F // 16             # 252
ROWS_P = FREE // 16               # 3780
T0_LOC = P * ROWS_P               # 472500
T0_GLOB = NCORE * T0_LOC          # 3780000
GF = T0_GLOB // P                 # 30240
GCH = 5040
NGC = GF // GCH                   # 6
T1R = GCH // 16                   # 315
T1N = T0_GLOB // 16               # 236250
T1PAD = 236256                    # 123*1920 + 96
T2N = T1PAD // 16                 # 14766
T2PAD = 14768
T3N = T2PAD // 16                 # 923
T3PAD = 928
T4N = T3PAD // 16                 # 58
T4PAD = 64
T5N = T4PAD // 16                 # 4
DECIM = 63
DEC_PC = PERCORE // DECIM         # 120000
DEC_PP = FREE // DECIM            # 960
DEC_T = TILE_F // DECIM           # 64
L = 15
CHALO = 19
NBH = 384
N_FFT = 512
HOP = 160
WIN_LEN = 400
NMEL = 40
FR_PC = 750
T_FRAMES = 1 + (N_PDM // DECIM - N_FFT) // HOP  # 5997
FH = 375
SAT = 9.223372036854775808e18


def _mel_fbanks_np():
    n_freqs = N_FFT // 2 + 1
    all_freqs = np.linspace(0.0, 16000 / 2, n_freqs)
    h2m = lambda f: 2595.0 * np.log10(1.0 + f / 700.0)
    m_pts = np.linspace(h2m(0.0), h2m(8000.0), NMEL + 2)
    f_pts = 700.0 * (10.0 ** (m_pts / 2595.0) - 1.0)
    f_diff = f_pts[1:] - f_pts[:-1]
    slopes = f_pts[None, :] - all_freqs[:, None]
    down = -slopes[:, :-2] / f_diff[:-1]
    up = slopes[:, 2:] / f_diff[1:]
    return np.maximum(0.0, np.minimum(down, up)).astype(np.float32)


_COMPILED = {}
_LAST_RES = None


def _build(taps_list, scale_int):
    import concourse.bass as bass
    import concourse.bacc as bacc
    import concourse.mybir as mybir
    import concourse.tile as tile

    dt = mybir.dt
    A = mybir.AluOpType
    ACTF = mybir.ActivationFunctionType

    nc = bacc.Bacc()
    pdm_in = nc.declare_dram_parameter("pdm", [P, FREE], dt.int32, isOutput=False)
    mask_in = nc.declare_dram_parameter("mask", [128, 5136], dt.float32, isOutput=False)
    cos_in = nc.declare_dram_parameter("cosm", [N_FFT, 257], dt.float32, isOutput=False)
    sin_in = nc.declare_dram_parameter("sinm", [N_FFT, 257], dt.float32, isOutput=False)
    fb_in = nc.declare_dram_parameter("fbm", [257, NMEL], dt.float32, isOutput=False)
    win_in = nc.declare_dram_parameter("winm", [128, 4], dt.float32, isOutput=False)
    f15_in = nc.declare_dram_parameter("f15", [1, L], dt.float32, isOutput=False)
    phi_in = nc.declare_dram_parameter("phiv", [1, 1], dt.int32, isOutput=False)
    r1_in = nc.declare_dram_parameter("r1v", [1, 1], dt.int32, isOutput=False)
    p16_in = nc.declare_dram_parameter("p16v", [1, 1], dt.int32, isOutput=False)
    r1m_in = nc.declare_dram_parameter("r1m1", [1, 1], dt.int32, isOutput=False)
    v0m_in = nc.declare_dram_parameter("v0m", [1, 1], dt.float32, isOutput=False)
    out_p = nc.declare_dram_parameter("out", [NMEL, FR_PC], dt.float32, isOutput=True)

    pA = nc.dram_tensor("pA", [P, FREE], dt.float32)
    pB = nc.dram_tensor("pB", [P, FREE], dt.float32)
    t0loc = nc.dram_tensor("t0loc", [1, T0_LOC], dt.float32)
    e0buf = nc.dram_tensor("e0buf", [1, 16 + T0_LOC + 16], dt.float32)
    h16i = nc.dram_tensor("h16i", [1, 32], dt.float32)
    h16o = nc.dram_tensor("h16o", [NCORE, 32], dt.float32)
    h16p = nc.dram_tensor("h16p", [NCORE + 2, 32], dt.float32)
    t1agi = nc.dram_tensor("t1agi", [1, 29532], dt.float32)
    t1ago = nc.dram_tensor("t1ago", [NCORE, 29532], dt.float32)
    vloc = nc.dram_tensor("vloc", [1, 1 + 472512], dt.float32)
    t1buf = nc.dram_tensor("t1buf", [1, T1PAD], dt.float32)
    t2buf = nc.dram_tensor("t2buf", [1, T2PAD], dt.float32)
    zbuf = nc.dram_tensor("zbuf", [1, 1 + T2PAD], dt.float32)
    zsbuf = nc.dram_tensor("zsbuf", [1, 1 + T1PAD], dt.float32)
    decb = nc.dram_tensor("decb", [1, CHALO + DEC_PC], dt.float32)
    ybuf = nc.dram_tensor("ybuf", [1, DEC_PC + NBH], dt.float32)
    h19i = nc.dram_tensor("h19i", [1, CHALO], dt.float32)
    h19o = nc.dram_tensor("h19o", [NCORE, CHALO], dt.float32)
    h19p = nc.dram_tensor("h19p", [NCORE + 1, CHALO], dt.float32)
    hnbi = nc.dram_tensor("hnbi", [1, NBH], dt.float32)
    hnbo = nc.dram_tensor("hnbo", [NCORE, NBH], dt.float32)
    hnbp = nc.dram_tensor("hnbp", [NCORE + 1, NBH], dt.float32)
    taild = nc.dram_tensor("taild", [1, CHALO + NBH], dt.float32)
    t3d = nc.dram_tensor("t3d", [1, T3N], dt.float32)
    z3buf = nc.dram_tensor("z3buf", [1, 1 + T3PAD], dt.float32)

    RG = [list(range(NCORE))]
    PHIS = [(4 * c) % 16 for c in range(NCORE)]
    R1S = [(T0_LOC * c - PHIS[c]) // 16 for c in range(NCORE)]
    MCS = [(R1S[c + 1] - R1S[c]) if c + 1 < NCORE else (T1N - R1S[c])
           for c in range(NCORE)]
    taps = [int(t) for t in taps_list]
    assert (1 << 15) == int(scale_int)

    with tile.TileContext(nc) as tc:
        pid = nc.gpsimd.partition_id()

        # ============ scan stages ============
        with tc.tile_pool(name="persist", bufs=1) as pp:
            mask = pp.tile([128, 5136], dt.float32)
            nc.sync.dma_start(mask[:], mask_in[:])
            nc.vector.tensor_copy(mask[:1, :1], mask[:1, :1])
            t0sb = pp.tile([P, ROWS_P], dt.float32)
            carry0 = pp.tile([P, ROWS_P], dt.float32)
            decsb = pp.tile([P, DEC_PP], dt.float32)

            for st in range(5):
                src = [None, pA, pB, pA, pB][st]
                dst = [pA, pB, pA, pB, pA][st]
                with tc.tile_pool(name=f"s{st}", bufs=3) as sp:
                    for t in range(NT):
                        fs = slice(t * TILE_F, (t + 1) * TILE_F)
                        if st == 0:
                            raw = sp.tile([P, TILE_F], dt.int32, tag="raw")
                            nc.sync.dma_start(raw[:], pdm_in[:, fs])
                            xt = sp.tile([P, TILE_F], dt.float32, tag="xt")
                            nc.scalar.activation(xt[:], raw[:], ACTF.Copy,
                                                 bias=-1.0, scale=2.0)
                        else:
                            pin = sp.tile([P, TILE_F], dt.float32, tag="pin")
                            nc.sync.dma_start(pin[:], src[:, fs])
                            nc.vector.tensor_copy(pin[:1, :1], pin[:1, :1])
                            xt = sp.tile([P, TILE_F], dt.float32, tag="xt")
                            rs = slice(t * ROWS_T, (t + 1) * ROWS_T)
                            nc.vector.tensor_tensor(
                                xt[:].rearrange("p (r s) -> p r s", s=16),
                                pin[:].rearrange("p (r s) -> p r s", s=16),
                                carry0[:, rs].broadcast_to([P, ROWS_T, 16]),
                                A.add)
                        po = sp.tile([P, TILE_F], dt.float32, tag="po")
                        nc.vector.tensor_tensor_scan(
                            po[:], mask[:P, :TILE_F], xt[:], 0.0, A.mult, A.add)
                        nc.vector.tensor_copy(
                            t0sb[:, t * ROWS_T:(t + 1) * ROWS_T], po[:, 15::16])
                        nc.scalar.dma_start(dst[:, fs], po[:])

                nc.sync.dma_start(h16i[0, :16], t0sb[:1, :16])
                nc.sync.dma_start(h16i[0, 16:], t0sb[P - 1:P, ROWS_P - 16:])
                nc.gpsimd.collective_compute(
                    "AllGather", A.bypass, replica_groups=RG,
                    ins=[h16i[:]], outs=[h16o[:]])
                # padded halo rows: row0=AG7, rows1..8=AG0..7, row9=AG0
                nc.sync.dma_start(h16p[0, :], h16o[NCORE - 1, :])
                nc.sync.dma_start(h16p[1:NCORE + 1, :], h16o[:])
                nc.sync.dma_start(h16p[NCORE + 1, :], h16o[0, :])
                # e0: [left16 halo][own t0][right16 halo]
                nc.gpsimd.dma_start(e0buf[0, :16], h16p[:][pid, 16:])
                nc.sync.dma_start(
                    bass.AP(e0buf, 16, [[ROWS_P, P], [1, ROWS_P]]), t0sb[:])
                nc.gpsimd.dma_start(
                    e0buf[0, 16 + T0_LOC:], h16p[:][pid + 2, :16])

                with tc.tile_pool(name=f"g{st}", bufs=1) as gp:
                    phir = nc.gpsimd.alloc_register(f"phir{st}")
                    nc.gpsimd.reg_load(phir, phi_in[:1, :1])
                    r1r = nc.gpsimd.alloc_register(f"r1r{st}")
                    nc.gpsimd.reg_load(r1r, r1_in[:1, :1])
                    p16r = nc.gpsimd.alloc_register(f"p16r{st}")
                    nc.gpsimd.reg_load(p16r, p16_in[:1, :1])
                    r1mr = nc.gpsimd.alloc_register(f"r1mr{st}")
                    nc.gpsimd.reg_load(r1mr, r1m_in[:1, :1])
                    # local extended-t0 scan: [92, 5136] from e0buf
                    ge = gp.tile([92, 5136], dt.float32, tag="ge")
                    nc.gpsimd.dma_start(
                        ge[:],
                        bass.AP(e0buf, bass.make_scalar_value(p16r),
                                [[5136, 92], [1, 5136]]))
                    nc.vector.tensor_copy(ge[:1, :1], ge[:1, :1])
                    gs = gp.tile([92, 5136], dt.float32, tag="gs")
                    nc.vector.tensor_tensor_scan(
                        gs[:], mask[:92, :5136], ge[:], 0.0, A.mult, A.add)
                    tx = gp.tile([92, 321], dt.float32, tag="tx")
                    nc.vector.tensor_copy(tx[:], gs[:, 15::16])
                    nc.sync.dma_start(
                        bass.AP(t1agi, 0, [[321, 92], [1, 321]]), tx[:])
                    nc.gpsimd.collective_compute(
                        "AllGather", A.bypass, replica_groups=RG,
                        ins=[t1agi[:]], outs=[t1ago[:]])
                    # compact ragged t1 via SBUF bounce; full-width copies in
                    # forward order so each overwrites the previous overhang
                    for c in range(NCORE):
                        cb = gp.tile([12, 2461], dt.float32, tag="cb")
                        nc.sync.dma_start(
                            cb[:], bass.AP(t1ago, c * 29532,
                                           [[2461, 12], [1, 2461]]))
                        nc.sync.dma_start(
                            bass.AP(t1buf, R1S[c], [[2461, 12], [1, 2461]]),
                            cb[:])
                    zt = gp.tile([1, 16], dt.float32, tag="zt")
                    nc.vector.memset(zt[:], 0.0)
                    nc.sync.dma_start(t1buf[0, T1N:T1PAD], zt[:1, :T1PAD - T1N])
                    nc.sync.dma_start(zbuf[0, :1], zt[:1, :1])
                    nc.sync.dma_start(zsbuf[0, :1], zt[:1, :1])

                    u1a = gp.tile([123, 1920], dt.float32, tag="u1a")
                    nc.sync.dma_start(
                        u1a[:], bass.AP(t1buf, 0, [[1920, 123], [1, 1920]]))
                    nc.vector.tensor_copy(u1a[:1, :1], u1a[:1, :1])
                    p2a = gp.tile([123, 1920], dt.float32, tag="p2a")
                    nc.vector.tensor_tensor_scan(
                        p2a[:], mask[:123, :1920], u1a[:], 0.0, A.mult, A.add)
                    u1b = gp.tile([1, 96], dt.float32, tag="u1b")
                    nc.sync.dma_start(u1b[:], t1buf[0, 123 * 1920:T1PAD])
                    nc.vector.tensor_copy(u1b[:1, :1], u1b[:1, :1])
                    p2b = gp.tile([1, 96], dt.float32, tag="p2b")
                    nc.vector.tensor_tensor_scan(
                        p2b[:], mask[:1, :96], u1b[:], 0.0, A.mult, A.add)
                    t2a = gp.tile([123, 120], dt.float32, tag="t2a")
                    nc.vector.tensor_copy(t2a[:], p2a[:, 15::16])
                    nc.sync.dma_start(
                        bass.AP(t2buf, 0, [[120, 123], [1, 120]]), t2a[:])
                    t2b = gp.tile([1, 6], dt.float32, tag="t2b")
                    nc.vector.tensor_copy(t2b[:], p2b[:, 15::16])
                    nc.sync.dma_start(t2buf[0, 123 * 120:T2N], t2b[:1, :])
                    nc.sync.dma_start(t2buf[0, T2N:T2PAD], zt[:1, :T2PAD - T2N])

                    u2 = gp.tile([13, 1136], dt.float32, tag="u2")
                    nc.sync.dma_start(
                        u2[:], bass.AP(t2buf, 0, [[1136, 13], [1, 1136]]))
                    nc.vector.tensor_copy(u2[:1, :1], u2[:1, :1])
                    p3 = gp.tile([13, 1136], dt.float32, tag="p3")
                    nc.vector.tensor_tensor_scan(
                        p3[:], mask[:13, :1136], u2[:], 0.0, A.mult, A.add)
                    t3x = gp.tile([13, 71], dt.float32, tag="t3x")
                    nc.vector.tensor_copy(t3x[:], p3[:, 15::16])
                    nc.sync.dma_start(
                        bass.AP(t3d, 0, [[71, 13], [1, 71]]), t3x[:])
                    u3 = gp.tile([1, T3PAD], dt.float32, tag="u3")
                    nc.vector.memset(u3[:], 0.0)
                    nc.sync.dma_start(u3[:1, :T3N], t3d[0, :])
                    nc.vector.tensor_copy(u3[:1, :1], u3[:1, :1])
                    p4 = gp.tile([1, T3PAD], dt.float32, tag="p4")
                    nc.vector.tensor_tensor_scan(
                        p4[:], mask[:1, :T3PAD], u3[:], 0.0, A.mult, A.add)
                    u4 = gp.tile([1, T4PAD], dt.float32, tag="u4")
                    nc.vector.memset(u4[:], 0.0)
                    nc.vector.tensor_copy(u4[:, :T4N], p4[:, 15::16])
                    p5 = gp.tile([1, T4PAD], dt.float32, tag="p5")
                    nc.vector.tensor_tensor_scan(
                        p5[:], mask[:1, :T4PAD], u4[:], 0.0, A.mult, A.add)
                    u5 = gp.tile([1, T5N], dt.float32, tag="u5")
                    nc.vector.tensor_copy(u5[:], p5[:, 15::16])
                    s5 = gp.tile([1, T5N], dt.float32, tag="s5")
                    nc.vector.tensor_tensor_scan(
                        s5[:], mask[:1, :T5N], u5[:], 0.0, A.mult, A.add)
                    nc.vector.tensor_tensor(
                        p5[:, 16:].rearrange("p (r s) -> p r s", s=16),
                        p5[:, 16:].rearrange("p (r s) -> p r s", s=16),
                        s5[:, :3].broadcast_to([1, 3, 16]), A.add)
                    nc.vector.tensor_tensor(
                        p4[:, 16:].rearrange("p (r s) -> p r s", s=16),
                        p4[:, 16:].rearrange("p (r s) -> p r s", s=16),
                        p5[:, :T4N - 1].broadcast_to([1, T4N - 1, 16]), A.add)
                    nc.sync.dma_start(z3buf[0, :1], zt[:1, :1])
                    nc.sync.dma_start(z3buf[0, 1:1 + T3PAD], p4[:1, :])
                    cz3 = gp.tile([13, 71], dt.float32, tag="t3x")
                    nc.sync.dma_start(
                        cz3[:], bass.AP(z3buf, 0, [[71, 13], [1, 71]]))
                    nc.vector.tensor_copy(cz3[:1, :1], cz3[:1, :1])
                    nc.vector.tensor_tensor(
                        p3[:].rearrange("p (r s) -> p r s", s=16),
                        p3[:].rearrange("p (r s) -> p r s", s=16),
                        cz3[:].broadcast_to([13, 71, 16]), A.add)
                    nc.sync.dma_start(
                        bass.AP(zbuf, 1, [[1136, 13], [1, 1136]]), p3[:])
                    cza = gp.tile([123, 120], dt.float32, tag="cza")
                    nc.sync.dma_start(
                        cza[:], bass.AP(zbuf, 0, [[120, 123], [1, 120]]))
                    nc.vector.tensor_copy(cza[:1, :1], cza[:1, :1])
                    nc.vector.tensor_tensor(
                        p2a[:].rearrange("p (r s) -> p r s", s=16),
                        p2a[:].rearrange("p (r s) -> p r s", s=16),
                        cza[:].broadcast_to([123, 120, 16]), A.add)
                    czb = gp.tile([1, 6], dt.float32, tag="czb")
                    nc.sync.dma_start(czb[:], zbuf[0, 123 * 120:123 * 120 + 6])
                    nc.vector.tensor_copy(czb[:1, :1], czb[:1, :1])
                    nc.vector.tensor_tensor(
                        p2b[:].rearrange("p (r s) -> p r s", s=16),
                        p2b[:].rearrange("p (r s) -> p r s", s=16),
                        czb[:].broadcast_to([1, 6, 16]), A.add)
                    nc.sync.dma_start(
                        bass.AP(zsbuf, 1, [[1920, 123], [1, 1920]]), p2a[:])
                    nc.sync.dma_start(
                        zsbuf[0, 1 + 123 * 1920:1 + T1PAD], p2b[:1, :])

                    # own scan_t0: p1_local + bcast(Zs at own rows)
                    ctb = gp.tile([92, 321], dt.float32, tag="tx")
                    nc.gpsimd.dma_start(
                        ctb[:],
                        bass.AP(zsbuf, bass.make_scalar_value(r1r),
                                [[321, 92], [1, 321]]))
                    nc.vector.tensor_copy(ctb[:1, :1], ctb[:1, :1])
                    nc.vector.tensor_tensor(
                        gs[:].rearrange("p (r s) -> p r s", s=16),
                        gs[:].rearrange("p (r s) -> p r s", s=16),
                        ctb[:].broadcast_to([92, 321, 16]), A.add)
                    nc.sync.dma_start(
                        bass.AP(vloc, 1, [[5136, 92], [1, 5136]]), gs[:])
                    # vloc[0]: 0 normally; for the phi==0 mid core (c=4):
                    # scan_t0[A_c - 1] = t1[A_c/16 - 1] + scan_t1[A_c/16 - 2]
                    sv = gp.tile([1, 2], dt.float32, tag="sv")
                    nc.gpsimd.dma_start(
                        sv[:1, :1],
                        bass.AP(t1buf, bass.make_scalar_value(r1mr),
                                [[1, 1], [1, 1]]))
                    nc.gpsimd.dma_start(
                        sv[:1, 1:],
                        bass.AP(zsbuf, bass.make_scalar_value(r1mr),
                                [[1, 1], [1, 1]]))
                    v0t = gp.tile([1, 1], dt.float32, tag="v0t")
                    nc.sync.dma_start(v0t[:], v0m_in[:])
                    nc.vector.tensor_copy(v0t[:1, :1], v0t[:1, :1])
                    sv2 = gp.tile([1, 1], dt.float32, tag="sv2")
                    nc.vector.tensor_tensor(sv2[:], sv[:1, :1], sv[:1, 1:],
                                            A.add)
                    nc.vector.tensor_tensor(sv2[:], sv2[:], v0t[:], A.mult)
                    nc.sync.dma_start(vloc[0, :1], sv2[:1, :])
                    nc.gpsimd.dma_start(
                        carry0[:],
                        bass.AP(vloc, bass.make_scalar_value(phir),
                                [[ROWS_P, P], [1, ROWS_P]]))
                    nc.vector.tensor_copy(carry0[:1, :1], carry0[:1, :1])

            # ============ phase B of stage 5: decimate ============
            with tc.tile_pool(name="pb5", bufs=3) as sp:
                for t in range(NT):
                    fs = slice(t * TILE_F, (t + 1) * TILE_F)
                    pin = sp.tile([P, TILE_F], dt.float32, tag="pin")
                    nc.sync.dma_start(pin[:], pA[:, fs])
                    nc.vector.tensor_copy(pin[:1, :1], pin[:1, :1])
                    xt = sp.tile([P, TILE_F], dt.float32, tag="xt")
                    rs = slice(t * ROWS_T, (t + 1) * ROWS_T)
                    nc.vector.tensor_tensor(
                        xt[:].rearrange("p (r s) -> p r s", s=16),
                        pin[:].rearrange("p (r s) -> p r s", s=16),
                        carry0[:, rs].broadcast_to([P, ROWS_T, 16]), A.add)
                    nc.vector.tensor_copy(
                        decsb[:, t * DEC_T:(t + 1) * DEC_T], xt[:, 0::DECIM])

            nc.sync.dma_start(
                bass.AP(decb, CHALO, [[DEC_PP, P], [1, DEC_PP]]), decsb[:])
            nc.sync.dma_start(h19i[:1, :], decsb[P - 1:P, DEC_PP - CHALO:])
            nc.gpsimd.collective_compute(
                "AllGather", A.bypass, replica_groups=RG,
                ins=[h19i[:]], outs=[h19o[:]])
            nc.sync.dma_start(hnbi[:1, :], decsb[:1, :NBH])
            nc.gpsimd.collective_compute(
                "AllGather", A.bypass, replica_groups=RG,
                ins=[hnbi[:]], outs=[hnbo[:]])
            with tc.tile_pool(name="hx", bufs=1) as hp:
                zt2 = hp.tile([1, NBH], dt.float32)
                nc.vector.memset(zt2[:], 0.0)
                nc.sync.dma_start(h19p[0, :], zt2[:1, :CHALO])
                nc.sync.dma_start(h19p[1:, :], h19o[:])
                nc.sync.dma_start(hnbp[:NCORE, :], hnbo[:])
                nc.sync.dma_start(hnbp[NCORE, :], hnbo[0, :])
                nc.gpsimd.dma_start(decb[0, :CHALO], h19p[:][pid, :])

        # ============ comb + FIR + y ============
        def emit_comb_fir(dp, src_ap, np_, width, ydst, ybase, yrstride):
            A_ = A
            NL = 6  # 12-bit limbs; MAC on gpsimd int32 (exact mod 2^32)
            cmb = dp.tile([np_, width], dt.float32, tag="cmb0")
            nc.sync.dma_start(cmb[:], src_ap)
            nc.vector.tensor_copy(cmb[:1, :1], cmb[:1, :1])
            cur = cmb
            w = width
            for it in range(5):
                nxt = dp.tile([np_, w - 1], dt.float32, tag=f"cmb{1 + it % 2}")
                nc.vector.tensor_tensor(
                    nxt[:], cur[:, 1:w], cur[:, :w - 1], A_.subtract)
                cur = nxt
                w -= 1
            nw = w
            ny = nw - (L - 1)
            satp = dp.tile([np_, nw], dt.float32, tag="satp")
            nc.vector.tensor_scalar(satp[:], cur[:], SAT, None, A_.is_ge)
            satn = dp.tile([np_, nw], dt.float32, tag="satn")
            nc.vector.tensor_scalar(satn[:], cur[:], -SAT, None, A_.is_le)
            sgn = dp.tile([np_, nw], dt.float32, tag="sgn")
            nc.vector.tensor_scalar(sgn[:], cur[:], 0.0, None, A_.is_ge)
            nc.vector.tensor_scalar(sgn[:], sgn[:], 2.0, -1.0, A_.mult, A_.add)
            mag = dp.tile([np_, nw], dt.float32, tag="mag")
            nc.vector.tensor_tensor(mag[:], cur[:], sgn[:], A_.mult)
            rs_ = dp.tile([np_, nw], dt.float32, tag="rs")
            nc.vector.tensor_tensor(rs_[:], satp[:], satn[:], A_.add)
            nc.vector.tensor_scalar(rs_[:], rs_[:], -1.0, 1.0, A_.mult, A_.add)
            nc.vector.tensor_tensor(rs_[:], rs_[:], sgn[:], A_.mult)
            rsi = dp.tile([np_, nw], dt.int32, tag="rsi")
            nc.vector.tensor_copy(rsi[:], rs_[:])
            bits = dp.tile([np_, nw], dt.int32, tag="bits")
            nc.vector.tensor_copy(bits[:], mag[:].bitcast(dt.int32))
            ex = dp.tile([np_, nw], dt.int32, tag="ex")
            nc.vector.tensor_scalar(ex[:], bits[:], 23, None,
                                    A_.logical_shift_right)
            nc.vector.tensor_scalar(ex[:], ex[:], 255, None, A_.bitwise_and)
            nc.vector.tensor_scalar(ex[:], ex[:], -150, None, A_.add)
            mant = dp.tile([np_, nw], dt.int32, tag="mant")
            nc.vector.tensor_scalar(mant[:], bits[:], 0x7FFFFF, 0x800000,
                                    A_.bitwise_and, A_.bitwise_or)
            nzm = dp.tile([np_, nw], dt.int32, tag="nzm")
            nc.vector.tensor_scalar(nzm[:], ex[:], -23, None, A_.is_ge)
            nc.vector.tensor_tensor(mant[:], mant[:], nzm[:], A_.mult)
            tmpa = dp.tile([np_, nw], dt.int32, tag="tmpa")
            tmpb = dp.tile([np_, nw], dt.int32, tag="tmpb")
            tmpr = dp.tile([np_, nw], dt.int32, tag="tmpr")
            tmps = dp.tile([np_, nw], dt.int32, tag="tmps")
            sel = dp.tile([np_, nw], dt.int32, tag="sel")
            dgp = dp.tile([1, 1], dt.int32, tag="dgp")
            nc.gpsimd.tensor_copy(dgp[:], rsi[:1, :1])  # touch rsi on gpsimd
            limbs = []
            for j in range(NL):
                # r = 12j - ex; limb = r>=0 ? (mant>>min(r,31))&4095
                #                          : (mant<<min(-r,11))&4095
                nc.vector.tensor_scalar(tmpr[:], ex[:], -1, 12 * j, A_.mult,
                                        A_.add)
                nc.vector.tensor_scalar(tmps[:], tmpr[:], 31, None, A_.min)
                nc.vector.tensor_scalar(tmps[:], tmps[:], 0, None, A_.max)
                nc.vector.tensor_tensor(tmpa[:], mant[:], tmps[:],
                                        A_.logical_shift_right)
                nc.vector.tensor_scalar(tmpa[:], tmpa[:], 4095, None,
                                        A_.bitwise_and)
                nc.vector.tensor_scalar(tmps[:], tmpr[:], -1, 0, A_.mult,
                                        A_.max)
                nc.vector.tensor_scalar(tmps[:], tmps[:], 11, None, A_.min)
                nc.vector.tensor_tensor(tmpb[:], mant[:], tmps[:],
                                        A_.logical_shift_left)
                nc.vector.tensor_scalar(tmpb[:], tmpb[:], 4095, None,
                                        A_.bitwise_and)
                nc.vector.tensor_scalar(sel[:], tmpr[:], 0, None, A_.is_ge)
                lj = dp.tile([np_, nw], dt.int32, tag=f"l{j}")
                nc.vector.select(lj[:], sel[:], tmpa[:], tmpb[:])
                nc.vector.tensor_tensor(lj[:], lj[:], rsi[:], A_.mult)
                limbs.append(lj)
            accA = dp.tile([np_, ny], dt.float32, tag="accA")
            accB = dp.tile([np_, ny], dt.float32, tag="accB")
            for k in range(L):
                o = L - 1 - k
                if k == 0:
                    nc.vector.tensor_scalar(accA[:], satp[:, o:o + ny],
                                            float(taps[k]), None, A_.mult)
                    nc.vector.tensor_scalar(accB[:], satn[:, o:o + ny],
                                            float(taps[k]), None, A_.mult)
                else:
                    nc.vector.scalar_tensor_tensor(
                        accA[:], satp[:, o:o + ny], float(taps[k]), accA[:],
                        A_.mult, A_.add)
                    nc.vector.scalar_tensor_tensor(
                        accB[:], satn[:, o:o + ny], float(taps[k]), accB[:],
                        A_.mult, A_.add)
            # symmetric taps: taps[k] == taps[14-k]. Pair-sums on DVE
            # (<= 8190 so fp32-exact), products + accumulate on gpsimd int32.
            cols = []
            for j in range(NL):
                cj = dp.tile([np_, ny], dt.int32, tag=f"c{j}")
                cols.append(cj)
            tapt = dp.tile([np_, ny], dt.int32, tag="tapt")
            tmpg = dp.tile([np_, ny], dt.int32, tag="tmpg")
            pr0 = dp.tile([np_, ny], dt.int32, tag="pr0")
            pr1 = dp.tile([np_, ny], dt.int32, tag="pr1")
            prt = [pr0, pr1]
            assert all(taps[k] == taps[L - 1 - k] for k in range(L))
            for k in range(8):
                o1 = L - 1 - k
                o2 = k
                nc.gpsimd.memset(tapt[:], taps[k])
                for j in range(NL):
                    if k == 7:
                        nc.gpsimd.tensor_mul(tmpg[:],
                                             limbs[j][:, 7:7 + ny], tapt[:])
                        nc.gpsimd.tensor_add(cols[j][:], cols[j][:], tmpg[:])
                        continue
                    pr = prt[j % 2]
                    nc.vector.tensor_tensor(pr[:], limbs[j][:, o1:o1 + ny],
                                            limbs[j][:, o2:o2 + ny], A_.add)
                    if k == 0:
                        nc.gpsimd.tensor_mul(cols[j][:], pr[:], tapt[:])
                    else:
                        nc.gpsimd.tensor_mul(tmpg[:], pr[:], tapt[:])
                        nc.gpsimd.tensor_add(cols[j][:], cols[j][:], tmpg[:])
            ai = dp.tile([np_, ny], dt.int32, tag="ai")
            nc.vector.tensor_copy(ai[:], accA[:])
            bi = dp.tile([np_, ny], dt.int32, tag="bi")
            nc.vector.tensor_copy(bi[:], accB[:])
            nc.gpsimd.tensor_copy(dgp[:], ai[:1, :1])  # touch ai on gpsimd
            par = dp.tile([np_, ny], dt.int32, tag="par")
            nc.gpsimd.tensor_sub(par[:], ai[:], bi[:])
            nc.vector.tensor_scalar(par[:], par[:], 1, 3, A_.bitwise_and,
                                    A_.logical_shift_left)
            nc.gpsimd.tensor_sub(cols[0][:], cols[0][:], ai[:])
            nc.gpsimd.tensor_add(cols[5][:], cols[5][:], par[:])
            # ripple: adds on gpsimd (values < 2^31 exact), shifts/masks on DVE
            carry = dp.tile([np_, ny], dt.int32, tag="cy")
            lmb = [None] * NL
            for j in range(NL):
                if j > 0:
                    nc.gpsimd.tensor_add(cols[j][:], cols[j][:], carry[:])
                if j < NL - 1:
                    nc.vector.tensor_scalar(carry[:], cols[j][:], 12, None,
                                            A_.arith_shift_right)
                if 1 <= j <= 4:
                    mj = dp.tile([np_, ny], dt.int32, tag=f"m{j}")
                    nc.vector.tensor_scalar(mj[:], cols[j][:], 4095, None,
                                            A_.bitwise_and)
                    lmb[j] = mj
            lmb[5] = cols[5]
            # y = acc >> 15: L24 = bits 15..38, H = bits 39..62 + sign bit 63
            l24 = dp.tile([np_, ny], dt.int32, tag="l24")
            nc.vector.tensor_scalar(l24[:], lmb[1][:], 3, None,
                                    A_.logical_shift_right)
            nc.vector.tensor_scalar(tmpa[:, :ny], lmb[2][:], 9, None,
                                    A_.logical_shift_left)
            nc.vector.tensor_tensor(l24[:], l24[:], tmpa[:, :ny], A_.bitwise_or)
            nc.vector.tensor_scalar(tmpa[:, :ny], lmb[3][:], 7, 21,
                                    A_.bitwise_and, A_.logical_shift_left)
            nc.vector.tensor_tensor(l24[:], l24[:], tmpa[:, :ny], A_.bitwise_or)
            hh = dp.tile([np_, ny], dt.int32, tag="hh")
            nc.vector.tensor_scalar(hh[:], lmb[3][:], 3, None,
                                    A_.logical_shift_right)
            nc.vector.tensor_scalar(tmpa[:, :ny], lmb[4][:], 9, None,
                                    A_.logical_shift_left)
            nc.vector.tensor_tensor(hh[:], hh[:], tmpa[:, :ny], A_.bitwise_or)
            nc.vector.tensor_scalar(tmpa[:, :ny], lmb[5][:], 7, 21,
                                    A_.bitwise_and, A_.logical_shift_left)
            nc.vector.tensor_tensor(hh[:], hh[:], tmpa[:, :ny], A_.bitwise_or)
            s63 = dp.tile([np_, ny], dt.int32, tag="s63")
            nc.vector.tensor_scalar(s63[:], lmb[5][:], 3, 1,
                                    A_.logical_shift_right, A_.bitwise_and)
            s63f = dp.tile([np_, ny], dt.float32, tag="s63f")
            nc.vector.tensor_copy(s63f[:], s63[:])
            hf = dp.tile([np_, ny], dt.float32, tag="hf")
            nc.vector.tensor_copy(hf[:], hh[:])
            nc.vector.scalar_tensor_tensor(
                hf[:], s63f[:], -16777216.0, hf[:], A_.mult, A_.add)
            lf24 = dp.tile([np_, ny], dt.float32, tag="lf24")
            nc.vector.tensor_copy(lf24[:], l24[:])
            yv = dp.tile([np_, ny], dt.float32, tag="yv")
            nc.vector.scalar_tensor_tensor(
                yv[:], hf[:], 16777216.0, lf24[:], A_.mult, A_.add)
            nc.sync.dma_start(
                bass.AP(ydst, ybase, [[yrstride, np_], [1, ny]]), yv[:])

        with tc.tile_pool(name="fir", bufs=1) as dp:
            emit_comb_fir(
                dp, bass.AP(decb, 0, [[DEC_PP, P], [1, DEC_PP + CHALO]]),
                P, DEC_PP + CHALO, ybuf, 0, DEC_PP)
            f15 = dp.tile([1, L], dt.float32, tag="f15t")
            nc.sync.dma_start(f15[:], f15_in[:])
            y15 = dp.tile([1, L], dt.float32, tag="y15")
            nc.sync.dma_start(y15[:], ybuf[0, :L])
            nc.vector.tensor_copy(y15[:1, :1], y15[:1, :1])
            nc.vector.tensor_tensor(y15[:], y15[:], f15[:], A.mult)
            nc.sync.dma_start(ybuf[0, :L], y15[:1, :])
            tl = dp.tile([1, CHALO + NBH], dt.float32, tag="tl")
            nc.sync.dma_start(tl[:, :CHALO], decb[0, DEC_PC:DEC_PC + CHALO])
            nc.gpsimd.dma_start(tl[:, CHALO:], hnbp[:][pid + 1, :])
            nc.sync.dma_start(taild[:], tl[:])
            emit_comb_fir(dp, taild[:], 1, CHALO + NBH, ybuf, DEC_PC, 1)

        # ============ STFT + mel + log ============
        with (tc.tile_pool(name="stft", bufs=1) as fp,
              tc.tile_pool(name="psum", bufs=1, space="PSUM") as psp):
            wint = fp.tile([128, 4], dt.float32)
            nc.sync.dma_start(wint[:], win_in[:])
            nc.vector.tensor_copy(wint[:1, :1], wint[:1, :1])
            cosm = fp.tile([128, 4 * 257], dt.float32)
            nc.sync.dma_start(
                cosm[:].rearrange("p (k f) -> p k f", f=257),
                bass.AP(cos_in, 0, [[257, 128], [128 * 257, 4], [1, 257]]))
            nc.vector.tensor_copy(cosm[:1, :1], cosm[:1, :1])
            sinm = fp.tile([128, 4 * 257], dt.float32)
            nc.sync.dma_start(
                sinm[:].rearrange("p (k f) -> p k f", f=257),
                bass.AP(sin_in, 0, [[257, 128], [128 * 257, 4], [1, 257]]))
            nc.vector.tensor_copy(sinm[:1, :1], sinm[:1, :1])
            fbm = fp.tile([128, 2 * NMEL], dt.float32)
            nc.sync.dma_start(
                fbm[:].rearrange("p (k f) -> p k f", f=NMEL),
                bass.AP(fb_in, 0, [[NMEL, 128], [128 * NMEL, 2], [1, NMEL]]))
            nc.vector.tensor_copy(fbm[:1, :1], fbm[:1, :1])
            fbm2 = fp.tile([1, NMEL], dt.float32)
            nc.sync.dma_start(fbm2[:], bass.AP(fb_in, 256 * NMEL, [[NMEL, 1], [1, NMEL]]))
            nc.vector.tensor_copy(fbm2[:1, :1], fbm2[:1, :1])
            xts = []
            for k in range(4):
                xk = fp.tile([128, FR_PC], dt.float32, tag=f"xk{k}")
                eng = [nc.sync, nc.scalar, nc.sync, nc.scalar][k]
                eng.dma_start(
                    xk[:], bass.AP(ybuf, 128 * k, [[1, 128], [HOP, FR_PC]]))
                nc.vector.tensor_copy(xk[:1, :1], xk[:1, :1])
                nc.vector.tensor_scalar(xk[:], xk[:], wint[:, k:k + 1], None,
                                        A.mult)
                xts.append(xk)
            pw0 = fp.tile([128, FR_PC], dt.float32, tag="pw0")
            pw1 = fp.tile([128, FR_PC], dt.float32, tag="pw1")
            pw2 = fp.tile([1, FR_PC], dt.float32, tag="pw2")
            pwr = [pw0, pw1, pw2]
            fcs = [(0, 128), (128, 256), (256, 257)]
            for fi, (f0, f1) in enumerate(fcs):
                for h in range(2):
                    hs = slice(h * FH, (h + 1) * FH)
                    pc = psp.tile([f1 - f0, FH], dt.float32, tag="pc")
                    ps = psp.tile([f1 - f0, FH], dt.float32, tag="ps")
                    for k in range(4):
                        nc.tensor.matmul(
                            pc[:], cosm[:, 257 * k + f0:257 * k + f1],
                            xts[k][:, hs], start=(k == 0), stop=(k == 3))
                    for k in range(4):
                        nc.tensor.matmul(
                            ps[:], sinm[:, 257 * k + f0:257 * k + f1],
                            xts[k][:, hs], start=(k == 0), stop=(k == 3))
                    t1_ = fp.tile([128, FH], dt.float32, tag="sq1")
                    nc.scalar.activation(t1_[:f1 - f0], pc[:], ACTF.Square)
                    t2_ = fp.tile([128, FH], dt.float32, tag="sq2")
                    nc.scalar.activation(t2_[:f1 - f0], ps[:], ACTF.Square)
                    nc.vector.tensor_tensor(pwr[fi][:, hs], t1_[:f1 - f0],
                                            t2_[:f1 - f0], A.add)
            lm = fp.tile([NMEL, FR_PC], dt.float32, tag="lm")
            for h in range(2):
                hs = slice(h * FH, (h + 1) * FH)
                mm = psp.tile([NMEL, FH], dt.float32, tag="mm")
                nc.tensor.matmul(mm[:], fbm[:, :NMEL], pwr[0][:, hs],
                                 start=True, stop=False)
                nc.tensor.matmul(mm[:], fbm[:, NMEL:], pwr[1][:, hs],
                                 start=False, stop=False)
                nc.tensor.matmul(mm[:], fbm2[:, :], pwr[2][:, hs],
                                 start=False, stop=True)
                xs = fp.tile([NMEL, FH], dt.float32, tag="xs")
                nc.vector.tensor_scalar(xs[:], mm[:], 1e-6, None, A.add)
                bx = fp.tile([NMEL, FH], dt.int32, tag="bx")
                nc.vector.tensor_copy(bx[:], xs[:].bitcast(dt.int32))
                ev = fp.tile([NMEL, FH], dt.int32, tag="ev")
                nc.vector.tensor_scalar(ev[:], bx[:], 23, None,
                                        A.logical_shift_right)
                nc.vector.tensor_scalar(ev[:], ev[:], -127, None, A.add)
                evf = fp.tile([NMEL, FH], dt.float32, tag="evf")
                nc.vector.tensor_copy(evf[:], ev[:])
                nc.vector.tensor_scalar(bx[:], bx[:], 0x7FFFFF, 127 << 23,
                                        A.bitwise_and, A.bitwise_or)
                lnm = fp.tile([NMEL, FH], dt.float32, tag="lnm")
                nc.scalar.activation(lnm[:], bx[:].bitcast(dt.float32), ACTF.Ln)
                nc.vector.scalar_tensor_tensor(
                    lm[:, hs], evf[:], 0.6931471805599453, lnm[:],
                    A.mult, A.add)
            nc.sync.dma_start(out_p[:], lm[:])

    nc.compile()
    return nc


def _constants():
    mask = np.ones((128, 5136), np.float32)
    mask[:, 0::16] = 0.0
    n = np.arange(N_FFT, dtype=np.float64)
    f = np.arange(257, dtype=np.float64)
    ang = 2.0 * np.pi * n[:, None] * f[None, :] / N_FFT
    cosm = np.cos(ang).astype(np.float32)
    sinm = (-np.sin(ang)).astype(np.float32)
    fbm = _mel_fbanks_np()
    t = np.arange(WIN_LEN, dtype=np.float32)
    win = (0.5 * (1.0 - np.cos(2.0 * np.pi * t / WIN_LEN))).astype(np.float32)
    pad_l = (N_FFT - WIN_LEN) // 2
    win_p = np.zeros(N_FFT, np.float32)
    win_p[pad_l:pad_l + WIN_LEN] = win
    winm = win_p.reshape(4, 128).T.copy()
    return mask, cosm, sinm, fbm, winm


def kernel(pdm_bits, taps, scale):
    from concourse.bass_utils import run_bass_kernel_spmd

    pdm = np.asarray(pdm_bits, dtype=np.int32)
    taps_l = [int(x) for x in np.asarray(taps).tolist()]
    key = (tuple(taps_l), int(scale))
    if key not in _COMPILED:
        _COMPILED[key] = _build(taps_l, int(scale))
    nc = _COMPILED[key]

    mask, cosm, sinm, fbm, winm = _constants()
    shards = pdm.reshape(NCORE, P, FREE)
    in_maps = []
    for c in range(NCORE):
        f15 = np.ones((1, L), np.float32)
        if c == 0:
            f15[:] = 0.0
        phi = (4 * c) % 16
        r1 = (472500 * c - phi) // 16
        in_maps.append({
            "pdm": shards[c],
            "mask": mask, "cosm": cosm, "sinm": sinm, "fbm": fbm,
            "winm": winm, "f15": f15,
            "phiv": np.array([[phi]], np.int32),
            "r1v": np.array([[r1]], np.int32),
            "p16v": np.array([[16 - phi]], np.int32),
            "r1m1": np.array([[max(r1 - 1, 0)]], np.int32),
            "v0m": np.array([[1.0 if (phi == 0 and c != 0) else 0.0]],
                            np.float32),
        })
    res = run_bass_kernel_spmd(nc, in_maps, list(range(NCORE)))
    global _LAST_RES
    _LAST_RES = res
    outs = [res.results[c]["out"] for c in range(NCORE)]
    full = np.concatenate(outs, axis=1)[:, :T_FRAMES]
    return full[None, None].astype(np.float32)
